# revision 4
# baseline (speedup 1.0000x reference)
"""Trainium2 Bass kernel for CustomRoPEAttention (B=2, S=2048, H=16, Dh=128).

Sharding: 8 cores = 2 batches x 4 head-groups (4 heads/core).

Head-pipelined structure: per head h, QKV^T projection (fp8 hi/lo DoubleRow
matmuls) + RoPE, then transposed-layout causal attention for that head while
later heads' projections stream -- this overlaps the ACT-engine exp work with
PE-engine matmul work across the whole kernel instead of serializing phases.

fp8 DoubleRow "dup trick": scores use stationary (k_hi,k_lo) pairs against a
broadcast (step-0) fp8 q moving operand, and the output projection uses
(wo_hi,wo_lo) pairs against broadcast ct_hi plus a wo_hi x ct_lo correction --
half / 0.75x the bf16 PE time at first-order-exact precision.

Host sums the 4 partial (transposed) output projections per batch.

Self-contained: hardcodes shapes from the problem spec.
"""
import math
from contextlib import ExitStack

import numpy as np
import ml_dtypes

import concourse.mybir as mybir
import concourse.tile as tile
from concourse import bacc
from concourse.bass_utils import run_bass_kernel_spmd
from concourse.masks import make_identity

S = 2048            # sequence
D = 2048            # hidden
NH = 16             # total heads
DH = 128            # head dim
HG = 4              # heads per core
GQ = HG * DH        # 512: per-core q/k/v feature width
B = 2
NCORES = 8
ROPE_THETA = 10000.0
SCALE = 1.0 / math.sqrt(DH)
SLAB = 512          # qkv sequence slab width
XSC = 16.0          # fp8 pre-scale for x
WSC = 512.0         # fp8 pre-scale for Wqkv / Wo
QSC = 16.0          # fp8 pre-scale for roped q/k (folded into cos/sin tables)
CSC = 16.0          # fp8 pre-scale for attention-out ct (folded into tones)
INV_SC = 1.0 / (XSC * WSC)
EXP_SCALE = SCALE / (QSC * QSC)
P3_SCALE = 1.0 / (WSC * CSC)   # applied host-side
F32 = mybir.dt.float32
BF16 = mybir.dt.bfloat16
F8 = mybir.dt.float8e4
MULT = mybir.AluOpType.mult
ADD = mybir.AluOpType.add
SUB = mybir.AluOpType.subtract
DR = mybir.MatmulPerfMode.DoubleRow
NB = S // 128       # 16 k/q blocks
IDENT = mybir.ActivationFunctionType.Identity
EXPF = mybir.ActivationFunctionType.Exp


def build_nc(reps=1, knobs=None):
    kn = {"sps": 2, "mmp": 3, "expb": 1, "wqb": 2, "qkb": 2, "stg": 2, "obp": 4}
    if knobs:
        kn.update(knobs)
    nc = bacc.Bacc(None, target_bir_lowering=False)
    # x^T hi/lo, slab-major pack: [p, ns, kc2, i, s]
    xh = nc.dram_tensor("xh", [128, 4, 8, 2, SLAB], F8, kind="ExternalInput")
    xl = nc.dram_tensor("xl", [128, 4, 8, 2, SLAB], F8, kind="ExternalInput")
    # per-mt packed qk weights: [mt, p, kc2, i, m]
    wqkh = nc.dram_tensor("wqkh", [8, 128, 8, 2, 128], F8, kind="ExternalInput")
    wqkl = nc.dram_tensor("wqkl", [8, 128, 8, 2, 128], F8, kind="ExternalInput")
    wvh = nc.dram_tensor("wvh", [8, 128, 2, GQ], F8, kind="ExternalInput")
    wvl = nc.dram_tensor("wvl", [8, 128, 2, GQ], F8, kind="ExternalInput")
    # out-proj fp8 packs: wo1[kh] = (hi,lo) pairs; wo2[g] = hi head-pair packs
    wo1 = nc.dram_tensor("wo1", [HG, 128, 2, D], F8, kind="ExternalInput")
    wo2 = nc.dram_tensor("wo2", [2, 128, 2, D], F8, kind="ExternalInput")
    bqkt = nc.dram_tensor("bqkt", [128, 8], F32, kind="ExternalInput")
    bv = nc.dram_tensor("bv", [1, GQ], F32, kind="ExternalInput")
    cost = nc.dram_tensor("cost", [128, S], BF16, kind="ExternalInput")    # cos^T * QSC
    sinrt = nc.dram_tensor("sinrt", [128, S], BF16, kind="ExternalInput")  # sin^T * QSC, rot sign
    maskd = nc.dram_tensor("maskd", [128, 128], BF16, kind="ExternalInput")  # triu 0/1 keep-mask
    tonesd = nc.dram_tensor("tonesd", [128, 1], BF16, kind="ExternalInput")  # 1/CSC
    outt = nc.dram_tensor("outt", [16, 128, S], BF16, kind="ExternalOutput")
    lrt = nc.dram_tensor("lrt", [HG, 1, 16, 128], F32)  # recip bounce: [16,128] -> [1,2048]

    with tile.TileContext(nc) as tc, ExitStack() as top:
        g = top.enter_context(tc.tile_pool(name="glob", bufs=1))
        tcos = g.tile([128, S], BF16)
        tsin = g.tile([128, S], BF16)
        tmask = g.tile([128, 128], BF16)
        ident_f = g.tile([128, 128], F32)
        make_identity(nc, ident_f[:])
        tbqkt = g.tile([128, 8], F32)
        tbvb = g.tile([128, GQ], F32)
        tones = g.tile([128, 1], BF16)
        tinv = g.tile([128, 1], F32)
        nc.vector.memset(tinv[:], INV_SC)

        # Whole-kernel residents
        res = top.enter_context(tc.tile_pool(name="res", bufs=1))
        vres = []  # 16 V k-block tiles [128(seq), GQ] bf16
        for t in range(NB):
            vres.append(res.tile([128, GQ], BF16, tag=f"v{t}", name=f"v{t}"))
        cth = {}
        for h in range(HG):
            for gq in range(4):
                cth[(h, gq)] = res.tile([128, 512], F8, tag=f"cth_{h}_{gq}",
                                        name=f"cth_{h}_{gq}")
        ctl = [res.tile([128, HG, 512], F8, tag=f"ctl{gq}", name=f"ctl{gq}")
               for gq in range(4)]

        for _rep in range(reps):
          phB = ExitStack()
          mmp = phB.enter_context(tc.tile_pool(name="mmp", bufs=kn["mmp"], space="PSUM"))
          sps = phB.enter_context(tc.tile_pool(name="sps", bufs=kn["sps"], space="PSUM"))
          smps = phB.enter_context(tc.tile_pool(name="smps", bufs=1, space="PSUM"))
          expp = phB.enter_context(tc.tile_pool(name="expp", bufs=kn["expb"]))
          lrp = phB.enter_context(tc.tile_pool(name="lrp", bufs=2))
          rbp = phB.enter_context(tc.tile_pool(name="rbp", bufs=2))
          ctsp = phB.enter_context(tc.tile_pool(name="ctsp", bufs=2))
          smt = smps.tile([128, 132], F32, tag="sm", name="smt")

          phA = ExitStack()
          xp = phA.enter_context(tc.tile_pool(name="xp", bufs=1))
          wqp = phA.enter_context(tc.tile_pool(name="wqp", bufs=kn["wqb"]))
          qkp = phA.enter_context(tc.tile_pool(name="qkp", bufs=kn["qkb"]))
          stg = phA.enter_context(tc.tile_pool(name="stg", bufs=kn["stg"]))
          sec0 = ExitStack()
          xsl = sec0.enter_context(tc.tile_pool(name="xsl", bufs=3))
          wvp = sec0.enter_context(tc.tile_pool(name="wvp", bufs=1))

          # ---- initial DMA order (startup-critical) ----
          wq_tiles = {}  # (h) -> (wqh, wql, wkh, wkl)

          def weights_dma(h):
              tl = []
              for mt, tag in ((h, "wqh"), (h, "wql"), (4 + h, "wkh"), (4 + h, "wkl")):
                  src = wqkh if tag.endswith("h") else wqkl
                  wt = wqp.tile([128, 8, 2, 128], F8, tag=tag, name=f"{tag}{h}")
                  nc.sync.dma_start(out=wt, in_=src[mt])
                  tl.append(wt)
              wq_tiles[h] = tl

          # head-0 q weights + first x slab first
          wt = wqp.tile([128, 8, 2, 128], F8, tag="wqh", name="wqh0")
          nc.sync.dma_start(out=wt, in_=wqkh[0])
          xres = [xp.tile([128, 8, 2, SLAB], F8, tag=f"x{ns}", name=f"x{ns}")
                  for ns in range(4)]
          nc.sync.dma_start(out=xres[0], in_=xh[:, 0])
          wt2 = wqp.tile([128, 8, 2, 128], F8, tag="wkh", name="wkh0")
          nc.sync.dma_start(out=wt2, in_=wqkh[4])
          wt3 = wqp.tile([128, 8, 2, 128], F8, tag="wql", name="wql0")
          nc.sync.dma_start(out=wt3, in_=wqkl[0])
          wt4 = wqp.tile([128, 8, 2, 128], F8, tag="wkl", name="wkl0")
          nc.sync.dma_start(out=wt4, in_=wqkl[4])
          wq_tiles[0] = [wt, wt3, wt2, wt4]
          # small consts needed by first psum copies
          nc.sync.dma_start(out=tbqkt, in_=bqkt[:])
          nc.sync.dma_start(out=tones, in_=tonesd[:])
          nc.sync.dma_start(out=tmask, in_=maskd[:])
          nc.sync.dma_start(out=xres[1], in_=xh[:, 1])
          nc.sync.dma_start(out=tcos, in_=cost[:])
          nc.sync.dma_start(out=tsin, in_=sinrt[:])
          nc.sync.dma_start(out=xres[2], in_=xh[:, 2])
          nc.sync.dma_start(out=tbvb, in_=bv[:].to_broadcast((128, GQ)))
          twvh, twvl = [], []
          for kc2 in range(8):
              wv_t = wvp.tile([128, 2, GQ], F8, tag=f"wvh{kc2}")
              nc.sync.dma_start(out=wv_t, in_=wvh[kc2])
              twvh.append(wv_t)
          nc.sync.dma_start(out=xres[3], in_=xh[:, 3])
          for kc2 in range(8):
              wv_t = wvp.tile([128, 2, GQ], F8, tag=f"wvl{kc2}")
              nc.sync.dma_start(out=wv_t, in_=wvl[kc2])
              twvl.append(wv_t)
          xlres = {}

          def xl_dma(ns):
              xt = xsl.tile([128, 8, 2, SLAB], F8, tag="xl", name=f"xl{ns}")
              nc.sync.dma_start(out=xt, in_=xl[:, ns])
              xlres[ns] = xt

          xl_dma(0)
          xl_dma(1)

          # ---- per-head state ----
          qf8 = {}
          kpair = {}
          expT = {}
          lrec_cur = {}
          recrow = {}

          def chain_qk(h, which, ns):
              wqh_, wql_, wkh_, wkl_ = wq_tiles[h]
              whi, wlo = (wqh_, wql_) if which == "q" else (wkh_, wkl_)
              sl = slice(ns * SLAB, (ns + 1) * SLAB)
              ps = mmp.tile([128, SLAB], F32, tag="mm")
              for pi, wt_ in enumerate((whi, wlo)):
                  for kc2 in range(8):
                      nc.tensor.matmul(ps[:], wt_[:, kc2, :, :], xres[ns][:, kc2, :, :],
                                       start=(pi == 0 and kc2 == 0),
                                       stop=(pi == 1 and kc2 == 7), perf_mode=DR)
              st = stg.tile([128, SLAB], BF16, tag="st")
              mt = h if which == "q" else 4 + h
              nc.scalar.activation(out=st[:], in_=ps[:], func=IDENT,
                                   scale=INV_SC, bias=tbqkt[:, mt:mt + 1])
              # RoPE: out = st*cos + swap(st)*sin_rot   (tables pre-scaled by QSC)
              sw = stg.tile([128, SLAB], BF16, tag="sw")
              nc.sync.dma_start(out=sw[0:64, :], in_=st[64:128, :])
              nc.sync.dma_start(out=sw[64:128, :], in_=st[0:64, :])
              m1 = stg.tile([128, SLAB], BF16, tag="m1")
              nc.vector.tensor_tensor(out=m1[:], in0=st[:], in1=tcos[:, sl], op=MULT)
              nc.vector.tensor_tensor(out=sw[:], in0=sw[:], in1=tsin[:, sl], op=MULT)
              if which == "q":
                  nc.vector.tensor_tensor(out=qf8[h][:, sl], in0=m1[:], in1=sw[:], op=ADD)
              else:
                  kb = stg.tile([128, SLAB], BF16, tag="kb")
                  nc.vector.tensor_tensor(out=kb[:], in0=m1[:], in1=sw[:], op=ADD)
                  nc.scalar.copy(out=kpair[h][:, 0, sl], in_=kb[:])
                  nc.vector.tensor_tensor(out=kpair[h][:, 1, sl], in0=kb[:],
                                          in1=kpair[h][:, 0, sl], op=SUB)

          def v_tile(t):
              ns, sti = divmod(t, 4)
              s0 = sti * 128
              pv = mmp.tile([128, GQ], F32, tag="mm")
              passes = [(xres[ns], twvh), (xlres[ns], twvh), (xres[ns], twvl)]
              for pi, (xt_, wv_) in enumerate(passes):
                  for kc2 in range(8):
                      nc.tensor.matmul(pv[:], xt_[:, kc2, :, s0:s0 + 128],
                                       wv_[kc2][:], start=(pi == 0 and kc2 == 0),
                                       stop=(pi == 2 and kc2 == 7), perf_mode=DR)
              nc.vector.scalar_tensor_tensor(
                  out=vres[t], in0=pv[:], scalar=tinv[:], in1=tbvb[:],
                  op0=MULT, op1=ADD)

          def rec_group(h, gq):
              # ship recip(ell) for q-blocks 4g..4g+3 to DRAM and back as a row
              rt = smt[0:4, 0:128]
              nc.tensor.transpose(rt, lrec_cur[h][:, 4 * gq:4 * gq + 4], ident_f[:])
              rts = lrp.tile([4, 128], F32, tag="rts")
              nc.vector.tensor_copy(out=rts[:], in_=rt)
              nc.sync.dma_start(out=lrt[h, 0, 4 * gq:4 * gq + 4, :], in_=rts[:])
              nc.sync.dma_start(out=recrow[h][:, 4 * gq:4 * gq + 4, :],
                                in_=lrt[h, :, 4 * gq:4 * gq + 4, :])

          def denom(h, b):
              # ell[q] for q-block b: sum_k exp tiles via ap-1 matmuls, then recip
              lp = smt[:, 128 + (b % 4):129 + (b % 4)]
              for j in range(b + 1):
                  nc.tensor.matmul(lp, expT[h][j][:, (b - j) * 128:(b - j + 1) * 128],
                                   tones[:], start=(j == 0), stop=(j == b))
              nc.vector.reciprocal(out=lrec_cur[h][:, b:b + 1], in_=lp)

          def sweep2_group(h, gq):
              # ct = (sum_k V^T[k] expS^T[k]) * recip -> split into fp8 hi/lo
              rbs = rbp.tile([128, 512], F32, tag="rbs")
              nc.gpsimd.partition_broadcast(
                  rbs[:], recrow[h][:, 4 * gq:4 * gq + 4, :])
              ct = mmp.tile([128, 512], F32, tag="mm")
              last = 4 * gq + 3
              for j in range(last + 1):
                  if j <= 4 * gq:
                      nc.tensor.matmul(ct[:], vres[j][:, h * 128:(h + 1) * 128],
                                       expT[h][j][:, (4 * gq - j) * 128:(4 * gq - j) * 128 + 512],
                                       start=(j == 0), stop=(j == last))
                  else:
                      w = (4 * gq + 4 - j) * 128
                      nc.tensor.matmul(ct[:, 512 - w:512], vres[j][:, h * 128:(h + 1) * 128],
                                       expT[h][j][:, 0:w], start=False, stop=(j == last))
              ctb = ctsp.tile([128, 512], BF16, tag="ctb")
              nc.vector.tensor_tensor(out=ctb[:], in0=ct[:], in1=rbs[:], op=MULT)
              nc.scalar.copy(out=cth[(h, gq)][:], in_=ctb[:])
              nc.vector.tensor_tensor(out=ctl[gq][:, h, :], in0=ctb[:],
                                      in1=cth[(h, gq)][:], op=SUB)

          def scores_head(h, interleave):
              expT[h] = []
              lrec_cur[h] = lrp.tile([128, 16], F32, tag="lrec", name="lrec", bufs=1)
              recrow[h] = lrp.tile([1, 16, 128], F32, tag="recrow", name="recrow", bufs=1)
              kp = kpair[h]
              qf = qf8[h]
              for i in range(NB):
                  w = (NB - i) * 128
                  ex = expp.tile([128, w], BF16, tag=f"expT{i}", name=f"expT{i}")
                  expT[h].append(ex)
                  for c0 in range(0, w, 1024):
                      cw = min(1024, w - c0)
                      sp = sps.tile([128, 1024], F32, tag="sp")
                      for s5 in range(0, cw, 512):
                          w5 = min(512, cw - s5)
                          q0 = i * 128 + c0 + s5
                          nc.tensor.matmul(
                              sp[:, s5:s5 + w5], kp[:, :, i * 128:(i + 1) * 128],
                              qf[:, q0:q0 + w5].unsqueeze(1).broadcast_to((128, 2, w5)),
                              start=True, stop=True, perf_mode=DR)
                      nc.scalar.activation(out=ex[:, c0:c0 + cw], in_=sp[:, 0:cw],
                                           func=EXPF, scale=EXP_SCALE)
                      if c0 == 0:
                          nc.gpsimd.tensor_tensor(out=ex[:, 0:128], in0=ex[:, 0:128],
                                                  in1=tmask[:], op=MULT)
                  if i >= 2:
                      denom(h, i - 2)
                      if i % 4 == 1 and i >= 5:
                          rec_group(h, (i - 5) // 4)
                  interleave(h, i)
              denom(h, NB - 2)
              denom(h, NB - 1)
              rec_group(h, 3)

          def interleave_sec0(h, i):
              if i == 0:
                  xl_dma(2)
              elif i == 4:
                  xl_dma(3)
              v_tile(i)
              if i == NB - 1:
                  sec0.close()

          def interleave_std(h, i):
              if h >= 1 and i in (0, 1):
                  sweep2_group(h - 1, 2 * i)
                  sweep2_group(h - 1, 2 * i + 1)

          # ---- emit sections ----
          for h in range(HG):
              qf8[h] = qkp.tile([128, S], F8, tag="qf8", name=f"qf8_{h}")
              kpair[h] = qkp.tile([128, 2, S], F8, tag="kpair", name=f"kpair_{h}")
              for ns in range(4):
                  chain_qk(h, "q", ns)
                  chain_qk(h, "k", ns)
              if h + 1 < HG:
                  weights_dma(h + 1)
              scores_head(h, interleave_sec0 if h == 0 else interleave_std)

          # close projection pools; open out-proj weight pools
          phA.close()
          tailp = ExitStack()
          wop = tailp.enter_context(tc.tile_pool(name="wop", bufs=1))
          obp = tailp.enter_context(tc.tile_pool(name="obp", bufs=kn["obp"]))
          wo1t = []
          for kh in range(HG):
              wt_ = wop.tile([128, 2, D], F8, tag=f"wo1_{kh}")
              nc.sync.dma_start(out=wt_, in_=wo1[kh])
              wo1t.append(wt_)
          wo2t = []
          for gp in range(2):
              wt_ = wop.tile([128, 2, D], F8, tag=f"wo2_{gp}")
              nc.sync.dma_start(out=wt_, in_=wo2[gp])
              wo2t.append(wt_)

          # ---- tail: head-3 sweep2 + output projection ----
          sweep2_group(HG - 1, 0)
          for gq in range(4):
              if gq + 1 < 4:
                  sweep2_group(HG - 1, gq + 1)
              for mt in range(16):
                  op = mmp.tile([128, 512], F32, tag="mm")
                  for kh in range(HG):
                      nc.tensor.matmul(
                          op[:], wo1t[kh][:, :, mt * 128:(mt + 1) * 128],
                          cth[(kh, gq)][:].unsqueeze(1).broadcast_to((128, 2, 512)),
                          start=(kh == 0), stop=False, perf_mode=DR)
                  for gp in range(2):
                      nc.tensor.matmul(
                          op[:], wo2t[gp][:, :, mt * 128:(mt + 1) * 128],
                          ctl[gq][:, 2 * gp:2 * gp + 2, :],
                          start=False, stop=(gp == 1), perf_mode=DR)
                  ob = obp.tile([128, 512], BF16, tag="ob")
                  if mt % 2 == 0:
                      nc.vector.tensor_copy(out=ob[:], in_=op[:])
                  else:
                      nc.scalar.copy(out=ob[:], in_=op[:])
                  nc.sync.dma_start(out=outt[mt, :, gq * 512:(gq + 1) * 512], in_=ob[:])
          tailp.close()
          phB.close()
    nc.finalize()
    return nc


_NC_CACHE = {}


def _get_nc(reps=1):
    if reps not in _NC_CACHE:
        _NC_CACHE[reps] = build_nc(reps)
    return _NC_CACHE[reps]


def _rope_tables(position_ids_b):
    pos = position_ids_b.astype(np.float32)
    inv_freq = (1.0 / (ROPE_THETA ** (np.arange(0, DH, 2, dtype=np.float32) / np.float32(DH))))
    ang = pos[:, None] * inv_freq[None, :]          # [S, 64]
    emb = np.concatenate([ang, ang], axis=-1)       # [S, 128]
    cosT = np.ascontiguousarray(np.cos(emb).T) * np.float32(QSC)   # [128, S]
    sinT = np.sin(emb).T * np.float32(QSC)
    sin_rot = np.concatenate([-sinT[0:64], sinT[64:128]], axis=0)
    return cosT.astype(ml_dtypes.bfloat16), np.ascontiguousarray(sin_rot).astype(ml_dtypes.bfloat16)


def _make_in_maps(inputs):
    hidden_states = np.asarray(inputs["hidden_states"], dtype=np.float32)
    position_ids = np.asarray(inputs["position_ids"])
    Wqkv = np.asarray(inputs["Wqkv"], dtype=np.float32)
    bqkv = np.asarray(inputs["bqkv"], dtype=np.float32)
    Wo = np.asarray(inputs["Wo"], dtype=np.float32)

    mask = np.triu(np.ones((128, 128), dtype=np.float32)).astype(ml_dtypes.bfloat16)
    tones = np.full((128, 1), 1.0 / CSC, dtype=ml_dtypes.bfloat16)
    tabs = [_rope_tables(np.asarray(position_ids)[b]) for b in range(B)]

    def _hilo(M, sc):
        Ms = M * np.float32(sc)
        hi = Ms.astype(ml_dtypes.float8_e4m3)
        lo = (Ms - hi.astype(np.float32)).astype(ml_dtypes.float8_e4m3)
        return hi, lo

    def _pack_pairs(M):
        # [D, C] -> [8, 128, 2, C] with row r = kc2*256 + i*128 + p
        C = M.shape[1]
        return np.ascontiguousarray(M.reshape(8, 2, 128, C).transpose(0, 2, 1, 3))

    def _pack_x(M):
        # [D, S] -> [128, 4, 8, 2, SLAB] partition-major, slab-major free
        return np.ascontiguousarray(
            M.reshape(8, 2, 128, 4, SLAB).transpose(2, 3, 0, 1, 4))

    xts = []
    for b in range(B):
        hi, lo = _hilo(np.ascontiguousarray(hidden_states[b].T), XSC)
        xts.append((_pack_x(hi.astype(np.float32)).astype(ml_dtypes.float8_e4m3),
                    _pack_x(lo.astype(np.float32)).astype(ml_dtypes.float8_e4m3)))

    in_maps = []
    for c in range(NCORES):
        b, hg = divmod(c, HG)
        qcols = slice(hg * GQ, (hg + 1) * GQ)
        kcols = slice(D + hg * GQ, D + (hg + 1) * GQ)
        vcols = slice(2 * D + hg * GQ, 2 * D + (hg + 1) * GQ)
        wqk_c = np.ascontiguousarray(np.concatenate([Wqkv[:, qcols], Wqkv[:, kcols]], axis=1))
        qk_h, qk_l = _hilo(wqk_c, WSC)
        # per-mt packing: [8(mt), 128(p), 8(kc2), 2(i), 128(m)]
        def _pack_mt(M8):
            P = _pack_pairs(M8.astype(np.float32))          # [8, 128, 2, 1024]
            P = P.reshape(8, 128, 2, 8, 128)                 # [kc2, p, i, mt, m]
            return np.ascontiguousarray(P.transpose(3, 1, 0, 2, 4)).astype(ml_dtypes.float8_e4m3)
        wqkh_c = _pack_mt(qk_h)
        wqkl_c = _pack_mt(qk_l)
        wv_c = np.ascontiguousarray(Wqkv[:, vcols])
        v_h, v_l = _hilo(wv_c, WSC)
        wvh_c = _pack_pairs(v_h.astype(np.float32)).astype(ml_dtypes.float8_e4m3)
        wvl_c = _pack_pairs(v_l.astype(np.float32)).astype(ml_dtypes.float8_e4m3)
        # out-proj fp8 packs
        wo_c = np.ascontiguousarray(Wo[hg * GQ:(hg + 1) * GQ, :])   # [512, D]
        wo_h, wo_l = _hilo(wo_c, WSC)
        wo_h = wo_h.astype(np.float32)
        wo_l = wo_l.astype(np.float32)
        wo1_c = np.empty((HG, 128, 2, D), np.float32)
        for kh in range(HG):
            wo1_c[kh, :, 0, :] = wo_h[kh * 128:(kh + 1) * 128, :]
            wo1_c[kh, :, 1, :] = wo_l[kh * 128:(kh + 1) * 128, :]
        wo2_c = np.empty((2, 128, 2, D), np.float32)
        for gp in range(2):
            wo2_c[gp, :, 0, :] = wo_h[gp * 256:gp * 256 + 128, :]
            wo2_c[gp, :, 1, :] = wo_h[gp * 256 + 128:gp * 256 + 256, :]
        bqk_c = np.concatenate([bqkv[qcols], bqkv[kcols]]).reshape(8, 128).T
        bv_c = bqkv[vcols].reshape(1, GQ)
        cosT, sin_rot = tabs[b]
        in_maps.append({
            "xh": xts[b][0], "xl": xts[b][1],
            "wqkh": wqkh_c, "wqkl": wqkl_c, "wvh": wvh_c, "wvl": wvl_c,
            "wo1": wo1_c.astype(ml_dtypes.float8_e4m3),
            "wo2": wo2_c.astype(ml_dtypes.float8_e4m3),
            "bqkt": np.ascontiguousarray(bqk_c),
            "bv": np.ascontiguousarray(bv_c),
            "cost": cosT, "sinrt": sin_rot, "maskd": mask,
            "tonesd": tones,
        })
    return in_maps


def kernel(hidden_states, position_ids, Wqkv, bqkv, Wo, bo, _reps=1):
    bo = np.asarray(bo, dtype=np.float32)
    in_maps = _make_in_maps({
        "hidden_states": hidden_states, "position_ids": position_ids,
        "Wqkv": Wqkv, "bqkv": bqkv, "Wo": Wo, "bo": bo,
    })
    nc = _get_nc(_reps)
    res = run_bass_kernel_spmd(nc, in_maps, core_ids=list(range(NCORES)))

    out = np.empty((B, S, D), dtype=np.float32)
    for b in range(B):
        acc = res.results[b * HG]["outt"].reshape(D, S).astype(np.float32).copy()
        for hg in range(1, HG):
            acc += res.results[b * HG + hg]["outt"].reshape(D, S).astype(np.float32)
        out[b] = acc.T * np.float32(P3_SCALE) + bo[None, :]
    return out


# revision 7
# speedup vs baseline: 1.1094x; 1.1094x over previous
"""Trainium2 Bass kernel for CustomRoPEAttention (B=2, S=2048, H=16, Dh=128).

Sharding: 8 cores = 2 batches x 4 head-groups (4 heads/core).

Head-pipelined structure: per head h, QKV^T projection (fp8 hi/lo DoubleRow
matmuls) + RoPE, then transposed-layout causal attention for that head while
later heads' projections stream -- this overlaps the ACT-engine exp work with
PE-engine matmul work across the whole kernel instead of serializing phases.

fp8 DoubleRow "dup trick": scores use stationary (k_hi,k_lo) pairs against a
broadcast (step-0) fp8 q moving operand, and the output projection uses
(wo_hi,wo_lo) pairs against broadcast ct_hi plus a wo_hi x ct_lo correction --
half / 0.75x the bf16 PE time at first-order-exact precision.

Host sums the 4 partial (transposed) output projections per batch.

Self-contained: hardcodes shapes from the problem spec.
"""
import math
from contextlib import ExitStack

import numpy as np
import ml_dtypes

import concourse.mybir as mybir
import concourse.tile as tile
from concourse import bacc
from concourse.bass_utils import run_bass_kernel_spmd
from concourse.masks import make_identity

S = 2048            # sequence
D = 2048            # hidden
NH = 16             # total heads
DH = 128            # head dim
HG = 4              # heads per core
GQ = HG * DH        # 512: per-core q/k/v feature width
B = 2
NCORES = 8
ROPE_THETA = 10000.0
SCALE = 1.0 / math.sqrt(DH)
SLAB = 512          # qkv sequence slab width
XSC = 16.0          # fp8 pre-scale for x
WSC = 512.0         # fp8 pre-scale for Wqkv / Wo
QSC = 16.0          # fp8 pre-scale for roped q/k (folded into cos/sin tables)
CSC = 16.0          # fp8 pre-scale for attention-out ct (folded into tones)
INV_SC = 1.0 / (XSC * WSC)
EXP_SCALE = SCALE / (QSC * QSC)
P3_SCALE = 1.0 / (WSC * CSC)   # applied host-side
F32 = mybir.dt.float32
BF16 = mybir.dt.bfloat16
F8 = mybir.dt.float8e4
MULT = mybir.AluOpType.mult
ADD = mybir.AluOpType.add
SUB = mybir.AluOpType.subtract
DR = mybir.MatmulPerfMode.DoubleRow
NB = S // 128       # 16 k/q blocks
IDENT = mybir.ActivationFunctionType.Identity
EXPF = mybir.ActivationFunctionType.Exp


def build_nc(reps=1, knobs=None):
    kn = {"sps": 2, "mmp": 3, "expb": 1, "wqb": 2, "qkb": 2, "stg": 2, "obp": 4}
    if knobs:
        kn.update(knobs)
    nc = bacc.Bacc(None, target_bir_lowering=False)
    # x^T hi/lo, slab-major pack: [p, ns, kc2, i, s]
    xh = nc.dram_tensor("xh", [128, 4, 8, 2, SLAB], F8, kind="ExternalInput")
    xl = nc.dram_tensor("xl", [128, 4, 8, 2, SLAB], F8, kind="ExternalInput")
    # per-mt packed qk weights: [mt, p, kc2, i, m]
    wqkh = nc.dram_tensor("wqkh", [8, 128, 8, 2, 128], F8, kind="ExternalInput")
    wvh = nc.dram_tensor("wvh", [8, 128, 2, GQ], F8, kind="ExternalInput")
    wvl = nc.dram_tensor("wvl", [8, 128, 2, GQ], F8, kind="ExternalInput")
    # out-proj fp8 packs: wo1[kh] = (hi,lo) pairs; wo2[g] = hi head-pair packs
    wo1 = nc.dram_tensor("wo1", [HG, 128, 2, D], F8, kind="ExternalInput")
    wo2 = nc.dram_tensor("wo2", [2, 128, 2, D], F8, kind="ExternalInput")
    bqkt = nc.dram_tensor("bqkt", [128, 8], F32, kind="ExternalInput")
    bv = nc.dram_tensor("bv", [1, GQ], F32, kind="ExternalInput")
    cost = nc.dram_tensor("cost", [128, S], BF16, kind="ExternalInput")    # cos^T * QSC
    sinrt = nc.dram_tensor("sinrt", [128, S], BF16, kind="ExternalInput")  # sin^T * QSC, rot sign
    maskd = nc.dram_tensor("maskd", [128, 128], BF16, kind="ExternalInput")  # triu 0/1 keep-mask
    tonesd = nc.dram_tensor("tonesd", [128, 1], BF16, kind="ExternalInput")  # 1/CSC
    outt = nc.dram_tensor("outt", [16, 128, S], BF16, kind="ExternalOutput")
    lrt = nc.dram_tensor("lrt", [HG, 1, 16, 128], F32)  # recip bounce: [16,128] -> [1,2048]

    with tile.TileContext(nc) as tc, ExitStack() as top:
        g = top.enter_context(tc.tile_pool(name="glob", bufs=1))
        tcos = g.tile([128, S], BF16)
        tsin = g.tile([128, S], BF16)
        tmask = g.tile([128, 128], BF16)
        ident_f = g.tile([128, 128], F32)
        make_identity(nc, ident_f[:])
        tbqkt = g.tile([128, 8], F32)
        tbvb = g.tile([128, GQ], F32)
        tones = g.tile([128, 1], BF16)
        tinv = g.tile([128, 1], F32)
        nc.vector.memset(tinv[:], INV_SC)

        # Whole-kernel residents
        res = top.enter_context(tc.tile_pool(name="res", bufs=1))
        vres = []  # 16 V k-block tiles [128(seq), GQ] bf16
        for t in range(NB):
            vres.append(res.tile([128, GQ], BF16, tag=f"v{t}", name=f"v{t}"))
        cth = {}
        for h in range(HG):
            for gq in range(4):
                cth[(h, gq)] = res.tile([128, 512], F8, tag=f"cth_{h}_{gq}",
                                        name=f"cth_{h}_{gq}")
        ctl = [res.tile([128, HG, 512], F8, tag=f"ctl{gq}", name=f"ctl{gq}")
               for gq in range(4)]

        for _rep in range(reps):
          phB = ExitStack()
          mmp = phB.enter_context(tc.tile_pool(name="mmp", bufs=kn["mmp"], space="PSUM"))
          sps = phB.enter_context(tc.tile_pool(name="sps", bufs=kn["sps"], space="PSUM"))
          smps = phB.enter_context(tc.tile_pool(name="smps", bufs=1, space="PSUM"))
          expp = phB.enter_context(tc.tile_pool(name="expp", bufs=kn["expb"]))
          lrp = phB.enter_context(tc.tile_pool(name="lrp", bufs=2))
          rbp = phB.enter_context(tc.tile_pool(name="rbp", bufs=2))
          ctsp = phB.enter_context(tc.tile_pool(name="ctsp", bufs=2))
          obp = phB.enter_context(tc.tile_pool(name="obp", bufs=kn["obp"]))
          smt = smps.tile([128, 132], F32, tag="sm", name="smt")

          phA = ExitStack()
          wqp = phA.enter_context(tc.tile_pool(name="wqp", bufs=kn["wqb"]))
          qkp = phA.enter_context(tc.tile_pool(name="qkp", bufs=kn["qkb"]))
          stg = phA.enter_context(tc.tile_pool(name="stg", bufs=kn["stg"]))
          xps = ExitStack()
          xp = xps.enter_context(tc.tile_pool(name="xp", bufs=1))
          sec0 = ExitStack()
          xsl = sec0.enter_context(tc.tile_pool(name="xsl", bufs=3))
          wvp = sec0.enter_context(tc.tile_pool(name="wvp", bufs=1))

          # ---- initial DMA order (startup-critical) ----
          wq_tiles = {}  # (h) -> (wqh, wql, wkh, wkl)

          def weights_dma(h):
              tl = []
              for mt, tag in ((h, "wqh"), (4 + h, "wkh")):
                  wt = wqp.tile([128, 8, 2, 128], F8, tag=tag, name=f"{tag}{h}")
                  nc.sync.dma_start(out=wt, in_=wqkh[mt])
                  tl.append(wt)
              wq_tiles[h] = tl

          # head-0 weights + first x slab first
          wt = wqp.tile([128, 8, 2, 128], F8, tag="wqh", name="wqh0")
          nc.sync.dma_start(out=wt, in_=wqkh[0])
          wt2 = wqp.tile([128, 8, 2, 128], F8, tag="wkh", name="wkh0")
          nc.sync.dma_start(out=wt2, in_=wqkh[4])
          wq_tiles[0] = [wt, wt2]
          xres = [xp.tile([128, 8, 2, SLAB], F8, tag=f"x{ns}", name=f"x{ns}")
                  for ns in range(4)]
          nc.sync.dma_start(out=xres[0], in_=xh[:, 0])
          # small consts needed by first psum copies / rope
          nc.sync.dma_start(out=tbqkt, in_=bqkt[:])
          nc.sync.dma_start(out=xres[1], in_=xh[:, 1])
          nc.sync.dma_start(out=tcos, in_=cost[:])
          nc.sync.dma_start(out=tsin, in_=sinrt[:])
          nc.sync.dma_start(out=xres[2], in_=xh[:, 2])
          nc.sync.dma_start(out=tones, in_=tonesd[:])
          nc.sync.dma_start(out=tmask, in_=maskd[:])
          nc.sync.dma_start(out=xres[3], in_=xh[:, 3])
          nc.sync.dma_start(out=tbvb, in_=bv[:].to_broadcast((128, GQ)))
          twvh, twvl = [], []
          xlres = {}

          def wv_dmas():
              for kc2 in range(8):
                  wv_t = wvp.tile([128, 2, GQ], F8, tag=f"wvh{kc2}")
                  nc.sync.dma_start(out=wv_t, in_=wvh[kc2])
                  twvh.append(wv_t)
              for kc2 in range(8):
                  wv_t = wvp.tile([128, 2, GQ], F8, tag=f"wvl{kc2}")
                  nc.sync.dma_start(out=wv_t, in_=wvl[kc2])
                  twvl.append(wv_t)

          def xl_dma(ns):
              xt = xsl.tile([128, 8, 2, SLAB], F8, tag="xl", name=f"xl{ns}")
              nc.sync.dma_start(out=xt, in_=xl[:, ns])
              xlres[ns] = xt

          # ---- per-head state ----
          qf8 = {}
          kpair = {}
          expT = {}
          lrec_cur = {}
          recrow = {}

          def chain_qk(h, which, ns):
              wqh_, wkh_ = wq_tiles[h]
              whi = wqh_ if which == "q" else wkh_
              sl = slice(ns * SLAB, (ns + 1) * SLAB)
              ps = mmp.tile([128, SLAB], F32, tag="mm")
              for kc2 in range(8):
                  nc.tensor.matmul(ps[:], whi[:, kc2, :, :], xres[ns][:, kc2, :, :],
                                   start=(kc2 == 0), stop=(kc2 == 7), perf_mode=DR)
              st = stg.tile([128, SLAB], BF16, tag="st")
              mt = h if which == "q" else 4 + h
              nc.vector.tensor_scalar(out=st[:], in0=ps[:], scalar1=INV_SC,
                                      scalar2=tbqkt[:, mt:mt + 1],
                                      op0=MULT, op1=ADD)
              # RoPE: out = st*cos + swap(st)*sin_rot   (tables pre-scaled by QSC)
              sw = stg.tile([128, SLAB], BF16, tag="sw")
              nc.sync.dma_start(out=sw[0:64, :], in_=st[64:128, :])
              nc.sync.dma_start(out=sw[64:128, :], in_=st[0:64, :])
              m1 = stg.tile([128, SLAB], BF16, tag="m1")
              nc.vector.tensor_tensor(out=m1[:], in0=st[:], in1=tcos[:, sl], op=MULT)
              nc.vector.tensor_tensor(out=sw[:], in0=sw[:], in1=tsin[:, sl], op=MULT)
              if which == "q":
                  nc.vector.tensor_tensor(out=qf8[h][:, sl], in0=m1[:], in1=sw[:], op=ADD)
              else:
                  kb = stg.tile([128, SLAB], BF16, tag="kb")
                  nc.vector.tensor_tensor(out=kb[:], in0=m1[:], in1=sw[:], op=ADD)
                  nc.scalar.copy(out=kpair[h][:, 0, sl], in_=kb[:])
                  nc.vector.tensor_tensor(out=kpair[h][:, 1, sl], in0=kb[:],
                                          in1=kpair[h][:, 0, sl], op=SUB)

          def v_tile(t):
              ns, sti = divmod(t, 4)
              s0 = sti * 128
              pv = mmp.tile([128, GQ], F32, tag="mm")
              passes = [(xres[ns], twvh), (xlres[ns], twvh), (xres[ns], twvl)]
              for pi, (xt_, wv_) in enumerate(passes):
                  for kc2 in range(8):
                      nc.tensor.matmul(pv[:], xt_[:, kc2, :, s0:s0 + 128],
                                       wv_[kc2][:], start=(pi == 0 and kc2 == 0),
                                       stop=(pi == 2 and kc2 == 7), perf_mode=DR)
              nc.vector.scalar_tensor_tensor(
                  out=vres[t], in0=pv[:], scalar=tinv[:], in1=tbvb[:],
                  op0=MULT, op1=ADD)

          def rec_group(h, gq):
              # ship recip(ell) for q-blocks 4g..4g+3 to DRAM and back as a row
              rt = smt[0:4, 0:128]
              nc.tensor.transpose(rt, lrec_cur[h][:, 4 * gq:4 * gq + 4], ident_f[:])
              rts = lrp.tile([4, 128], F32, tag="rts")
              nc.vector.tensor_copy(out=rts[:], in_=rt)
              nc.sync.dma_start(out=lrt[h, 0, 4 * gq:4 * gq + 4, :], in_=rts[:])
              nc.sync.dma_start(out=recrow[h][:, 4 * gq:4 * gq + 4, :],
                                in_=lrt[h, :, 4 * gq:4 * gq + 4, :])

          def denom(h, b):
              # ell[q] for q-block b: sum_k exp tiles via ap-1 matmuls, then recip
              lp = smt[:, 128 + (b % 4):129 + (b % 4)]
              for j in range(b + 1):
                  nc.tensor.matmul(lp, expT[h][j][:, (b - j) * 128:(b - j + 1) * 128],
                                   tones[:], start=(j == 0), stop=(j == b))
              nc.vector.reciprocal(out=lrec_cur[h][:, b:b + 1], in_=lp)

          def sweep2_group(h, gq):
              # ct = (sum_k V^T[k] expS^T[k]) * recip -> split into fp8 hi/lo
              rbs = rbp.tile([128, 512], F32, tag="rbs")
              nc.gpsimd.partition_broadcast(
                  rbs[:], recrow[h][:, 4 * gq:4 * gq + 4, :])
              ct = mmp.tile([128, 512], F32, tag="mm")
              last = 4 * gq + 3
              for j in range(last + 1):
                  if j <= 4 * gq:
                      nc.tensor.matmul(ct[:], vres[j][:, h * 128:(h + 1) * 128],
                                       expT[h][j][:, (4 * gq - j) * 128:(4 * gq - j) * 128 + 512],
                                       start=(j == 0), stop=(j == last))
                  else:
                      w = (4 * gq + 4 - j) * 128
                      nc.tensor.matmul(ct[:, 512 - w:512], vres[j][:, h * 128:(h + 1) * 128],
                                       expT[h][j][:, 0:w], start=False, stop=(j == last))
              ctb = ctsp.tile([128, 512], BF16, tag="ctb")
              nc.vector.tensor_tensor(out=ctb[:], in0=ct[:], in1=rbs[:], op=MULT)
              nc.scalar.copy(out=cth[(h, gq)][:], in_=ctb[:])
              nc.vector.tensor_tensor(out=ctl[gq][:, h, :], in0=ctb[:],
                                      in1=cth[(h, gq)][:], op=SUB)

          def scores_head(h, interleave):
              expT[h] = []
              lrec_cur[h] = lrp.tile([128, 16], F32, tag="lrec", name="lrec", bufs=1)
              recrow[h] = lrp.tile([1, 16, 128], F32, tag="recrow", name="recrow", bufs=1)
              kp = kpair[h]
              qf = qf8[h]
              for i in range(NB):
                  w = (NB - i) * 128
                  ex = expp.tile([128, w], BF16, tag=f"expT{i}", name=f"expT{i}")
                  expT[h].append(ex)
                  for c0 in range(0, w, 1024):
                      cw = min(1024, w - c0)
                      sp = sps.tile([128, 1024], F32, tag="sp")
                      for s5 in range(0, cw, 512):
                          w5 = min(512, cw - s5)
                          q0 = i * 128 + c0 + s5
                          nc.tensor.matmul(
                              sp[:, s5:s5 + w5], kp[:, :, i * 128:(i + 1) * 128],
                              qf[:, q0:q0 + w5].unsqueeze(1).broadcast_to((128, 2, w5)),
                              start=True, stop=True, perf_mode=DR)
                      nc.scalar.activation(out=ex[:, c0:c0 + cw], in_=sp[:, 0:cw],
                                           func=EXPF, scale=EXP_SCALE)
                      if c0 == 0:
                          nc.gpsimd.tensor_tensor(out=ex[:, 0:128], in0=ex[:, 0:128],
                                                  in1=tmask[:], op=MULT)
                  if i >= 2:
                      denom(h, i - 2)
                      if i % 4 == 1 and i >= 5:
                          rec_group(h, (i - 5) // 4)
                  interleave(h, i)
              denom(h, NB - 2)
              denom(h, NB - 1)
              rec_group(h, 3)

          def alloc_qk(h):
              qf8[h] = qkp.tile([128, S], F8, tag="qf8", name=f"qf8_{h}")
              kpair[h] = qkp.tile([128, 2, S], F8, tag="kpair", name=f"kpair_{h}")

          wo1t = []
          wo2t = []
          tailp = ExitStack()

          def open_wop():
              xps.close()
              wop = tailp.enter_context(tc.tile_pool(name="wop", bufs=1))
              for kh in range(HG):
                  wt_ = wop.tile([128, 2, D], F8, tag=f"wo1_{kh}", name=f"wo1_{kh}")
                  nc.sync.dma_start(out=wt_, in_=wo1[kh])
                  wo1t.append(wt_)
              for gp in range(2):
                  wt_ = wop.tile([128, 2, D], F8, tag=f"wo2_{gp}", name=f"wo2_{gp}")
                  nc.sync.dma_start(out=wt_, in_=wo2[gp])
                  wo2t.append(wt_)

          def p3_mt(gq, mt):
              op = mmp.tile([128, 512], F32, tag="mm")
              for kh in range(HG):
                  nc.tensor.matmul(
                      op[:], wo1t[kh][:, :, mt * 128:(mt + 1) * 128],
                      cth[(kh, gq)][:].unsqueeze(1).broadcast_to((128, 2, 512)),
                      start=(kh == 0), stop=False, perf_mode=DR)
              for gp in range(2):
                  nc.tensor.matmul(
                      op[:], wo2t[gp][:, :, mt * 128:(mt + 1) * 128],
                      ctl[gq][:, 2 * gp:2 * gp + 2, :],
                      start=False, stop=(gp == 1), perf_mode=DR)
              ob = obp.tile([128, 512], BF16, tag="ob")
              if mt % 2 == 0:
                  nc.vector.tensor_copy(out=ob[:], in_=op[:])
              else:
                  nc.scalar.copy(out=ob[:], in_=op[:])
              nc.sync.dma_start(out=outt[mt, :, gq * 512:(gq + 1) * 512], in_=ob[:])

          def mk_sched(h):
              # schedule of extra PE work per score block of head h
              sched = {i: [] for i in range(NB)}
              if h == 0:
                  # chains for head 1 at blocks 2-5, v tiles 2/block at 7-14
                  sched[2].append(lambda: alloc_qk(1))
                  for idx, (which, ns) in enumerate(
                          (w, n) for n in range(4) for w in ("q", "k")):
                      sched[2 + idx // 2].append(
                          lambda w=which, n=ns: chain_qk(1, w, n))
                  sched[3].append(lambda: xl_dma(2))
                  sched[6].append(lambda: xl_dma(3))
                  for t in range(NB):
                      sched[7 + t // 2].append(lambda t=t: v_tile(t))
                  sched[10].append(lambda: weights_dma(2))
                  sched[15].append(sec0.close)
              else:
                  sched[0] += [lambda: sweep2_group(h - 1, 0),
                               lambda: sweep2_group(h - 1, 1)]
                  sched[1] += [lambda: sweep2_group(h - 1, 2),
                               lambda: sweep2_group(h - 1, 3)]
                  if h < 3:
                      sched[2].append(lambda: alloc_qk(h + 1))
                      for idx, (which, ns) in enumerate(
                              (w, n) for n in range(4) for w in ("q", "k")):
                          sched[2 + idx // 2].append(
                              lambda w=which, n=ns: chain_qk(h + 1, w, n))
                      if h + 2 < HG:
                          sched[10].append(lambda: weights_dma(h + 2))
                  if h == 2:
                      sched[6].append(open_wop)
                  if h == 3:
                      sched[8].append(lambda: sweep2_group(3, 0))
                      for j in range(16):       # p3 gq0 spread over blocks 9-14
                          sched[9 + min(j // 3, 5)].append(
                              lambda mt=j: p3_mt(0, mt))
                      sched[12].append(lambda: sweep2_group(3, 1))
                      for j in range(6):        # first p3 gq1 pieces
                          sched[13 + j // 2].append(lambda mt=j: p3_mt(1, mt))
              return sched

          def run_sched(sched, h, i):
              for fn in sched[i]:
                  fn()

          # ---- emit sections ----
          alloc_qk(0)
          for ns in range(4):
              chain_qk(0, "q", ns)
              chain_qk(0, "k", ns)
          weights_dma(1)
          wv_dmas()
          xl_dma(0)
          xl_dma(1)
          for h in range(HG):
              sched = mk_sched(h)
              scores_head(h, lambda hh, i, sched=sched: run_sched(sched, hh, i))

          # ---- tail: rest of the output projection ----
          for mt in range(6, 16):
              p3_mt(1, mt)
          sweep2_group(3, 2)
          for mt in range(16):
              p3_mt(2, mt)
          sweep2_group(3, 3)
          for mt in range(16):
              p3_mt(3, mt)
          tailp.close()
          phA.close()
          phB.close()
    nc.finalize()
    return nc


_NC_CACHE = {}


def _get_nc(reps=1):
    if reps not in _NC_CACHE:
        _NC_CACHE[reps] = build_nc(reps)
    return _NC_CACHE[reps]


def _rope_tables(position_ids_b):
    pos = position_ids_b.astype(np.float32)
    inv_freq = (1.0 / (ROPE_THETA ** (np.arange(0, DH, 2, dtype=np.float32) / np.float32(DH))))
    ang = pos[:, None] * inv_freq[None, :]          # [S, 64]
    emb = np.concatenate([ang, ang], axis=-1)       # [S, 128]
    cosT = np.ascontiguousarray(np.cos(emb).T) * np.float32(QSC)   # [128, S]
    sinT = np.sin(emb).T * np.float32(QSC)
    sin_rot = np.concatenate([-sinT[0:64], sinT[64:128]], axis=0)
    return cosT.astype(ml_dtypes.bfloat16), np.ascontiguousarray(sin_rot).astype(ml_dtypes.bfloat16)


def _make_in_maps(inputs):
    hidden_states = np.asarray(inputs["hidden_states"], dtype=np.float32)
    position_ids = np.asarray(inputs["position_ids"])
    Wqkv = np.asarray(inputs["Wqkv"], dtype=np.float32)
    bqkv = np.asarray(inputs["bqkv"], dtype=np.float32)
    Wo = np.asarray(inputs["Wo"], dtype=np.float32)

    mask = np.triu(np.ones((128, 128), dtype=np.float32)).astype(ml_dtypes.bfloat16)
    tones = np.full((128, 1), 1.0 / CSC, dtype=ml_dtypes.bfloat16)
    tabs = [_rope_tables(np.asarray(position_ids)[b]) for b in range(B)]

    def _hilo(M, sc):
        Ms = M * np.float32(sc)
        hi = Ms.astype(ml_dtypes.float8_e4m3)
        lo = (Ms - hi.astype(np.float32)).astype(ml_dtypes.float8_e4m3)
        return hi, lo

    def _pack_pairs(M):
        # [D, C] -> [8, 128, 2, C] with row r = kc2*256 + i*128 + p
        C = M.shape[1]
        return np.ascontiguousarray(M.reshape(8, 2, 128, C).transpose(0, 2, 1, 3))

    def _pack_x(M):
        # [D, S] -> [128, 4, 8, 2, SLAB] partition-major, slab-major free
        return np.ascontiguousarray(
            M.reshape(8, 2, 128, 4, SLAB).transpose(2, 3, 0, 1, 4))

    xts = []
    for b in range(B):
        hi, lo = _hilo(np.ascontiguousarray(hidden_states[b].T), XSC)
        xts.append((_pack_x(hi.astype(np.float32)).astype(ml_dtypes.float8_e4m3),
                    _pack_x(lo.astype(np.float32)).astype(ml_dtypes.float8_e4m3)))

    in_maps = []
    for c in range(NCORES):
        b, hg = divmod(c, HG)
        qcols = slice(hg * GQ, (hg + 1) * GQ)
        kcols = slice(D + hg * GQ, D + (hg + 1) * GQ)
        vcols = slice(2 * D + hg * GQ, 2 * D + (hg + 1) * GQ)
        wqk_c = np.ascontiguousarray(np.concatenate([Wqkv[:, qcols], Wqkv[:, kcols]], axis=1))
        qk_h, qk_l = _hilo(wqk_c, WSC)
        # per-mt packing: [8(mt), 128(p), 8(kc2), 2(i), 128(m)]
        def _pack_mt(M8):
            P = _pack_pairs(M8.astype(np.float32))          # [8, 128, 2, 1024]
            P = P.reshape(8, 128, 2, 8, 128)                 # [kc2, p, i, mt, m]
            return np.ascontiguousarray(P.transpose(3, 1, 0, 2, 4)).astype(ml_dtypes.float8_e4m3)
        wqkh_c = _pack_mt(qk_h)
        wqkl_c = _pack_mt(qk_l)
        wv_c = np.ascontiguousarray(Wqkv[:, vcols])
        v_h, v_l = _hilo(wv_c, WSC)
        wvh_c = _pack_pairs(v_h.astype(np.float32)).astype(ml_dtypes.float8_e4m3)
        wvl_c = _pack_pairs(v_l.astype(np.float32)).astype(ml_dtypes.float8_e4m3)
        # out-proj fp8 packs
        wo_c = np.ascontiguousarray(Wo[hg * GQ:(hg + 1) * GQ, :])   # [512, D]
        wo_h, wo_l = _hilo(wo_c, WSC)
        wo_h = wo_h.astype(np.float32)
        wo_l = wo_l.astype(np.float32)
        wo1_c = np.empty((HG, 128, 2, D), np.float32)
        for kh in range(HG):
            wo1_c[kh, :, 0, :] = wo_h[kh * 128:(kh + 1) * 128, :]
            wo1_c[kh, :, 1, :] = wo_l[kh * 128:(kh + 1) * 128, :]
        wo2_c = np.empty((2, 128, 2, D), np.float32)
        for gp in range(2):
            wo2_c[gp, :, 0, :] = wo_h[gp * 256:gp * 256 + 128, :]
            wo2_c[gp, :, 1, :] = wo_h[gp * 256 + 128:gp * 256 + 256, :]
        bqk_c = np.concatenate([bqkv[qcols], bqkv[kcols]]).reshape(8, 128).T
        bv_c = bqkv[vcols].reshape(1, GQ)
        cosT, sin_rot = tabs[b]
        in_maps.append({
            "xh": xts[b][0], "xl": xts[b][1],
            "wqkh": wqkh_c, "wqkl": wqkl_c, "wvh": wvh_c, "wvl": wvl_c,
            "wo1": wo1_c.astype(ml_dtypes.float8_e4m3),
            "wo2": wo2_c.astype(ml_dtypes.float8_e4m3),
            "bqkt": np.ascontiguousarray(bqk_c),
            "bv": np.ascontiguousarray(bv_c),
            "cost": cosT, "sinrt": sin_rot, "maskd": mask,
            "tonesd": tones,
        })
    return in_maps


def kernel(hidden_states, position_ids, Wqkv, bqkv, Wo, bo, _reps=1):
    bo = np.asarray(bo, dtype=np.float32)
    in_maps = _make_in_maps({
        "hidden_states": hidden_states, "position_ids": position_ids,
        "Wqkv": Wqkv, "bqkv": bqkv, "Wo": Wo, "bo": bo,
    })
    nc = _get_nc(_reps)
    res = run_bass_kernel_spmd(nc, in_maps, core_ids=list(range(NCORES)))

    out = np.empty((B, S, D), dtype=np.float32)
    for b in range(B):
        acc = res.results[b * HG]["outt"].reshape(D, S).astype(np.float32).copy()
        for hg in range(1, HG):
            acc += res.results[b * HG + hg]["outt"].reshape(D, S).astype(np.float32)
        out[b] = acc.T * np.float32(P3_SCALE) + bo[None, :]
    return out


# revision 8
# speedup vs baseline: 1.1422x; 1.0296x over previous
"""Trainium2 Bass kernel for CustomRoPEAttention (B=2, S=2048, H=16, Dh=128).

Sharding: 8 cores = 2 batches x 4 head-groups (4 heads/core).

Head-pipelined structure: per head h, QKV^T projection (fp8 hi/lo DoubleRow
matmuls) + RoPE, then transposed-layout causal attention for that head while
later heads' projections stream -- this overlaps the ACT-engine exp work with
PE-engine matmul work across the whole kernel instead of serializing phases.

fp8 DoubleRow "dup trick": scores use stationary (k_hi,k_lo) pairs against a
broadcast (step-0) fp8 q moving operand, and the output projection uses
(wo_hi,wo_lo) pairs against broadcast ct_hi plus a wo_hi x ct_lo correction --
half / 0.75x the bf16 PE time at first-order-exact precision.

Host sums the 4 partial (transposed) output projections per batch.

Self-contained: hardcodes shapes from the problem spec.
"""
import math
from contextlib import ExitStack

import numpy as np
import ml_dtypes

import concourse.mybir as mybir
import concourse.tile as tile
from concourse import bacc
from concourse.bass_utils import run_bass_kernel_spmd
from concourse.masks import make_identity

S = 2048            # sequence
D = 2048            # hidden
NH = 16             # total heads
DH = 128            # head dim
HG = 4              # heads per core
GQ = HG * DH        # 512: per-core q/k/v feature width
B = 2
NCORES = 8
ROPE_THETA = 10000.0
SCALE = 1.0 / math.sqrt(DH)
SLAB = 512          # qkv sequence slab width
XSC = 16.0          # fp8 pre-scale for x
WSC = 512.0         # fp8 pre-scale for Wqkv / Wo
QSC = 16.0          # fp8 pre-scale for roped q/k (folded into cos/sin tables)
CSC = 16.0          # fp8 pre-scale for attention-out ct (folded into tones)
INV_SC = 1.0 / (XSC * WSC)
EXP_SCALE = SCALE / (QSC * QSC)
P3_SCALE = 1.0 / (WSC * CSC)   # applied host-side
F32 = mybir.dt.float32
BF16 = mybir.dt.bfloat16
F8 = mybir.dt.float8e4
MULT = mybir.AluOpType.mult
ADD = mybir.AluOpType.add
SUB = mybir.AluOpType.subtract
DR = mybir.MatmulPerfMode.DoubleRow
NB = S // 128       # 16 k/q blocks
IDENT = mybir.ActivationFunctionType.Identity
EXPF = mybir.ActivationFunctionType.Exp


def build_nc(reps=1, knobs=None):
    kn = {"sps": 4, "mmp": 3, "expb": 1, "wqb": 2, "qkb": 2, "stg": 2, "obp": 4}
    if knobs:
        kn.update(knobs)
    nc = bacc.Bacc(None, target_bir_lowering=False)
    # x^T hi/lo, slab-major pack: [p, ns, kc2, i, s]
    xh = nc.dram_tensor("xh", [128, 4, 8, 2, SLAB], F8, kind="ExternalInput")
    xl = nc.dram_tensor("xl", [128, 4, 8, 2, SLAB], F8, kind="ExternalInput")
    # per-mt packed qk weights: [mt, p, kc2, i, m]
    wqkh = nc.dram_tensor("wqkh", [8, 128, 8, 2, 128], F8, kind="ExternalInput")
    wvh = nc.dram_tensor("wvh", [8, 128, 2, GQ], F8, kind="ExternalInput")
    wvl = nc.dram_tensor("wvl", [8, 128, 2, GQ], F8, kind="ExternalInput")
    # out-proj fp8 packs: wo1[kh] = (hi,lo) pairs; wo2[g] = hi head-pair packs
    wo1 = nc.dram_tensor("wo1", [HG, 128, 2, D], F8, kind="ExternalInput")
    wo2 = nc.dram_tensor("wo2", [2, 128, 2, D], F8, kind="ExternalInput")
    bqkt = nc.dram_tensor("bqkt", [128, 8], F32, kind="ExternalInput")
    bvc = nc.dram_tensor("bvc", [128, HG], F32, kind="ExternalInput")  # v bias * CSC
    cost = nc.dram_tensor("cost", [128, S], BF16, kind="ExternalInput")    # cos^T * QSC
    sinrt = nc.dram_tensor("sinrt", [128, S], BF16, kind="ExternalInput")  # sin^T * QSC, rot sign
    maskd = nc.dram_tensor("maskd", [128, 128], BF16, kind="ExternalInput")  # triu 0/1 keep-mask
    tonesd = nc.dram_tensor("tonesd", [128, 1], BF16, kind="ExternalInput")  # 1/CSC
    outt = nc.dram_tensor("outt", [16, 128, S], BF16, kind="ExternalOutput")
    lrt = nc.dram_tensor("lrt", [HG, 1, 16, 128], F32)  # recip bounce: [16,128] -> [1,2048]

    with tile.TileContext(nc) as tc, ExitStack() as top:
        g = top.enter_context(tc.tile_pool(name="glob", bufs=1))
        tcos = g.tile([128, S], BF16)
        tsin = g.tile([128, S], BF16)
        tmask = g.tile([128, 128], BF16)
        ident_f = g.tile([128, 128], F32)
        make_identity(nc, ident_f[:])
        tbqkt = g.tile([128, 8], F32)
        tbvc = g.tile([128, HG], F32)
        tones = g.tile([128, 1], BF16)
        tinv = g.tile([128, 1], F32)
        nc.vector.memset(tinv[:], INV_SC)

        # Whole-kernel residents
        res = top.enter_context(tc.tile_pool(name="res", bufs=1))
        vres = []  # 16 V k-block tiles [128(seq), GQ] bf16
        for t in range(NB):
            vres.append(res.tile([128, GQ], BF16, tag=f"v{t}", name=f"v{t}"))
        cth = {}
        for h in range(HG):
            for gq in range(4):
                cth[(h, gq)] = res.tile([128, 512], F8, tag=f"cth_{h}_{gq}",
                                        name=f"cth_{h}_{gq}")
        ctl = [res.tile([128, HG, 512], F8, tag=f"ctl{gq}", name=f"ctl{gq}")
               for gq in range(4)]

        for _rep in range(reps):
          phB = ExitStack()
          mmp = phB.enter_context(tc.tile_pool(name="mmp", bufs=kn["mmp"], space="PSUM"))
          sps = phB.enter_context(tc.tile_pool(name="sps", bufs=kn["sps"], space="PSUM"))
          smps = phB.enter_context(tc.tile_pool(name="smps", bufs=1, space="PSUM"))
          expp = phB.enter_context(tc.tile_pool(name="expp", bufs=kn["expb"]))
          lrp = phB.enter_context(tc.tile_pool(name="lrp", bufs=2))
          rbp = phB.enter_context(tc.tile_pool(name="rbp", bufs=2))
          ctsp = phB.enter_context(tc.tile_pool(name="ctsp", bufs=2))
          obp = phB.enter_context(tc.tile_pool(name="obp", bufs=kn["obp"]))
          smt = smps.tile([128, 132], F32, tag="sm", name="smt")

          phA = ExitStack()
          wqp = phA.enter_context(tc.tile_pool(name="wqp", bufs=kn["wqb"]))
          qkp = phA.enter_context(tc.tile_pool(name="qkp", bufs=kn["qkb"]))
          stg = phA.enter_context(tc.tile_pool(name="stg", bufs=kn["stg"]))
          xps = ExitStack()
          xp = xps.enter_context(tc.tile_pool(name="xp", bufs=1))
          sec0 = ExitStack()
          xsl = sec0.enter_context(tc.tile_pool(name="xsl", bufs=3))
          wvp = sec0.enter_context(tc.tile_pool(name="wvp", bufs=1))

          # ---- initial DMA order (startup-critical) ----
          wq_tiles = {}  # (h) -> (wqh, wql, wkh, wkl)

          def weights_dma(h):
              tl = []
              for mt, tag in ((h, "wqh"), (4 + h, "wkh")):
                  wt = wqp.tile([128, 8, 2, 128], F8, tag=tag, name=f"{tag}{h}")
                  nc.sync.dma_start(out=wt, in_=wqkh[mt])
                  tl.append(wt)
              wq_tiles[h] = tl

          # head-0 weights + first x slab first
          wt = wqp.tile([128, 8, 2, 128], F8, tag="wqh", name="wqh0")
          nc.sync.dma_start(out=wt, in_=wqkh[0])
          wt2 = wqp.tile([128, 8, 2, 128], F8, tag="wkh", name="wkh0")
          nc.sync.dma_start(out=wt2, in_=wqkh[4])
          wq_tiles[0] = [wt, wt2]
          xres = [xp.tile([128, 8, 2, SLAB], F8, tag=f"x{ns}", name=f"x{ns}")
                  for ns in range(4)]
          nc.sync.dma_start(out=xres[0], in_=xh[:, 0])
          # small consts needed by first psum copies / rope
          nc.sync.dma_start(out=tbqkt, in_=bqkt[:])
          nc.sync.dma_start(out=xres[1], in_=xh[:, 1])
          nc.sync.dma_start(out=tcos, in_=cost[:])
          nc.sync.dma_start(out=tsin, in_=sinrt[:])
          nc.sync.dma_start(out=xres[2], in_=xh[:, 2])
          nc.sync.dma_start(out=tones, in_=tonesd[:])
          nc.sync.dma_start(out=tmask, in_=maskd[:])
          nc.sync.dma_start(out=xres[3], in_=xh[:, 3])
          nc.sync.dma_start(out=tbvc, in_=bvc[:])
          twvh, twvl = [], []
          xlres = {}

          def wv_dmas():
              for kc2 in range(8):
                  wv_t = wvp.tile([128, 2, GQ], F8, tag=f"wvh{kc2}")
                  nc.sync.dma_start(out=wv_t, in_=wvh[kc2])
                  twvh.append(wv_t)
              for kc2 in range(8):
                  wv_t = wvp.tile([128, 2, GQ], F8, tag=f"wvl{kc2}")
                  nc.sync.dma_start(out=wv_t, in_=wvl[kc2])
                  twvl.append(wv_t)

          def xl_dma(ns):
              xt = xsl.tile([128, 8, 2, SLAB], F8, tag="xl", name=f"xl{ns}")
              nc.sync.dma_start(out=xt, in_=xl[:, ns])
              xlres[ns] = xt

          # ---- per-head state ----
          qf8 = {}
          kpair = {}
          expT = {}
          lrec_cur = {}
          recrow = {}

          def chain_qk(h, which, ns):
              wqh_, wkh_ = wq_tiles[h]
              whi = wqh_ if which == "q" else wkh_
              sl = slice(ns * SLAB, (ns + 1) * SLAB)
              ps = mmp.tile([128, SLAB], F32, tag="mm")
              for kc2 in range(8):
                  nc.tensor.matmul(ps[:], whi[:, kc2, :, :], xres[ns][:, kc2, :, :],
                                   start=(kc2 == 0), stop=(kc2 == 7), perf_mode=DR)
              st = stg.tile([128, SLAB], BF16, tag="st")
              mt = h if which == "q" else 4 + h
              nc.vector.tensor_scalar(out=st[:], in0=ps[:], scalar1=INV_SC,
                                      scalar2=tbqkt[:, mt:mt + 1],
                                      op0=MULT, op1=ADD)
              # RoPE: out = st*cos + swap(st)*sin_rot   (tables pre-scaled by QSC)
              sw = stg.tile([128, SLAB], BF16, tag="sw")
              nc.sync.dma_start(out=sw[0:64, :], in_=st[64:128, :])
              nc.sync.dma_start(out=sw[64:128, :], in_=st[0:64, :])
              m1 = stg.tile([128, SLAB], BF16, tag="m1")
              nc.vector.tensor_tensor(out=m1[:], in0=st[:], in1=tcos[:, sl], op=MULT)
              nc.vector.tensor_tensor(out=sw[:], in0=sw[:], in1=tsin[:, sl], op=MULT)
              if which == "q":
                  nc.vector.tensor_tensor(out=qf8[h][:, sl], in0=m1[:], in1=sw[:], op=ADD)
              else:
                  kb = stg.tile([128, SLAB], BF16, tag="kb")
                  nc.vector.tensor_tensor(out=kb[:], in0=m1[:], in1=sw[:], op=ADD)
                  nc.scalar.copy(out=kpair[h][:, 0, sl], in_=kb[:])
                  nc.vector.tensor_tensor(out=kpair[h][:, 1, sl], in0=kb[:],
                                          in1=kpair[h][:, 0, sl], op=SUB)

          def v_tile(t):
              ns, sti = divmod(t, 4)
              s0 = sti * 128
              pv = mmp.tile([128, GQ], F32, tag="mm")
              passes = [(xres[ns], twvh), (xlres[ns], twvh), (xres[ns], twvl)]
              for pi, (xt_, wv_) in enumerate(passes):
                  for kc2 in range(8):
                      nc.tensor.matmul(pv[:], xt_[:, kc2, :, s0:s0 + 128],
                                       wv_[kc2][:], start=(pi == 0 and kc2 == 0),
                                       stop=(pi == 2 and kc2 == 7), perf_mode=DR)
              nc.scalar.activation(out=vres[t], in_=pv[:], func=IDENT,
                                   scale=INV_SC)

          def rec_group(h, gq):
              # ship recip(ell) for q-blocks 4g..4g+3 to DRAM and back as a row
              rt = smt[0:4, 0:128]
              nc.tensor.transpose(rt, lrec_cur[h][:, 4 * gq:4 * gq + 4], ident_f[:])
              rts = lrp.tile([4, 128], F32, tag="rts")
              nc.vector.tensor_copy(out=rts[:], in_=rt)
              nc.sync.dma_start(out=lrt[h, 0, 4 * gq:4 * gq + 4, :], in_=rts[:])
              nc.sync.dma_start(out=recrow[h][:, 4 * gq:4 * gq + 4, :],
                                in_=lrt[h, :, 4 * gq:4 * gq + 4, :])

          def denom(h, b):
              # ell[q] for q-block b: sum_k exp tiles via ap-1 matmuls, then recip
              lp = smt[:, 128 + (b % 4):129 + (b % 4)]
              for j in range(b + 1):
                  nc.tensor.matmul(lp, expT[h][j][:, (b - j) * 128:(b - j + 1) * 128],
                                   tones[:], start=(j == 0), stop=(j == b))
              nc.vector.reciprocal(out=lrec_cur[h][:, b:b + 1], in_=lp)

          def sweep2_group(h, gq):
              # ct = (sum_k V^T[k] expS^T[k]) * recip -> split into fp8 hi/lo
              rbs = rbp.tile([128, 512], F32, tag="rbs")
              nc.gpsimd.partition_broadcast(
                  rbs[:], recrow[h][:, 4 * gq:4 * gq + 4, :])
              ct = mmp.tile([128, 512], F32, tag="mm")
              last = 4 * gq + 3
              for j in range(last + 1):
                  if j <= 4 * gq:
                      nc.tensor.matmul(ct[:], vres[j][:, h * 128:(h + 1) * 128],
                                       expT[h][j][:, (4 * gq - j) * 128:(4 * gq - j) * 128 + 512],
                                       start=(j == 0), stop=(j == last))
                  else:
                      w = (4 * gq + 4 - j) * 128
                      nc.tensor.matmul(ct[:, 512 - w:512], vres[j][:, h * 128:(h + 1) * 128],
                                       expT[h][j][:, 0:w], start=False, stop=(j == last))
              ctb = ctsp.tile([128, 512], BF16, tag="ctb")
              nc.vector.tensor_tensor(out=ctb[:], in0=ct[:], in1=rbs[:], op=MULT)
              nc.scalar.activation(out=cth[(h, gq)][:], in_=ctb[:], func=IDENT,
                                   bias=tbvc[:, h:h + 1])
              nc.vector.scalar_tensor_tensor(
                  out=ctl[gq][:, h, :], in0=ctb[:], scalar=tbvc[:, h:h + 1],
                  in1=cth[(h, gq)][:], op0=ADD, op1=SUB)

          def scores_head(h, interleave):
              expT[h] = []
              lrec_cur[h] = lrp.tile([128, 16], F32, tag="lrec", name="lrec", bufs=1)
              recrow[h] = lrp.tile([1, 16, 128], F32, tag="recrow", name="recrow", bufs=1)
              kp = kpair[h]
              qf = qf8[h]
              for i in range(NB):
                  w = (NB - i) * 128
                  ex = expp.tile([128, w], BF16, tag=f"expT{i}", name=f"expT{i}")
                  expT[h].append(ex)
                  for c0 in range(0, w, 512):
                      cw = min(512, w - c0)
                      sp = sps.tile([128, 512], F32, tag="sp")
                      q0 = i * 128 + c0
                      nc.tensor.matmul(
                          sp[:, 0:cw], kp[:, :, i * 128:(i + 1) * 128],
                          qf[:, q0:q0 + cw].unsqueeze(1).broadcast_to((128, 2, cw)),
                          start=True, stop=True, perf_mode=DR)
                      nc.scalar.activation(out=ex[:, c0:c0 + cw], in_=sp[:, 0:cw],
                                           func=EXPF, scale=EXP_SCALE)
                      if c0 == 0:
                          nc.gpsimd.tensor_tensor(out=ex[:, 0:128], in0=ex[:, 0:128],
                                                  in1=tmask[:], op=MULT)
                  if i >= 2:
                      denom(h, i - 2)
                      if i % 4 == 1 and i >= 5:
                          rec_group(h, (i - 5) // 4)
                  interleave(h, i)
              denom(h, NB - 2)
              denom(h, NB - 1)
              rec_group(h, 3)

          def alloc_qk(h):
              qf8[h] = qkp.tile([128, S], F8, tag="qf8", name=f"qf8_{h}")
              kpair[h] = qkp.tile([128, 2, S], F8, tag="kpair", name=f"kpair_{h}")

          wo1t = []
          wo2t = []
          tailp = ExitStack()

          def open_wop():
              xps.close()
              wop = tailp.enter_context(tc.tile_pool(name="wop", bufs=1))
              for kh in range(HG):
                  wt_ = wop.tile([128, 2, D], F8, tag=f"wo1_{kh}", name=f"wo1_{kh}")
                  nc.sync.dma_start(out=wt_, in_=wo1[kh])
                  wo1t.append(wt_)
              for gp in range(2):
                  wt_ = wop.tile([128, 2, D], F8, tag=f"wo2_{gp}", name=f"wo2_{gp}")
                  nc.sync.dma_start(out=wt_, in_=wo2[gp])
                  wo2t.append(wt_)

          def p3_mt(gq, mt):
              op = mmp.tile([128, 512], F32, tag="mm")
              for kh in range(HG):
                  nc.tensor.matmul(
                      op[:], wo1t[kh][:, :, mt * 128:(mt + 1) * 128],
                      cth[(kh, gq)][:].unsqueeze(1).broadcast_to((128, 2, 512)),
                      start=(kh == 0), stop=False, perf_mode=DR)
              for gp in range(2):
                  nc.tensor.matmul(
                      op[:], wo2t[gp][:, :, mt * 128:(mt + 1) * 128],
                      ctl[gq][:, 2 * gp:2 * gp + 2, :],
                      start=False, stop=(gp == 1), perf_mode=DR)
              ob = obp.tile([128, 512], BF16, tag="ob")
              if mt % 2 == 0:
                  nc.vector.tensor_copy(out=ob[:], in_=op[:])
              else:
                  nc.scalar.copy(out=ob[:], in_=op[:])
              nc.sync.dma_start(out=outt[mt, :, gq * 512:(gq + 1) * 512], in_=ob[:])

          def mk_sched(h):
              # schedule of extra PE work per score block of head h
              sched = {i: [] for i in range(NB)}
              if h == 0:
                  sched[2].append(lambda: alloc_qk(1))
                  for idx, (which, ns) in enumerate(
                          (w, n) for n in range(4) for w in ("q", "k")):
                      sched[2 + idx].append(
                          lambda w=which, n=ns: chain_qk(1, w, n))
                  sched[2].append(lambda: xl_dma(2))
                  sched[5].append(lambda: xl_dma(3))
                  for t in range(13):           # v tiles 0-12 at blocks 3-15
                      sched[3 + t].append(lambda t=t: v_tile(t))
                  sched[10].append(lambda: weights_dma(2))
              else:
                  if h == 1:
                      for t in (13, 14, 15):
                          sched[0].append(lambda t=t: v_tile(t))
                      sched[0].append(sec0.close)
                  sched[0].append(lambda: sweep2_group(h - 1, 3))
                  if h < 3:
                      sched[2].append(lambda: alloc_qk(h + 1))
                      for idx, (which, ns) in enumerate(
                              (w, n) for n in range(4) for w in ("q", "k")):
                          sched[2 + idx].append(
                              lambda w=which, n=ns: chain_qk(h + 1, w, n))
                      if h + 2 < HG:
                          sched[10].append(lambda: weights_dma(h + 2))
                  if h == 2:
                      sched[6].append(open_wop)
                  if h == 3:
                      for j in range(16):       # p3 gq0 at blocks 8-14
                          sched[8 + min(j // 3, 6)].append(
                              lambda mt=j: p3_mt(0, mt))
                      for j in range(8):        # p3 gq1 first half, blocks 12-15
                          sched[12 + j // 2].append(lambda mt=j: p3_mt(1, mt))
              # own sweeps (recip rows for gq land 2 blocks after rec_group)
              sched[7].append(lambda: sweep2_group(h, 0))
              sched[11].append(lambda: sweep2_group(h, 1))
              sched[15].append(lambda: sweep2_group(h, 2))
              return sched

          def run_sched(sched, h, i):
              for fn in sched[i]:
                  fn()

          # ---- emit sections ----
          alloc_qk(0)
          for ns in range(4):
              chain_qk(0, "q", ns)
              chain_qk(0, "k", ns)
          weights_dma(1)
          wv_dmas()
          xl_dma(0)
          xl_dma(1)
          for h in range(HG):
              sched = mk_sched(h)
              scores_head(h, lambda hh, i, sched=sched: run_sched(sched, hh, i))

          # ---- tail: rest of the output projection ----
          for mt in range(8, 16):
              p3_mt(1, mt)
          sweep2_group(3, 3)
          for mt in range(16):
              p3_mt(2, mt)
          for mt in range(16):
              p3_mt(3, mt)
          tailp.close()
          phA.close()
          phB.close()
    nc.finalize()
    return nc


_NC_CACHE = {}


def _get_nc(reps=1):
    if reps not in _NC_CACHE:
        _NC_CACHE[reps] = build_nc(reps)
    return _NC_CACHE[reps]


def _rope_tables(position_ids_b):
    pos = position_ids_b.astype(np.float32)
    inv_freq = (1.0 / (ROPE_THETA ** (np.arange(0, DH, 2, dtype=np.float32) / np.float32(DH))))
    ang = pos[:, None] * inv_freq[None, :]          # [S, 64]
    emb = np.concatenate([ang, ang], axis=-1)       # [S, 128]
    cosT = np.ascontiguousarray(np.cos(emb).T) * np.float32(QSC)   # [128, S]
    sinT = np.sin(emb).T * np.float32(QSC)
    sin_rot = np.concatenate([-sinT[0:64], sinT[64:128]], axis=0)
    return cosT.astype(ml_dtypes.bfloat16), np.ascontiguousarray(sin_rot).astype(ml_dtypes.bfloat16)


def _make_in_maps(inputs):
    hidden_states = np.asarray(inputs["hidden_states"], dtype=np.float32)
    position_ids = np.asarray(inputs["position_ids"])
    Wqkv = np.asarray(inputs["Wqkv"], dtype=np.float32)
    bqkv = np.asarray(inputs["bqkv"], dtype=np.float32)
    Wo = np.asarray(inputs["Wo"], dtype=np.float32)

    mask = np.triu(np.ones((128, 128), dtype=np.float32)).astype(ml_dtypes.bfloat16)
    tones = np.full((128, 1), 1.0 / CSC, dtype=ml_dtypes.bfloat16)
    tabs = [_rope_tables(np.asarray(position_ids)[b]) for b in range(B)]

    def _hilo(M, sc):
        Ms = M * np.float32(sc)
        hi = Ms.astype(ml_dtypes.float8_e4m3)
        lo = (Ms - hi.astype(np.float32)).astype(ml_dtypes.float8_e4m3)
        return hi, lo

    def _pack_pairs(M):
        # [D, C] -> [8, 128, 2, C] with row r = kc2*256 + i*128 + p
        C = M.shape[1]
        return np.ascontiguousarray(M.reshape(8, 2, 128, C).transpose(0, 2, 1, 3))

    def _pack_x(M):
        # [D, S] -> [128, 4, 8, 2, SLAB] partition-major, slab-major free
        return np.ascontiguousarray(
            M.reshape(8, 2, 128, 4, SLAB).transpose(2, 3, 0, 1, 4))

    xts = []
    for b in range(B):
        hi, lo = _hilo(np.ascontiguousarray(hidden_states[b].T), XSC)
        xts.append((_pack_x(hi.astype(np.float32)).astype(ml_dtypes.float8_e4m3),
                    _pack_x(lo.astype(np.float32)).astype(ml_dtypes.float8_e4m3)))

    in_maps = []
    for c in range(NCORES):
        b, hg = divmod(c, HG)
        qcols = slice(hg * GQ, (hg + 1) * GQ)
        kcols = slice(D + hg * GQ, D + (hg + 1) * GQ)
        vcols = slice(2 * D + hg * GQ, 2 * D + (hg + 1) * GQ)
        wqk_c = np.ascontiguousarray(np.concatenate([Wqkv[:, qcols], Wqkv[:, kcols]], axis=1))
        qk_h, qk_l = _hilo(wqk_c, WSC)
        # per-mt packing: [8(mt), 128(p), 8(kc2), 2(i), 128(m)]
        def _pack_mt(M8):
            P = _pack_pairs(M8.astype(np.float32))          # [8, 128, 2, 1024]
            P = P.reshape(8, 128, 2, 8, 128)                 # [kc2, p, i, mt, m]
            return np.ascontiguousarray(P.transpose(3, 1, 0, 2, 4)).astype(ml_dtypes.float8_e4m3)
        wqkh_c = _pack_mt(qk_h)
        wqkl_c = _pack_mt(qk_l)
        wv_c = np.ascontiguousarray(Wqkv[:, vcols])
        v_h, v_l = _hilo(wv_c, WSC)
        wvh_c = _pack_pairs(v_h.astype(np.float32)).astype(ml_dtypes.float8_e4m3)
        wvl_c = _pack_pairs(v_l.astype(np.float32)).astype(ml_dtypes.float8_e4m3)
        # out-proj fp8 packs
        wo_c = np.ascontiguousarray(Wo[hg * GQ:(hg + 1) * GQ, :])   # [512, D]
        wo_h, wo_l = _hilo(wo_c, WSC)
        wo_h = wo_h.astype(np.float32)
        wo_l = wo_l.astype(np.float32)
        wo1_c = np.empty((HG, 128, 2, D), np.float32)
        for kh in range(HG):
            wo1_c[kh, :, 0, :] = wo_h[kh * 128:(kh + 1) * 128, :]
            wo1_c[kh, :, 1, :] = wo_l[kh * 128:(kh + 1) * 128, :]
        wo2_c = np.empty((2, 128, 2, D), np.float32)
        for gp in range(2):
            wo2_c[gp, :, 0, :] = wo_h[gp * 256:gp * 256 + 128, :]
            wo2_c[gp, :, 1, :] = wo_h[gp * 256 + 128:gp * 256 + 256, :]
        bqk_c = np.concatenate([bqkv[qcols], bqkv[kcols]]).reshape(8, 128).T
        bvc_c = np.ascontiguousarray(bqkv[vcols].reshape(HG, 128).T * np.float32(CSC))
        cosT, sin_rot = tabs[b]
        in_maps.append({
            "xh": xts[b][0], "xl": xts[b][1],
            "wqkh": wqkh_c, "wqkl": wqkl_c, "wvh": wvh_c, "wvl": wvl_c,
            "wo1": wo1_c.astype(ml_dtypes.float8_e4m3),
            "wo2": wo2_c.astype(ml_dtypes.float8_e4m3),
            "bqkt": np.ascontiguousarray(bqk_c),
            "bvc": bvc_c,
            "cost": cosT, "sinrt": sin_rot, "maskd": mask,
            "tonesd": tones,
        })
    return in_maps


def kernel(hidden_states, position_ids, Wqkv, bqkv, Wo, bo, _reps=1):
    bo = np.asarray(bo, dtype=np.float32)
    in_maps = _make_in_maps({
        "hidden_states": hidden_states, "position_ids": position_ids,
        "Wqkv": Wqkv, "bqkv": bqkv, "Wo": Wo, "bo": bo,
    })
    nc = _get_nc(_reps)
    res = run_bass_kernel_spmd(nc, in_maps, core_ids=list(range(NCORES)))

    out = np.empty((B, S, D), dtype=np.float32)
    for b in range(B):
        acc = res.results[b * HG]["outt"].reshape(D, S).astype(np.float32).copy()
        for hg in range(1, HG):
            acc += res.results[b * HG + hg]["outt"].reshape(D, S).astype(np.float32)
        out[b] = acc.T * np.float32(P3_SCALE) + bo[None, :]
    return out


# revision 9
# speedup vs baseline: 1.1909x; 1.0426x over previous
"""Trainium2 Bass kernel for CustomRoPEAttention (B=2, S=2048, H=16, Dh=128).

Sharding: 8 cores = 2 batches x 4 head-groups (4 heads/core).

Head-pipelined structure: per head h, QKV^T projection (fp8 hi/lo DoubleRow
matmuls) + RoPE, then transposed-layout causal attention for that head while
later heads' projections stream -- this overlaps the ACT-engine exp work with
PE-engine matmul work across the whole kernel instead of serializing phases.

fp8 DoubleRow "dup trick": scores use stationary (k_hi,k_lo) pairs against a
broadcast (step-0) fp8 q moving operand, and the output projection uses
(wo_hi,wo_lo) pairs against broadcast ct_hi plus a wo_hi x ct_lo correction --
half / 0.75x the bf16 PE time at first-order-exact precision.

Host sums the 4 partial (transposed) output projections per batch.

Self-contained: hardcodes shapes from the problem spec.
"""
import math
from contextlib import ExitStack

import numpy as np
import ml_dtypes

import concourse.mybir as mybir
import concourse.tile as tile
from concourse import bacc
from concourse.bass_utils import run_bass_kernel_spmd
from concourse.masks import make_identity

S = 2048            # sequence
D = 2048            # hidden
NH = 16             # total heads
DH = 128            # head dim
HG = 4              # heads per core
GQ = HG * DH        # 512: per-core q/k/v feature width
B = 2
NCORES = 8
ROPE_THETA = 10000.0
SCALE = 1.0 / math.sqrt(DH)
SLAB = 512          # qkv sequence slab width
XSC = 16.0          # fp8 pre-scale for x
WSC = 512.0         # fp8 pre-scale for Wqkv / Wo
QSC = 16.0          # fp8 pre-scale for roped q/k (folded into cos/sin tables)
CSC = 16.0          # fp8 pre-scale for attention-out ct (folded into tones)
INV_SC = 1.0 / (XSC * WSC)
EXP_SCALE = SCALE / (QSC * QSC)
P3_SCALE = 1.0 / (WSC * CSC)   # applied host-side
F32 = mybir.dt.float32
BF16 = mybir.dt.bfloat16
F8 = mybir.dt.float8e4
MULT = mybir.AluOpType.mult
ADD = mybir.AluOpType.add
SUB = mybir.AluOpType.subtract
DR = mybir.MatmulPerfMode.DoubleRow
NB = S // 128       # 16 k/q blocks
IDENT = mybir.ActivationFunctionType.Identity
EXPF = mybir.ActivationFunctionType.Exp


def build_nc(reps=1, knobs=None):
    kn = {"sps": 4, "mmp": 3, "expb": 1, "wqb": 2, "qkb": 2, "stg": 2, "obp": 6}
    if knobs:
        kn.update(knobs)
    nc = bacc.Bacc(None, target_bir_lowering=False)
    # x^T hi/lo, slab-major pack: [p, ns, kc2, i, s]
    xh = nc.dram_tensor("xh", [128, 4, 8, 2, SLAB], F8, kind="ExternalInput")
    xl = nc.dram_tensor("xl", [128, 4, 8, 2, SLAB], F8, kind="ExternalInput")
    # per-mt packed qk weights: [mt, p, kc2, i, m]
    wqkh = nc.dram_tensor("wqkh", [8, 128, 8, 2, 128], F8, kind="ExternalInput")
    wvh = nc.dram_tensor("wvh", [8, 128, 2, GQ], F8, kind="ExternalInput")
    wvl = nc.dram_tensor("wvl", [8, 128, 2, GQ], F8, kind="ExternalInput")
    # out-proj fp8 packs: wo1[kh] = (hi,lo) pairs; wo2[g] = hi head-pair packs
    wo1 = nc.dram_tensor("wo1", [HG, 128, 2, D], F8, kind="ExternalInput")
    wo2 = nc.dram_tensor("wo2", [2, 128, 2, D], F8, kind="ExternalInput")
    bqkt = nc.dram_tensor("bqkt", [128, 8], F32, kind="ExternalInput")
    bvc = nc.dram_tensor("bvc", [128, HG], F32, kind="ExternalInput")  # v bias * CSC
    cost = nc.dram_tensor("cost", [128, S], BF16, kind="ExternalInput")    # cos^T * QSC
    sinrt = nc.dram_tensor("sinrt", [128, S], BF16, kind="ExternalInput")  # sin^T * QSC, rot sign
    maskd = nc.dram_tensor("maskd", [128, 128], BF16, kind="ExternalInput")  # triu 0/1 keep-mask
    tonesd = nc.dram_tensor("tonesd", [128, 1], BF16, kind="ExternalInput")  # 1/CSC
    outt = nc.dram_tensor("outt", [16, 128, S], BF16, kind="ExternalOutput")
    lrt = nc.dram_tensor("lrt", [HG, 1, 16, 128], F32)  # recip bounce: [16,128] -> [1,2048]

    with tile.TileContext(nc) as tc, ExitStack() as top:
        g = top.enter_context(tc.tile_pool(name="glob", bufs=1))
        tcos = g.tile([128, S], BF16)
        tsin = g.tile([128, S], BF16)
        tmask = g.tile([128, 128], BF16)
        ident_f = g.tile([128, 128], F32)
        make_identity(nc, ident_f[:])
        tbqkt = g.tile([128, 8], F32)
        tbvc = g.tile([128, HG], F32)
        tones = g.tile([128, 1], BF16)
        tinv = g.tile([128, 1], F32)
        nc.vector.memset(tinv[:], INV_SC)

        # Whole-kernel residents
        res = top.enter_context(tc.tile_pool(name="res", bufs=1))
        vres = []  # 16 V k-block tiles [128(seq), GQ] bf16
        for t in range(NB):
            vres.append(res.tile([128, GQ], BF16, tag=f"v{t}", name=f"v{t}"))
        cth = {}
        for h in range(HG):
            for gq in range(4):
                cth[(h, gq)] = res.tile([128, 512], F8, tag=f"cth_{h}_{gq}",
                                        name=f"cth_{h}_{gq}")
        ctl = [res.tile([128, HG, 512], F8, tag=f"ctl{gq}", name=f"ctl{gq}")
               for gq in range(4)]

        for _rep in range(reps):
          phB = ExitStack()
          mmp = phB.enter_context(tc.tile_pool(name="mmp", bufs=kn["mmp"], space="PSUM"))
          sps = phB.enter_context(tc.tile_pool(name="sps", bufs=kn["sps"], space="PSUM"))
          smps = phB.enter_context(tc.tile_pool(name="smps", bufs=1, space="PSUM"))
          expp = phB.enter_context(tc.tile_pool(name="expp", bufs=kn["expb"]))
          lrp = phB.enter_context(tc.tile_pool(name="lrp", bufs=2))
          rbp = phB.enter_context(tc.tile_pool(name="rbp", bufs=2))
          ctsp = phB.enter_context(tc.tile_pool(name="ctsp", bufs=2))
          obp = phB.enter_context(tc.tile_pool(name="obp", bufs=kn["obp"]))
          smt = smps.tile([128, 132], F32, tag="sm", name="smt")

          phA = ExitStack()
          wqp = phA.enter_context(tc.tile_pool(name="wqp", bufs=kn["wqb"]))
          qkp = phA.enter_context(tc.tile_pool(name="qkp", bufs=kn["qkb"]))
          stg = phA.enter_context(tc.tile_pool(name="stg", bufs=kn["stg"]))
          xps = ExitStack()
          xp = xps.enter_context(tc.tile_pool(name="xp", bufs=1))
          sec0 = ExitStack()
          xsl = sec0.enter_context(tc.tile_pool(name="xsl", bufs=3))
          wvp = sec0.enter_context(tc.tile_pool(name="wvp", bufs=1))

          # ---- initial DMA order (startup-critical) ----
          wq_tiles = {}  # (h) -> (wqh, wql, wkh, wkl)

          def weights_dma(h):
              tl = []
              for mt, tag in ((h, "wqh"), (4 + h, "wkh")):
                  wt = wqp.tile([128, 8, 2, 128], F8, tag=tag, name=f"{tag}{h}")
                  nc.sync.dma_start(out=wt, in_=wqkh[mt])
                  tl.append(wt)
              wq_tiles[h] = tl

          # head-0 weights + first x slab first
          wt = wqp.tile([128, 8, 2, 128], F8, tag="wqh", name="wqh0")
          nc.sync.dma_start(out=wt, in_=wqkh[0])
          wt2 = wqp.tile([128, 8, 2, 128], F8, tag="wkh", name="wkh0")
          nc.sync.dma_start(out=wt2, in_=wqkh[4])
          wq_tiles[0] = [wt, wt2]
          xres = [xp.tile([128, 8, 2, SLAB], F8, tag=f"x{ns}", name=f"x{ns}")
                  for ns in range(4)]
          nc.sync.dma_start(out=xres[0], in_=xh[:, 0])
          # small consts needed by first psum copies / rope
          nc.sync.dma_start(out=tbqkt, in_=bqkt[:])
          nc.sync.dma_start(out=xres[1], in_=xh[:, 1])
          nc.sync.dma_start(out=tcos, in_=cost[:])
          nc.sync.dma_start(out=tsin, in_=sinrt[:])
          nc.sync.dma_start(out=xres[2], in_=xh[:, 2])
          nc.sync.dma_start(out=tones, in_=tonesd[:])
          nc.sync.dma_start(out=tmask, in_=maskd[:])
          nc.sync.dma_start(out=xres[3], in_=xh[:, 3])
          nc.sync.dma_start(out=tbvc, in_=bvc[:])
          twvh, twvl = [], []
          xlres = {}

          def wv_dmas():
              for kc2 in range(8):
                  wv_t = wvp.tile([128, 2, GQ], F8, tag=f"wvh{kc2}")
                  nc.sync.dma_start(out=wv_t, in_=wvh[kc2])
                  twvh.append(wv_t)
              for kc2 in range(8):
                  wv_t = wvp.tile([128, 2, GQ], F8, tag=f"wvl{kc2}")
                  nc.sync.dma_start(out=wv_t, in_=wvl[kc2])
                  twvl.append(wv_t)

          def xl_dma(ns):
              xt = xsl.tile([128, 8, 2, SLAB], F8, tag="xl", name=f"xl{ns}")
              nc.sync.dma_start(out=xt, in_=xl[:, ns])
              xlres[ns] = xt

          # ---- per-head state ----
          qf8 = {}
          kpair = {}
          expT = {}
          lrec_cur = {}
          recrow = {}

          def chain_qk(h, which, ns):
              wqh_, wkh_ = wq_tiles[h]
              whi = wqh_ if which == "q" else wkh_
              sl = slice(ns * SLAB, (ns + 1) * SLAB)
              ps = mmp.tile([128, SLAB], F32, tag="mm")
              for kc2 in range(8):
                  nc.tensor.matmul(ps[:], whi[:, kc2, :, :], xres[ns][:, kc2, :, :],
                                   start=(kc2 == 0), stop=(kc2 == 7), perf_mode=DR)
              st = stg.tile([128, SLAB], BF16, tag="st")
              mt = h if which == "q" else 4 + h
              nc.vector.tensor_scalar(out=st[:], in0=ps[:], scalar1=INV_SC,
                                      scalar2=tbqkt[:, mt:mt + 1],
                                      op0=MULT, op1=ADD)
              # RoPE: out = st*cos + swap(st)*sin_rot   (tables pre-scaled by QSC)
              sw = stg.tile([128, SLAB], BF16, tag="sw")
              nc.sync.dma_start(out=sw[0:64, :], in_=st[64:128, :])
              nc.sync.dma_start(out=sw[64:128, :], in_=st[0:64, :])
              m1 = stg.tile([128, SLAB], BF16, tag="m1")
              nc.vector.tensor_tensor(out=m1[:], in0=st[:], in1=tcos[:, sl], op=MULT)
              nc.vector.tensor_tensor(out=sw[:], in0=sw[:], in1=tsin[:, sl], op=MULT)
              if which == "q":
                  nc.vector.tensor_tensor(out=qf8[h][:, sl], in0=m1[:], in1=sw[:], op=ADD)
              else:
                  kb = stg.tile([128, SLAB], BF16, tag="kb")
                  nc.vector.tensor_tensor(out=kb[:], in0=m1[:], in1=sw[:], op=ADD)
                  nc.scalar.copy(out=kpair[h][:, 0, sl], in_=kb[:])
                  nc.vector.tensor_tensor(out=kpair[h][:, 1, sl], in0=kb[:],
                                          in1=kpair[h][:, 0, sl], op=SUB)

          def v_tile(t):
              ns, sti = divmod(t, 4)
              s0 = sti * 128
              pv = mmp.tile([128, GQ], F32, tag="mm")
              passes = [(xres[ns], twvh), (xlres[ns], twvh), (xres[ns], twvl)]
              for pi, (xt_, wv_) in enumerate(passes):
                  for kc2 in range(8):
                      nc.tensor.matmul(pv[:], xt_[:, kc2, :, s0:s0 + 128],
                                       wv_[kc2][:], start=(pi == 0 and kc2 == 0),
                                       stop=(pi == 2 and kc2 == 7), perf_mode=DR)
              nc.scalar.activation(out=vres[t], in_=pv[:], func=IDENT,
                                   scale=INV_SC)

          def rec_group(h, gq):
              # ship recip(ell) for q-blocks 4g..4g+3 to DRAM and back as a row
              rt = smt[0:4, 0:128]
              nc.tensor.transpose(rt, lrec_cur[h][:, 4 * gq:4 * gq + 4], ident_f[:])
              rts = lrp.tile([4, 128], F32, tag="rts")
              nc.vector.tensor_copy(out=rts[:], in_=rt)
              nc.sync.dma_start(out=lrt[h, 0, 4 * gq:4 * gq + 4, :], in_=rts[:])
              nc.sync.dma_start(out=recrow[h][:, 4 * gq:4 * gq + 4, :],
                                in_=lrt[h, :, 4 * gq:4 * gq + 4, :])

          def denom(h, b):
              # ell[q] for q-block b: sum_k exp tiles via ap-1 matmuls, then recip
              lp = smt[:, 128 + (b % 4):129 + (b % 4)]
              for j in range(b + 1):
                  nc.tensor.matmul(lp, expT[h][j][:, (b - j) * 128:(b - j + 1) * 128],
                                   tones[:], start=(j == 0), stop=(j == b))
              nc.vector.reciprocal(out=lrec_cur[h][:, b:b + 1], in_=lp)

          def sweep2_group(h, gq, split_at=None, mid_fn=None):
              # ct = (sum_k V^T[k] expS^T[k]) * recip -> split into fp8 hi/lo
              rbs = rbp.tile([128, 512], F32, tag="rbs")
              nc.gpsimd.partition_broadcast(
                  rbs[:], recrow[h][:, 4 * gq:4 * gq + 4, :])
              ct = mmp.tile([128, 512], F32, tag="mm")
              last = 4 * gq + 3
              for j in range(last + 1):
                  if split_at is not None and j == split_at:
                      mid_fn()
                  if j <= 4 * gq:
                      nc.tensor.matmul(ct[:], vres[j][:, h * 128:(h + 1) * 128],
                                       expT[h][j][:, (4 * gq - j) * 128:(4 * gq - j) * 128 + 512],
                                       start=(j == 0), stop=(j == last))
                  else:
                      w = (4 * gq + 4 - j) * 128
                      nc.tensor.matmul(ct[:, 512 - w:512], vres[j][:, h * 128:(h + 1) * 128],
                                       expT[h][j][:, 0:w], start=False, stop=(j == last))
              ctb = ctsp.tile([128, 512], BF16, tag="ctb")
              nc.vector.tensor_tensor(out=ctb[:], in0=ct[:], in1=rbs[:], op=MULT)
              nc.gpsimd.tensor_scalar(out=cth[(h, gq)][:], in0=ctb[:],
                                      scalar1=tbvc[:, h:h + 1], scalar2=None,
                                      op0=ADD)
              nc.vector.scalar_tensor_tensor(
                  out=ctl[gq][:, h, :], in0=ctb[:], scalar=tbvc[:, h:h + 1],
                  in1=cth[(h, gq)][:], op0=ADD, op1=SUB)

          def scores_head(h, interleave):
              expT[h] = []
              lrec_cur[h] = lrp.tile([128, 16], F32, tag="lrec", name="lrec", bufs=1)
              recrow[h] = lrp.tile([1, 16, 128], F32, tag="recrow", name="recrow", bufs=1)
              kp = kpair[h]
              qf = qf8[h]
              for i in range(NB):
                  w = (NB - i) * 128
                  ex = expp.tile([128, w], BF16, tag=f"expT{i}", name=f"expT{i}")
                  expT[h].append(ex)
                  for c0 in range(0, w, 512):
                      cw = min(512, w - c0)
                      sp = sps.tile([128, 512], F32, tag="sp")
                      q0 = i * 128 + c0
                      nc.tensor.matmul(
                          sp[:, 0:cw], kp[:, :, i * 128:(i + 1) * 128],
                          qf[:, q0:q0 + cw].unsqueeze(1).broadcast_to((128, 2, cw)),
                          start=True, stop=True, perf_mode=DR)
                      nc.scalar.activation(out=ex[:, c0:c0 + cw], in_=sp[:, 0:cw],
                                           func=EXPF, scale=EXP_SCALE)
                      if c0 == 0:
                          nc.vector.tensor_tensor(out=ex[:, 0:128], in0=ex[:, 0:128],
                                                  in1=tmask[:], op=MULT)
                  if i >= 2:
                      denom(h, i - 2)
                      if i % 4 == 1 and i >= 5:
                          rec_group(h, (i - 5) // 4)
                  interleave(h, i)
              denom(h, NB - 2)
              denom(h, NB - 1)
              rec_group(h, 3)

          def alloc_qk(h):
              qf8[h] = qkp.tile([128, S], F8, tag="qf8", name=f"qf8_{h}")
              kpair[h] = qkp.tile([128, 2, S], F8, tag="kpair", name=f"kpair_{h}")

          wo1t = []
          wo2t = []
          tailp = ExitStack()

          def open_wop():
              xps.close()
              wop = tailp.enter_context(tc.tile_pool(name="wop", bufs=1))
              for kh in range(HG):
                  wt_ = wop.tile([128, 2, D], F8, tag=f"wo1_{kh}", name=f"wo1_{kh}")
                  nc.sync.dma_start(out=wt_, in_=wo1[kh])
                  wo1t.append(wt_)
              for gp in range(2):
                  wt_ = wop.tile([128, 2, D], F8, tag=f"wo2_{gp}", name=f"wo2_{gp}")
                  nc.sync.dma_start(out=wt_, in_=wo2[gp])
                  wo2t.append(wt_)

          def p3_mt(gq, mt):
              op = mmp.tile([128, 512], F32, tag="mm")
              for kh in range(HG):
                  nc.tensor.matmul(
                      op[:], wo1t[kh][:, :, mt * 128:(mt + 1) * 128],
                      cth[(kh, gq)][:].unsqueeze(1).broadcast_to((128, 2, 512)),
                      start=(kh == 0), stop=False, perf_mode=DR)
              for gp in range(2):
                  nc.tensor.matmul(
                      op[:], wo2t[gp][:, :, mt * 128:(mt + 1) * 128],
                      ctl[gq][:, 2 * gp:2 * gp + 2, :],
                      start=False, stop=(gp == 1), perf_mode=DR)
              ob = obp.tile([128, 512], BF16, tag="ob")
              if mt % 2 == 0:
                  nc.vector.tensor_copy(out=ob[:], in_=op[:])
              else:
                  nc.scalar.copy(out=ob[:], in_=op[:])
              nc.sync.dma_start(out=outt[mt, :, gq * 512:(gq + 1) * 512], in_=ob[:])

          def mk_sched(h):
              # schedule of extra PE work per score block of head h
              sched = {i: [] for i in range(NB)}
              if h == 0:
                  sched[2].append(lambda: alloc_qk(1))
                  for idx, (which, ns) in enumerate(
                          (w, n) for n in range(4) for w in ("q", "k")):
                      sched[2 + idx].append(
                          lambda w=which, n=ns: chain_qk(1, w, n))
                  sched[2].append(lambda: xl_dma(2))
                  sched[5].append(lambda: xl_dma(3))
                  for t in range(13):           # v tiles 0-12 at blocks 3-15
                      sched[3 + t].append(lambda t=t: v_tile(t))
                  sched[10].append(lambda: weights_dma(2))
              else:
                  if h == 1:
                      def _mid():
                          for t in (13, 14, 15):
                              v_tile(t)
                          sec0.close()
                      sched[0].append(
                          lambda: sweep2_group(0, 3, split_at=13, mid_fn=_mid))
                  else:
                      sched[0].append(lambda: sweep2_group(h - 1, 3))
                  if h < 3:
                      sched[2].append(lambda: alloc_qk(h + 1))
                      for idx, (which, ns) in enumerate(
                              (w, n) for n in range(4) for w in ("q", "k")):
                          sched[2 + idx].append(
                              lambda w=which, n=ns: chain_qk(h + 1, w, n))
                      if h + 2 < HG:
                          sched[10].append(lambda: weights_dma(h + 2))
                  if h == 2:
                      sched[6].append(open_wop)
                  if h == 3:
                      for j in range(16):       # p3 gq0 at blocks 8-14
                          sched[8 + min(j // 3, 6)].append(
                              lambda mt=j: p3_mt(0, mt))
                      for j in range(8):        # p3 gq1 first half, blocks 12-15
                          sched[12 + j // 2].append(lambda mt=j: p3_mt(1, mt))
              # own sweeps (recip rows for gq land 2 blocks after rec_group)
              sched[7].append(lambda: sweep2_group(h, 0))
              sched[11].append(lambda: sweep2_group(h, 1))
              sched[15].append(lambda: sweep2_group(h, 2))
              return sched

          def run_sched(sched, h, i):
              for fn in sched[i]:
                  fn()

          # ---- emit sections ----
          alloc_qk(0)
          for ns in range(4):
              chain_qk(0, "q", ns)
              chain_qk(0, "k", ns)
          weights_dma(1)
          wv_dmas()
          xl_dma(0)
          xl_dma(1)
          for h in range(HG):
              sched = mk_sched(h)
              scores_head(h, lambda hh, i, sched=sched: run_sched(sched, hh, i))

          # ---- tail: rest of the output projection ----
          for mt in range(8, 16):
              p3_mt(1, mt)
          sweep2_group(3, 3)
          for mt in range(16):
              p3_mt(2, mt)
          for mt in range(16):
              p3_mt(3, mt)
          tailp.close()
          phA.close()
          phB.close()
    nc.finalize()
    return nc


_NC_CACHE = {}


def _get_nc(reps=1):
    if reps not in _NC_CACHE:
        _NC_CACHE[reps] = build_nc(reps)
    return _NC_CACHE[reps]


def _rope_tables(position_ids_b):
    pos = position_ids_b.astype(np.float32)
    inv_freq = (1.0 / (ROPE_THETA ** (np.arange(0, DH, 2, dtype=np.float32) / np.float32(DH))))
    ang = pos[:, None] * inv_freq[None, :]          # [S, 64]
    emb = np.concatenate([ang, ang], axis=-1)       # [S, 128]
    cosT = np.ascontiguousarray(np.cos(emb).T) * np.float32(QSC)   # [128, S]
    sinT = np.sin(emb).T * np.float32(QSC)
    sin_rot = np.concatenate([-sinT[0:64], sinT[64:128]], axis=0)
    return cosT.astype(ml_dtypes.bfloat16), np.ascontiguousarray(sin_rot).astype(ml_dtypes.bfloat16)


def _make_in_maps(inputs):
    hidden_states = np.asarray(inputs["hidden_states"], dtype=np.float32)
    position_ids = np.asarray(inputs["position_ids"])
    Wqkv = np.asarray(inputs["Wqkv"], dtype=np.float32)
    bqkv = np.asarray(inputs["bqkv"], dtype=np.float32)
    Wo = np.asarray(inputs["Wo"], dtype=np.float32)

    mask = np.triu(np.ones((128, 128), dtype=np.float32)).astype(ml_dtypes.bfloat16)
    tones = np.full((128, 1), 1.0 / CSC, dtype=ml_dtypes.bfloat16)
    tabs = [_rope_tables(np.asarray(position_ids)[b]) for b in range(B)]

    def _hilo(M, sc):
        Ms = M * np.float32(sc)
        hi = Ms.astype(ml_dtypes.float8_e4m3)
        lo = (Ms - hi.astype(np.float32)).astype(ml_dtypes.float8_e4m3)
        return hi, lo

    def _pack_pairs(M):
        # [D, C] -> [8, 128, 2, C] with row r = kc2*256 + i*128 + p
        C = M.shape[1]
        return np.ascontiguousarray(M.reshape(8, 2, 128, C).transpose(0, 2, 1, 3))

    def _pack_x(M):
        # [D, S] -> [128, 4, 8, 2, SLAB] partition-major, slab-major free
        return np.ascontiguousarray(
            M.reshape(8, 2, 128, 4, SLAB).transpose(2, 3, 0, 1, 4))

    xts = []
    for b in range(B):
        hi, lo = _hilo(np.ascontiguousarray(hidden_states[b].T), XSC)
        xts.append((_pack_x(hi.astype(np.float32)).astype(ml_dtypes.float8_e4m3),
                    _pack_x(lo.astype(np.float32)).astype(ml_dtypes.float8_e4m3)))

    in_maps = []
    for c in range(NCORES):
        b, hg = divmod(c, HG)
        qcols = slice(hg * GQ, (hg + 1) * GQ)
        kcols = slice(D + hg * GQ, D + (hg + 1) * GQ)
        vcols = slice(2 * D + hg * GQ, 2 * D + (hg + 1) * GQ)
        wqk_c = np.ascontiguousarray(np.concatenate([Wqkv[:, qcols], Wqkv[:, kcols]], axis=1))
        qk_h, qk_l = _hilo(wqk_c, WSC)
        # per-mt packing: [8(mt), 128(p), 8(kc2), 2(i), 128(m)]
        def _pack_mt(M8):
            P = _pack_pairs(M8.astype(np.float32))          # [8, 128, 2, 1024]
            P = P.reshape(8, 128, 2, 8, 128)                 # [kc2, p, i, mt, m]
            return np.ascontiguousarray(P.transpose(3, 1, 0, 2, 4)).astype(ml_dtypes.float8_e4m3)
        wqkh_c = _pack_mt(qk_h)
        wqkl_c = _pack_mt(qk_l)
        wv_c = np.ascontiguousarray(Wqkv[:, vcols])
        v_h, v_l = _hilo(wv_c, WSC)
        wvh_c = _pack_pairs(v_h.astype(np.float32)).astype(ml_dtypes.float8_e4m3)
        wvl_c = _pack_pairs(v_l.astype(np.float32)).astype(ml_dtypes.float8_e4m3)
        # out-proj fp8 packs
        wo_c = np.ascontiguousarray(Wo[hg * GQ:(hg + 1) * GQ, :])   # [512, D]
        wo_h, wo_l = _hilo(wo_c, WSC)
        wo_h = wo_h.astype(np.float32)
        wo_l = wo_l.astype(np.float32)
        wo1_c = np.empty((HG, 128, 2, D), np.float32)
        for kh in range(HG):
            wo1_c[kh, :, 0, :] = wo_h[kh * 128:(kh + 1) * 128, :]
            wo1_c[kh, :, 1, :] = wo_l[kh * 128:(kh + 1) * 128, :]
        wo2_c = np.empty((2, 128, 2, D), np.float32)
        for gp in range(2):
            wo2_c[gp, :, 0, :] = wo_h[gp * 256:gp * 256 + 128, :]
            wo2_c[gp, :, 1, :] = wo_h[gp * 256 + 128:gp * 256 + 256, :]
        bqk_c = np.concatenate([bqkv[qcols], bqkv[kcols]]).reshape(8, 128).T
        bvc_c = np.ascontiguousarray(bqkv[vcols].reshape(HG, 128).T * np.float32(CSC))
        cosT, sin_rot = tabs[b]
        in_maps.append({
            "xh": xts[b][0], "xl": xts[b][1],
            "wqkh": wqkh_c, "wqkl": wqkl_c, "wvh": wvh_c, "wvl": wvl_c,
            "wo1": wo1_c.astype(ml_dtypes.float8_e4m3),
            "wo2": wo2_c.astype(ml_dtypes.float8_e4m3),
            "bqkt": np.ascontiguousarray(bqk_c),
            "bvc": bvc_c,
            "cost": cosT, "sinrt": sin_rot, "maskd": mask,
            "tonesd": tones,
        })
    return in_maps


def kernel(hidden_states, position_ids, Wqkv, bqkv, Wo, bo, _reps=1):
    bo = np.asarray(bo, dtype=np.float32)
    in_maps = _make_in_maps({
        "hidden_states": hidden_states, "position_ids": position_ids,
        "Wqkv": Wqkv, "bqkv": bqkv, "Wo": Wo, "bo": bo,
    })
    nc = _get_nc(_reps)
    res = run_bass_kernel_spmd(nc, in_maps, core_ids=list(range(NCORES)))

    out = np.empty((B, S, D), dtype=np.float32)
    for b in range(B):
        acc = res.results[b * HG]["outt"].reshape(D, S).astype(np.float32).copy()
        for hg in range(1, HG):
            acc += res.results[b * HG + hg]["outt"].reshape(D, S).astype(np.float32)
        out[b] = acc.T * np.float32(P3_SCALE) + bo[None, :]
    return out


# revision 10
# speedup vs baseline: 1.2212x; 1.0255x over previous
"""Trainium2 Bass kernel for CustomRoPEAttention (B=2, S=2048, H=16, Dh=128).

Sharding: 8 cores = 2 batches x 4 head-groups (4 heads/core).

Head-pipelined structure: per head h, QKV^T projection (fp8 hi/lo DoubleRow
matmuls) + RoPE, then transposed-layout causal attention for that head while
later heads' projections stream -- this overlaps the ACT-engine exp work with
PE-engine matmul work across the whole kernel instead of serializing phases.

fp8 DoubleRow "dup trick": scores use stationary (k_hi,k_lo) pairs against a
broadcast (step-0) fp8 q moving operand, and the output projection uses
(wo_hi,wo_lo) pairs against broadcast ct_hi plus a wo_hi x ct_lo correction --
half / 0.75x the bf16 PE time at first-order-exact precision.

Host sums the 4 partial (transposed) output projections per batch.

Self-contained: hardcodes shapes from the problem spec.
"""
import math
from contextlib import ExitStack

import numpy as np
import ml_dtypes

import concourse.mybir as mybir
import concourse.tile as tile
from concourse import bacc
from concourse.bass_utils import run_bass_kernel_spmd
from concourse.masks import make_identity

S = 2048            # sequence
D = 2048            # hidden
NH = 16             # total heads
DH = 128            # head dim
HG = 4              # heads per core
GQ = HG * DH        # 512: per-core q/k/v feature width
B = 2
NCORES = 8
ROPE_THETA = 10000.0
SCALE = 1.0 / math.sqrt(DH)
SLAB = 512          # qkv sequence slab width
XSC = 16.0          # fp8 pre-scale for x
WSC = 512.0         # fp8 pre-scale for Wqkv / Wo
QSC = 16.0          # fp8 pre-scale for roped q/k (folded into cos/sin tables)
CSC = 16.0          # fp8 pre-scale for attention-out ct (folded into tones)
INV_SC = 1.0 / (XSC * WSC)
EXP_SCALE = SCALE / (QSC * QSC)
P3_SCALE = 1.0 / (WSC * CSC)   # applied host-side
F32 = mybir.dt.float32
BF16 = mybir.dt.bfloat16
F8 = mybir.dt.float8e4
MULT = mybir.AluOpType.mult
ADD = mybir.AluOpType.add
SUB = mybir.AluOpType.subtract
DR = mybir.MatmulPerfMode.DoubleRow
NB = S // 128       # 16 k/q blocks
IDENT = mybir.ActivationFunctionType.Identity
EXPF = mybir.ActivationFunctionType.Exp


def build_nc(reps=1, knobs=None):
    kn = {"sps": 4, "mmp": 3, "expb": 1, "wqb": 2, "qkb": 2, "stg": 2, "obp": 6}
    if knobs:
        kn.update(knobs)
    nc = bacc.Bacc(None, target_bir_lowering=False)
    # x^T hi/lo, slab-major pack: [p, ns, kc2, i, s]
    xh = nc.dram_tensor("xh", [128, 4, 8, 2, SLAB], F8, kind="ExternalInput")
    xl = nc.dram_tensor("xl", [128, 4, 8, 2, SLAB], F8, kind="ExternalInput")
    # per-mt packed qk weights: [mt, p, kc2, i, m]
    wqkh = nc.dram_tensor("wqkh", [8, 128, 8, 2, 128], F8, kind="ExternalInput")
    wvh = nc.dram_tensor("wvh", [8, 128, 2, GQ], F8, kind="ExternalInput")
    wvl = nc.dram_tensor("wvl", [8, 128, 2, GQ], F8, kind="ExternalInput")
    # out-proj fp8 packs: wo1[kh] = (hi,lo) pairs; wo2[g] = hi head-pair packs
    wo1 = nc.dram_tensor("wo1", [HG, 128, 2, D], F8, kind="ExternalInput")
    wo2 = nc.dram_tensor("wo2", [2, 128, 2, D], F8, kind="ExternalInput")
    bqkt = nc.dram_tensor("bqkt", [128, 8], F32, kind="ExternalInput")
    bvc = nc.dram_tensor("bvc", [128, HG], F32, kind="ExternalInput")  # v bias * CSC
    cost = nc.dram_tensor("cost", [128, S], BF16, kind="ExternalInput")    # cos^T * QSC
    sinrt = nc.dram_tensor("sinrt", [128, S], BF16, kind="ExternalInput")  # sin^T * QSC, rot sign
    maskd = nc.dram_tensor("maskd", [128, 128], BF16, kind="ExternalInput")  # triu 0/1 keep-mask
    tonesd = nc.dram_tensor("tonesd", [128, 1], BF16, kind="ExternalInput")  # 1/CSC
    outt = nc.dram_tensor("outt", [16, 128, S], BF16, kind="ExternalOutput")
    lrt = nc.dram_tensor("lrt", [HG, 1, 16, 128], F32)  # recip bounce: [16,128] -> [1,2048]

    with tile.TileContext(nc) as tc, ExitStack() as top:
        g = top.enter_context(tc.tile_pool(name="glob", bufs=1))
        tcos = g.tile([128, S], BF16)
        tsin = g.tile([128, S], BF16)
        tmask = g.tile([128, 128], BF16)
        ident_f = g.tile([128, 128], F32)
        make_identity(nc, ident_f[:])
        tbqkt = g.tile([128, 8], F32)
        tbvc = g.tile([128, HG], F32)
        tones = g.tile([128, 1], BF16)
        tinv = g.tile([128, 1], F32)
        nc.vector.memset(tinv[:], INV_SC)

        # Whole-kernel residents
        res = top.enter_context(tc.tile_pool(name="res", bufs=1))
        vres = []  # 16 V k-block tiles [128(seq), GQ] bf16
        for t in range(NB):
            vres.append(res.tile([128, GQ], BF16, tag=f"v{t}", name=f"v{t}"))
        cth = {}
        for h in range(HG):
            for gq in range(4):
                cth[(h, gq)] = res.tile([128, 512], F8, tag=f"cth_{h}_{gq}",
                                        name=f"cth_{h}_{gq}")
        ctl = [res.tile([128, HG, 512], F8, tag=f"ctl{gq}", name=f"ctl{gq}")
               for gq in range(4)]

        for _rep in range(reps):
          phB = ExitStack()
          mmp = phB.enter_context(tc.tile_pool(name="mmp", bufs=kn["mmp"], space="PSUM"))
          sps = phB.enter_context(tc.tile_pool(name="sps", bufs=kn["sps"], space="PSUM"))
          smps = phB.enter_context(tc.tile_pool(name="smps", bufs=1, space="PSUM"))
          expp = phB.enter_context(tc.tile_pool(name="expp", bufs=kn["expb"]))
          lrp = phB.enter_context(tc.tile_pool(name="lrp", bufs=2))
          rbp = phB.enter_context(tc.tile_pool(name="rbp", bufs=2))
          ctsp = phB.enter_context(tc.tile_pool(name="ctsp", bufs=2))
          obp = phB.enter_context(tc.tile_pool(name="obp", bufs=kn["obp"]))
          smt = smps.tile([128, 132], F32, tag="sm", name="smt")

          phA = ExitStack()
          wqp = phA.enter_context(tc.tile_pool(name="wqp", bufs=kn["wqb"]))
          qkp = phA.enter_context(tc.tile_pool(name="qkp", bufs=kn["qkb"]))
          stg = phA.enter_context(tc.tile_pool(name="stg", bufs=kn["stg"]))
          xps = ExitStack()
          xp = xps.enter_context(tc.tile_pool(name="xp", bufs=1))
          sec0 = ExitStack()
          xsl = sec0.enter_context(tc.tile_pool(name="xsl", bufs=3))
          wvp = sec0.enter_context(tc.tile_pool(name="wvp", bufs=1))

          # ---- initial DMA order (startup-critical) ----
          wq_tiles = {}  # (h) -> (wqh, wql, wkh, wkl)

          def weights_dma(h):
              tl = []
              for mt, tag in ((h, "wqh"), (4 + h, "wkh")):
                  wt = wqp.tile([128, 8, 2, 128], F8, tag=tag, name=f"{tag}{h}")
                  nc.sync.dma_start(out=wt, in_=wqkh[mt])
                  tl.append(wt)
              wq_tiles[h] = tl

          # head-0 weights + first x slab first
          wt = wqp.tile([128, 8, 2, 128], F8, tag="wqh", name="wqh0")
          nc.sync.dma_start(out=wt, in_=wqkh[0])
          wt2 = wqp.tile([128, 8, 2, 128], F8, tag="wkh", name="wkh0")
          nc.sync.dma_start(out=wt2, in_=wqkh[4])
          wq_tiles[0] = [wt, wt2]
          xres = [xp.tile([128, 8, 2, SLAB], F8, tag=f"x{ns}", name=f"x{ns}")
                  for ns in range(4)]
          nc.sync.dma_start(out=xres[0], in_=xh[:, 0])
          # small consts needed by first psum copies / rope
          nc.sync.dma_start(out=tbqkt, in_=bqkt[:])
          nc.sync.dma_start(out=xres[1], in_=xh[:, 1])
          nc.sync.dma_start(out=tcos, in_=cost[:])
          nc.sync.dma_start(out=tsin, in_=sinrt[:])
          nc.sync.dma_start(out=xres[2], in_=xh[:, 2])
          nc.sync.dma_start(out=tones, in_=tonesd[:])
          nc.sync.dma_start(out=tmask, in_=maskd[:])
          nc.sync.dma_start(out=xres[3], in_=xh[:, 3])
          nc.sync.dma_start(out=tbvc, in_=bvc[:])
          twvh, twvl = [], []
          xlres = {}

          def wv_dmas():
              for kc2 in range(8):
                  wv_t = wvp.tile([128, 2, GQ], F8, tag=f"wvh{kc2}")
                  nc.sync.dma_start(out=wv_t, in_=wvh[kc2])
                  twvh.append(wv_t)
              for kc2 in range(8):
                  wv_t = wvp.tile([128, 2, GQ], F8, tag=f"wvl{kc2}")
                  nc.sync.dma_start(out=wv_t, in_=wvl[kc2])
                  twvl.append(wv_t)

          def xl_dma(ns):
              xt = xsl.tile([128, 8, 2, SLAB], F8, tag="xl", name=f"xl{ns}")
              nc.sync.dma_start(out=xt, in_=xl[:, ns])
              xlres[ns] = xt

          # ---- per-head state ----
          qf8 = {}
          kpair = {}
          expT = {}
          lrec_cur = {}
          recrow = {}

          def chain_qk(h, which, ns):
              wqh_, wkh_ = wq_tiles[h]
              whi = wqh_ if which == "q" else wkh_
              sl = slice(ns * SLAB, (ns + 1) * SLAB)
              ps = mmp.tile([128, SLAB], F32, tag="mm")
              for kc2 in range(8):
                  nc.tensor.matmul(ps[:], whi[:, kc2, :, :], xres[ns][:, kc2, :, :],
                                   start=(kc2 == 0), stop=(kc2 == 7), perf_mode=DR)
              st = stg.tile([128, SLAB], BF16, tag="st")
              mt = h if which == "q" else 4 + h
              nc.vector.tensor_scalar(out=st[:], in0=ps[:], scalar1=INV_SC,
                                      scalar2=tbqkt[:, mt:mt + 1],
                                      op0=MULT, op1=ADD)
              # RoPE: out = st*cos + swap(st)*sin_rot   (tables pre-scaled by QSC)
              sw = stg.tile([128, SLAB], BF16, tag="sw")
              nc.vector.tensor_copy(out=sw[0:64, :], in_=st[64:128, :])
              nc.vector.tensor_copy(out=sw[64:128, :], in_=st[0:64, :])
              m1 = stg.tile([128, SLAB], BF16, tag="m1")
              nc.vector.tensor_tensor(out=m1[:], in0=st[:], in1=tcos[:, sl], op=MULT)
              nc.vector.tensor_tensor(out=sw[:], in0=sw[:], in1=tsin[:, sl], op=MULT)
              if which == "q":
                  nc.vector.tensor_tensor(out=qf8[h][:, sl], in0=m1[:], in1=sw[:], op=ADD)
              else:
                  kb = stg.tile([128, SLAB], BF16, tag="kb")
                  nc.vector.tensor_tensor(out=kb[:], in0=m1[:], in1=sw[:], op=ADD)
                  nc.scalar.copy(out=kpair[h][:, 0, sl], in_=kb[:])
                  nc.gpsimd.tensor_tensor(out=kpair[h][:, 1, sl], in0=kb[:],
                                          in1=kpair[h][:, 0, sl], op=SUB)

          def v_tile(t):
              ns, sti = divmod(t, 4)
              s0 = sti * 128
              pv = mmp.tile([128, GQ], F32, tag="mm")
              passes = [(xres[ns], twvh), (xlres[ns], twvh), (xres[ns], twvl)]
              for pi, (xt_, wv_) in enumerate(passes):
                  for kc2 in range(8):
                      nc.tensor.matmul(pv[:], xt_[:, kc2, :, s0:s0 + 128],
                                       wv_[kc2][:], start=(pi == 0 and kc2 == 0),
                                       stop=(pi == 2 and kc2 == 7), perf_mode=DR)
              nc.scalar.activation(out=vres[t], in_=pv[:], func=IDENT,
                                   scale=INV_SC)

          def rec_group(h, gq):
              # ship recip(ell) for q-blocks 4g..4g+3 to DRAM and back as a row
              rt = smt[0:4, 0:128]
              nc.tensor.transpose(rt, lrec_cur[h][:, 4 * gq:4 * gq + 4], ident_f[:])
              rts = lrp.tile([4, 128], F32, tag="rts")
              nc.vector.tensor_copy(out=rts[:], in_=rt)
              nc.sync.dma_start(out=lrt[h, 0, 4 * gq:4 * gq + 4, :], in_=rts[:])
              nc.sync.dma_start(out=recrow[h][:, 4 * gq:4 * gq + 4, :],
                                in_=lrt[h, :, 4 * gq:4 * gq + 4, :])

          def denom(h, b):
              # ell[q] for q-block b: sum_k exp tiles via ap-1 matmuls, then recip
              lp = smt[:, 128 + (b % 4):129 + (b % 4)]
              for j in range(b + 1):
                  nc.tensor.matmul(lp, expT[h][j][:, (b - j) * 128:(b - j + 1) * 128],
                                   tones[:], start=(j == 0), stop=(j == b))
              nc.vector.reciprocal(out=lrec_cur[h][:, b:b + 1], in_=lp)

          def sweep2_group(h, gq, split_at=None, mid_fn=None):
              # ct = (sum_k V^T[k] expS^T[k]) * recip -> split into fp8 hi/lo
              rbs = rbp.tile([128, 512], F32, tag="rbs")
              nc.gpsimd.partition_broadcast(
                  rbs[:], recrow[h][:, 4 * gq:4 * gq + 4, :])
              ct = mmp.tile([128, 512], F32, tag="mm")
              last = 4 * gq + 3
              for j in range(last + 1):
                  if split_at is not None and j == split_at:
                      mid_fn()
                  if j <= 4 * gq:
                      nc.tensor.matmul(ct[:], vres[j][:, h * 128:(h + 1) * 128],
                                       expT[h][j][:, (4 * gq - j) * 128:(4 * gq - j) * 128 + 512],
                                       start=(j == 0), stop=(j == last))
                  else:
                      w = (4 * gq + 4 - j) * 128
                      nc.tensor.matmul(ct[:, 512 - w:512], vres[j][:, h * 128:(h + 1) * 128],
                                       expT[h][j][:, 0:w], start=False, stop=(j == last))
              ctb = ctsp.tile([128, 512], BF16, tag="ctb")
              nc.vector.tensor_tensor(out=ctb[:], in0=ct[:], in1=rbs[:], op=MULT)
              nc.gpsimd.tensor_scalar(out=cth[(h, gq)][:], in0=ctb[:],
                                      scalar1=tbvc[:, h:h + 1], scalar2=None,
                                      op0=ADD)
              nc.vector.scalar_tensor_tensor(
                  out=ctl[gq][:, h, :], in0=ctb[:], scalar=tbvc[:, h:h + 1],
                  in1=cth[(h, gq)][:], op0=ADD, op1=SUB)

          def scores_head(h, interleave):
              expT[h] = []
              lrec_cur[h] = lrp.tile([128, 16], F32, tag="lrec", name="lrec", bufs=1)
              recrow[h] = lrp.tile([1, 16, 128], F32, tag="recrow", name="recrow", bufs=1)
              kp = kpair[h]
              qf = qf8[h]
              for i in range(NB):
                  w = (NB - i) * 128
                  ex = expp.tile([128, w], BF16, tag=f"expT{i}", name=f"expT{i}")
                  expT[h].append(ex)
                  for c0 in range(0, w, 512):
                      cw = min(512, w - c0)
                      sp = sps.tile([128, 512], F32, tag="sp")
                      q0 = i * 128 + c0
                      nc.tensor.matmul(
                          sp[:, 0:cw], kp[:, :, i * 128:(i + 1) * 128],
                          qf[:, q0:q0 + cw].unsqueeze(1).broadcast_to((128, 2, cw)),
                          start=True, stop=True, perf_mode=DR)
                      nc.scalar.activation(out=ex[:, c0:c0 + cw], in_=sp[:, 0:cw],
                                           func=EXPF, scale=EXP_SCALE)
                      if c0 == 0:
                          nc.vector.tensor_tensor(out=ex[:, 0:128], in0=ex[:, 0:128],
                                                  in1=tmask[:], op=MULT)
                  if i >= 2:
                      denom(h, i - 2)
                      if i % 4 == 1 and i >= 5:
                          rec_group(h, (i - 5) // 4)
                  interleave(h, i)
              denom(h, NB - 2)
              denom(h, NB - 1)
              rec_group(h, 3)

          def alloc_qk(h):
              qf8[h] = qkp.tile([128, S], F8, tag="qf8", name=f"qf8_{h}")
              kpair[h] = qkp.tile([128, 2, S], F8, tag="kpair", name=f"kpair_{h}")

          wo1t = []
          wo2t = []
          tailp = ExitStack()

          def open_wop():
              xps.close()
              wop = tailp.enter_context(tc.tile_pool(name="wop", bufs=1))
              for kh in range(HG):
                  wt_ = wop.tile([128, 2, D], F8, tag=f"wo1_{kh}", name=f"wo1_{kh}")
                  nc.sync.dma_start(out=wt_, in_=wo1[kh])
                  wo1t.append(wt_)
              for gp in range(2):
                  wt_ = wop.tile([128, 2, D], F8, tag=f"wo2_{gp}", name=f"wo2_{gp}")
                  nc.sync.dma_start(out=wt_, in_=wo2[gp])
                  wo2t.append(wt_)

          def p3_mt(gq, mt):
              op = mmp.tile([128, 512], F32, tag="mm")
              for kh in range(HG):
                  nc.tensor.matmul(
                      op[:], wo1t[kh][:, :, mt * 128:(mt + 1) * 128],
                      cth[(kh, gq)][:].unsqueeze(1).broadcast_to((128, 2, 512)),
                      start=(kh == 0), stop=False, perf_mode=DR)
              for gp in range(2):
                  nc.tensor.matmul(
                      op[:], wo2t[gp][:, :, mt * 128:(mt + 1) * 128],
                      ctl[gq][:, 2 * gp:2 * gp + 2, :],
                      start=False, stop=(gp == 1), perf_mode=DR)
              ob = obp.tile([128, 512], BF16, tag="ob")
              if mt % 2 == 0:
                  nc.vector.tensor_copy(out=ob[:], in_=op[:])
              else:
                  nc.scalar.copy(out=ob[:], in_=op[:])
              nc.sync.dma_start(out=outt[mt, :, gq * 512:(gq + 1) * 512], in_=ob[:])

          def mk_sched(h):
              # schedule of extra PE work per score block of head h
              sched = {i: [] for i in range(NB)}
              if h == 0:
                  sched[2].append(lambda: alloc_qk(1))
                  for idx, (which, ns) in enumerate(
                          (w, n) for n in range(4) for w in ("q", "k")):
                      sched[2 + idx].append(
                          lambda w=which, n=ns: chain_qk(1, w, n))
                  sched[2].append(lambda: xl_dma(2))
                  sched[5].append(lambda: xl_dma(3))
                  for t in range(13):           # v tiles 0-12 at blocks 3-15
                      sched[3 + t].append(lambda t=t: v_tile(t))
                  sched[10].append(lambda: weights_dma(2))
              else:
                  if h == 1:
                      def _mid():
                          for t in (13, 14, 15):
                              v_tile(t)
                          sec0.close()
                      sched[0].append(
                          lambda: sweep2_group(0, 3, split_at=13, mid_fn=_mid))
                  else:
                      sched[0].append(lambda: sweep2_group(h - 1, 3))
                  if h < 3:
                      sched[2].append(lambda: alloc_qk(h + 1))
                      for idx, (which, ns) in enumerate(
                              (w, n) for n in range(4) for w in ("q", "k")):
                          sched[2 + idx].append(
                              lambda w=which, n=ns: chain_qk(h + 1, w, n))
                      if h + 2 < HG:
                          sched[10].append(lambda: weights_dma(h + 2))
                  if h == 2:
                      sched[6].append(open_wop)
                  if h == 3:
                      for j in range(16):       # p3 gq0 at blocks 8-14
                          sched[8 + min(j // 3, 6)].append(
                              lambda mt=j: p3_mt(0, mt))
                      for j in range(8):        # p3 gq1 first half, blocks 12-15
                          sched[12 + j // 2].append(lambda mt=j: p3_mt(1, mt))
              # own sweeps (recip rows for gq land 2 blocks after rec_group)
              sched[7].append(lambda: sweep2_group(h, 0))
              sched[11].append(lambda: sweep2_group(h, 1))
              sched[15].append(lambda: sweep2_group(h, 2))
              return sched

          def run_sched(sched, h, i):
              for fn in sched[i]:
                  fn()

          # ---- emit sections ----
          alloc_qk(0)
          for ns in range(4):
              chain_qk(0, "q", ns)
              chain_qk(0, "k", ns)
          weights_dma(1)
          wv_dmas()
          xl_dma(0)
          xl_dma(1)
          for h in range(HG):
              sched = mk_sched(h)
              scores_head(h, lambda hh, i, sched=sched: run_sched(sched, hh, i))

          # ---- tail: rest of the output projection ----
          for mt in range(8, 16):
              p3_mt(1, mt)
          sweep2_group(3, 3)
          for mt in range(16):
              p3_mt(2, mt)
          for mt in range(16):
              p3_mt(3, mt)
          tailp.close()
          phA.close()
          phB.close()
    nc.finalize()
    return nc


_NC_CACHE = {}


def _get_nc(reps=1):
    if reps not in _NC_CACHE:
        _NC_CACHE[reps] = build_nc(reps)
    return _NC_CACHE[reps]


def _rope_tables(position_ids_b):
    pos = position_ids_b.astype(np.float32)
    inv_freq = (1.0 / (ROPE_THETA ** (np.arange(0, DH, 2, dtype=np.float32) / np.float32(DH))))
    ang = pos[:, None] * inv_freq[None, :]          # [S, 64]
    emb = np.concatenate([ang, ang], axis=-1)       # [S, 128]
    cosT = np.ascontiguousarray(np.cos(emb).T) * np.float32(QSC)   # [128, S]
    sinT = np.sin(emb).T * np.float32(QSC)
    sin_rot = np.concatenate([-sinT[0:64], sinT[64:128]], axis=0)
    return cosT.astype(ml_dtypes.bfloat16), np.ascontiguousarray(sin_rot).astype(ml_dtypes.bfloat16)


def _make_in_maps(inputs):
    hidden_states = np.asarray(inputs["hidden_states"], dtype=np.float32)
    position_ids = np.asarray(inputs["position_ids"])
    Wqkv = np.asarray(inputs["Wqkv"], dtype=np.float32)
    bqkv = np.asarray(inputs["bqkv"], dtype=np.float32)
    Wo = np.asarray(inputs["Wo"], dtype=np.float32)

    mask = np.triu(np.ones((128, 128), dtype=np.float32)).astype(ml_dtypes.bfloat16)
    tones = np.full((128, 1), 1.0 / CSC, dtype=ml_dtypes.bfloat16)
    tabs = [_rope_tables(np.asarray(position_ids)[b]) for b in range(B)]

    def _hilo(M, sc):
        Ms = M * np.float32(sc)
        hi = Ms.astype(ml_dtypes.float8_e4m3)
        lo = (Ms - hi.astype(np.float32)).astype(ml_dtypes.float8_e4m3)
        return hi, lo

    def _pack_pairs(M):
        # [D, C] -> [8, 128, 2, C] with row r = kc2*256 + i*128 + p
        C = M.shape[1]
        return np.ascontiguousarray(M.reshape(8, 2, 128, C).transpose(0, 2, 1, 3))

    def _pack_x(M):
        # [D, S] -> [128, 4, 8, 2, SLAB] partition-major, slab-major free
        return np.ascontiguousarray(
            M.reshape(8, 2, 128, 4, SLAB).transpose(2, 3, 0, 1, 4))

    xts = []
    for b in range(B):
        hi, lo = _hilo(np.ascontiguousarray(hidden_states[b].T), XSC)
        xts.append((_pack_x(hi.astype(np.float32)).astype(ml_dtypes.float8_e4m3),
                    _pack_x(lo.astype(np.float32)).astype(ml_dtypes.float8_e4m3)))

    in_maps = []
    for c in range(NCORES):
        b, hg = divmod(c, HG)
        qcols = slice(hg * GQ, (hg + 1) * GQ)
        kcols = slice(D + hg * GQ, D + (hg + 1) * GQ)
        vcols = slice(2 * D + hg * GQ, 2 * D + (hg + 1) * GQ)
        wqk_c = np.ascontiguousarray(np.concatenate([Wqkv[:, qcols], Wqkv[:, kcols]], axis=1))
        qk_h, qk_l = _hilo(wqk_c, WSC)
        # per-mt packing: [8(mt), 128(p), 8(kc2), 2(i), 128(m)]
        def _pack_mt(M8):
            P = _pack_pairs(M8.astype(np.float32))          # [8, 128, 2, 1024]
            P = P.reshape(8, 128, 2, 8, 128)                 # [kc2, p, i, mt, m]
            return np.ascontiguousarray(P.transpose(3, 1, 0, 2, 4)).astype(ml_dtypes.float8_e4m3)
        wqkh_c = _pack_mt(qk_h)
        wqkl_c = _pack_mt(qk_l)
        wv_c = np.ascontiguousarray(Wqkv[:, vcols])
        v_h, v_l = _hilo(wv_c, WSC)
        wvh_c = _pack_pairs(v_h.astype(np.float32)).astype(ml_dtypes.float8_e4m3)
        wvl_c = _pack_pairs(v_l.astype(np.float32)).astype(ml_dtypes.float8_e4m3)
        # out-proj fp8 packs
        wo_c = np.ascontiguousarray(Wo[hg * GQ:(hg + 1) * GQ, :])   # [512, D]
        wo_h, wo_l = _hilo(wo_c, WSC)
        wo_h = wo_h.astype(np.float32)
        wo_l = wo_l.astype(np.float32)
        wo1_c = np.empty((HG, 128, 2, D), np.float32)
        for kh in range(HG):
            wo1_c[kh, :, 0, :] = wo_h[kh * 128:(kh + 1) * 128, :]
            wo1_c[kh, :, 1, :] = wo_l[kh * 128:(kh + 1) * 128, :]
        wo2_c = np.empty((2, 128, 2, D), np.float32)
        for gp in range(2):
            wo2_c[gp, :, 0, :] = wo_h[gp * 256:gp * 256 + 128, :]
            wo2_c[gp, :, 1, :] = wo_h[gp * 256 + 128:gp * 256 + 256, :]
        bqk_c = np.concatenate([bqkv[qcols], bqkv[kcols]]).reshape(8, 128).T
        bvc_c = np.ascontiguousarray(bqkv[vcols].reshape(HG, 128).T * np.float32(CSC))
        cosT, sin_rot = tabs[b]
        in_maps.append({
            "xh": xts[b][0], "xl": xts[b][1],
            "wqkh": wqkh_c, "wqkl": wqkl_c, "wvh": wvh_c, "wvl": wvl_c,
            "wo1": wo1_c.astype(ml_dtypes.float8_e4m3),
            "wo2": wo2_c.astype(ml_dtypes.float8_e4m3),
            "bqkt": np.ascontiguousarray(bqk_c),
            "bvc": bvc_c,
            "cost": cosT, "sinrt": sin_rot, "maskd": mask,
            "tonesd": tones,
        })
    return in_maps


def kernel(hidden_states, position_ids, Wqkv, bqkv, Wo, bo, _reps=1):
    bo = np.asarray(bo, dtype=np.float32)
    in_maps = _make_in_maps({
        "hidden_states": hidden_states, "position_ids": position_ids,
        "Wqkv": Wqkv, "bqkv": bqkv, "Wo": Wo, "bo": bo,
    })
    nc = _get_nc(_reps)
    res = run_bass_kernel_spmd(nc, in_maps, core_ids=list(range(NCORES)))

    out = np.empty((B, S, D), dtype=np.float32)
    for b in range(B):
        acc = res.results[b * HG]["outt"].reshape(D, S).astype(np.float32).copy()
        for hg in range(1, HG):
            acc += res.results[b * HG + hg]["outt"].reshape(D, S).astype(np.float32)
        out[b] = acc.T * np.float32(P3_SCALE) + bo[None, :]
    return out


# revision 11
# speedup vs baseline: 1.2624x; 1.0337x over previous
"""Trainium2 Bass kernel for CustomRoPEAttention (B=2, S=2048, H=16, Dh=128).

Sharding: 8 cores = 2 batches x 4 head-groups (4 heads/core).

Head-pipelined structure: per head h, QKV^T projection (fp8 hi/lo DoubleRow
matmuls) + RoPE, then transposed-layout causal attention for that head while
later heads' projections stream -- this overlaps the ACT-engine exp work with
PE-engine matmul work across the whole kernel instead of serializing phases.

fp8 DoubleRow "dup trick": scores use stationary (k_hi,k_lo) pairs against a
broadcast (step-0) fp8 q moving operand, and the output projection uses
(wo_hi,wo_lo) pairs against broadcast ct_hi plus a wo_hi x ct_lo correction --
half / 0.75x the bf16 PE time at first-order-exact precision.

Host sums the 4 partial (transposed) output projections per batch.

Self-contained: hardcodes shapes from the problem spec.
"""
import math
from contextlib import ExitStack

import numpy as np
import ml_dtypes

import concourse.mybir as mybir
import concourse.tile as tile
from concourse import bacc
from concourse.bass_utils import run_bass_kernel_spmd
from concourse.masks import make_identity

S = 2048            # sequence
D = 2048            # hidden
NH = 16             # total heads
DH = 128            # head dim
HG = 4              # heads per core
GQ = HG * DH        # 512: per-core q/k/v feature width
B = 2
NCORES = 8
ROPE_THETA = 10000.0
SCALE = 1.0 / math.sqrt(DH)
SLAB = 512          # qkv sequence slab width
XSC = 16.0          # fp8 pre-scale for x
WSC = 512.0         # fp8 pre-scale for Wqkv / Wo
QSC = 16.0          # fp8 pre-scale for roped q/k (folded into cos/sin tables)
CSC = 16.0          # fp8 pre-scale for attention-out ct (folded into tones)
INV_SC = 1.0 / (XSC * WSC)
EXP_SCALE = SCALE / (QSC * QSC)
P3_SCALE = 1.0 / (WSC * CSC)   # applied host-side
F32 = mybir.dt.float32
BF16 = mybir.dt.bfloat16
F8 = mybir.dt.float8e4
MULT = mybir.AluOpType.mult
ADD = mybir.AluOpType.add
SUB = mybir.AluOpType.subtract
DR = mybir.MatmulPerfMode.DoubleRow
NB = S // 128       # 16 k/q blocks
IDENT = mybir.ActivationFunctionType.Identity
EXPF = mybir.ActivationFunctionType.Exp


def build_nc(reps=1, knobs=None):
    kn = {"sps": 4, "mmp": 3, "expb": 1, "wqb": 2, "qkb": 2, "stg": 2, "obp": 6}
    if knobs:
        kn.update(knobs)
    nc = bacc.Bacc(None, target_bir_lowering=False)
    # x^T hi/lo, slab-major pack: [p, ns, kc2, i, s]
    xh = nc.dram_tensor("xh", [128, 4, 8, 2, SLAB], F8, kind="ExternalInput")
    xl = nc.dram_tensor("xl", [128, 4, 8, 2, SLAB], F8, kind="ExternalInput")
    # per-mt packed qk weights: [mt, p, kc2, i, m]
    wqkh = nc.dram_tensor("wqkh", [8, 128, 8, 2, 128], F8, kind="ExternalInput")
    wvh = nc.dram_tensor("wvh", [8, 128, 2, GQ], F8, kind="ExternalInput")
    wvl = nc.dram_tensor("wvl", [8, 128, 2, GQ], F8, kind="ExternalInput")
    # out-proj fp8 packs: wo1[kh] = (hi,lo) pairs; wo2[g] = hi head-pair packs
    wo1 = nc.dram_tensor("wo1", [HG, 128, 2, D], F8, kind="ExternalInput")
    wo2 = nc.dram_tensor("wo2", [2, 128, 2, D], F8, kind="ExternalInput")
    bqkt = nc.dram_tensor("bqkt", [128, 8], F32, kind="ExternalInput")
    bvc = nc.dram_tensor("bvc", [128, HG], F32, kind="ExternalInput")  # v bias * CSC
    cost = nc.dram_tensor("cost", [128, S], BF16, kind="ExternalInput")    # cos^T * QSC
    sinrt = nc.dram_tensor("sinrt", [128, S], BF16, kind="ExternalInput")  # sin^T * QSC, rot sign
    maskd = nc.dram_tensor("maskd", [128, 128], BF16, kind="ExternalInput")  # triu 0/1 keep-mask
    tonesd = nc.dram_tensor("tonesd", [128, 1], BF16, kind="ExternalInput")  # 1/CSC
    outt = nc.dram_tensor("outt", [16, 128, S], BF16, kind="ExternalOutput")
    lrt = nc.dram_tensor("lrt", [HG, 1, 16, 128], F32)  # recip bounce: [16,128] -> [1,2048]

    with tile.TileContext(nc) as tc, ExitStack() as top:
        g = top.enter_context(tc.tile_pool(name="glob", bufs=1))
        tcos = g.tile([128, S], BF16)
        tsin = g.tile([128, S], BF16)
        tmask = g.tile([128, 128], BF16)
        ident_f = g.tile([128, 128], F32)
        make_identity(nc, ident_f[:])
        tbqkt = g.tile([128, 8], F32)
        tbvc = g.tile([128, HG], F32)
        tones = g.tile([128, 1], BF16)
        tinv = g.tile([128, 1], F32)
        nc.vector.memset(tinv[:], INV_SC)

        # Whole-kernel residents
        res = top.enter_context(tc.tile_pool(name="res", bufs=1))
        vres = []  # 16 V k-block tiles [128(seq), GQ] bf16
        for t in range(NB):
            vres.append(res.tile([128, GQ], BF16, tag=f"v{t}", name=f"v{t}"))
        cth = {}
        for h in range(HG):
            for gq in range(4):
                cth[(h, gq)] = res.tile([128, 512], F8, tag=f"cth_{h}_{gq}",
                                        name=f"cth_{h}_{gq}")
        ctl = [res.tile([128, HG, 512], F8, tag=f"ctl{gq}", name=f"ctl{gq}")
               for gq in range(4)]

        for _rep in range(reps):
          phB = ExitStack()
          mmp = phB.enter_context(tc.tile_pool(name="mmp", bufs=kn["mmp"], space="PSUM"))
          sps = phB.enter_context(tc.tile_pool(name="sps", bufs=kn["sps"], space="PSUM"))
          smps = phB.enter_context(tc.tile_pool(name="smps", bufs=1, space="PSUM"))
          expp = phB.enter_context(tc.tile_pool(name="expp", bufs=kn["expb"]))
          lrp = phB.enter_context(tc.tile_pool(name="lrp", bufs=2))
          rbp = phB.enter_context(tc.tile_pool(name="rbp", bufs=2))
          ctsp = phB.enter_context(tc.tile_pool(name="ctsp", bufs=2))
          obp = phB.enter_context(tc.tile_pool(name="obp", bufs=kn["obp"]))
          smt = smps.tile([128, 132], F32, tag="sm", name="smt")

          phA = ExitStack()
          wqp = phA.enter_context(tc.tile_pool(name="wqp", bufs=kn["wqb"]))
          qkp = phA.enter_context(tc.tile_pool(name="qkp", bufs=kn["qkb"]))
          stg = phA.enter_context(tc.tile_pool(name="stg", bufs=kn["stg"]))
          xps = ExitStack()
          xp = xps.enter_context(tc.tile_pool(name="xp", bufs=1))
          sec0 = ExitStack()
          xsl = sec0.enter_context(tc.tile_pool(name="xsl", bufs=3))
          wvp = sec0.enter_context(tc.tile_pool(name="wvp", bufs=1))

          # ---- initial DMA order (startup-critical) ----
          wq_tiles = {}  # (h) -> (wqh, wql, wkh, wkl)

          def weights_dma(h):
              tl = []
              for mt, tag in ((h, "wqh"), (4 + h, "wkh")):
                  wt = wqp.tile([128, 8, 2, 128], F8, tag=tag, name=f"{tag}{h}")
                  nc.sync.dma_start(out=wt, in_=wqkh[mt])
                  tl.append(wt)
              wq_tiles[h] = tl

          # head-0 weights + first x slab first
          wt = wqp.tile([128, 8, 2, 128], F8, tag="wqh", name="wqh0")
          nc.sync.dma_start(out=wt[:, 0:2], in_=wqkh[0, :, 0:2])
          xres = [xp.tile([128, 8, 2, SLAB], F8, tag=f"x{ns}", name=f"x{ns}")
                  for ns in range(4)]
          nc.sync.dma_start(out=xres[0][:, 0:2], in_=xh[:, 0, 0:2])
          nc.sync.dma_start(out=wt[:, 2:8], in_=wqkh[0, :, 2:8])
          nc.sync.dma_start(out=xres[0][:, 2:5], in_=xh[:, 0, 2:5])
          wt2 = wqp.tile([128, 8, 2, 128], F8, tag="wkh", name="wkh0")
          nc.sync.dma_start(out=wt2, in_=wqkh[4])
          wq_tiles[0] = [wt, wt2]
          nc.sync.dma_start(out=xres[0][:, 5:8], in_=xh[:, 0, 5:8])
          # small consts needed by first psum copies / rope
          nc.sync.dma_start(out=tbqkt, in_=bqkt[:])
          nc.sync.dma_start(out=xres[1], in_=xh[:, 1])
          nc.sync.dma_start(out=tcos, in_=cost[:])
          nc.sync.dma_start(out=tsin, in_=sinrt[:])
          nc.sync.dma_start(out=xres[2], in_=xh[:, 2])
          nc.sync.dma_start(out=tones, in_=tonesd[:])
          nc.sync.dma_start(out=tmask, in_=maskd[:])
          nc.sync.dma_start(out=xres[3], in_=xh[:, 3])
          nc.sync.dma_start(out=tbvc, in_=bvc[:])
          twvh, twvl = [], []
          xlres = {}

          def wv_dmas(which):
              if which == "h":
                  wvht = wvp.tile([128, 8, 2, GQ], F8, tag="wvh", name="wvht")
                  nc.sync.dma_start(out=wvht, in_=wvh.rearrange("a p b c -> p a b c"))
                  twvh.append(wvht)
              else:
                  wvlt = wvp.tile([128, 8, 2, GQ], F8, tag="wvl", name="wvlt")
                  nc.sync.dma_start(out=wvlt, in_=wvl.rearrange("a p b c -> p a b c"))
                  twvl.append(wvlt)

          def xl_dma(ns):
              xt = xsl.tile([128, 8, 2, SLAB], F8, tag="xl", name=f"xl{ns}")
              nc.sync.dma_start(out=xt, in_=xl[:, ns])
              xlres[ns] = xt

          # ---- per-head state ----
          qf8 = {}
          kpair = {}
          expT = {}
          lrec_cur = {}
          recrow = {}

          def chain_qk(h, which, ns):
              wqh_, wkh_ = wq_tiles[h]
              whi = wqh_ if which == "q" else wkh_
              sl = slice(ns * SLAB, (ns + 1) * SLAB)
              ps = mmp.tile([128, SLAB], F32, tag="mm")
              for kc2 in range(8):
                  nc.tensor.matmul(ps[:], whi[:, kc2, :, :], xres[ns][:, kc2, :, :],
                                   start=(kc2 == 0), stop=(kc2 == 7), perf_mode=DR)
              st = stg.tile([128, SLAB], BF16, tag="st")
              mt = h if which == "q" else 4 + h
              nc.vector.tensor_scalar(out=st[:], in0=ps[:], scalar1=INV_SC,
                                      scalar2=tbqkt[:, mt:mt + 1],
                                      op0=MULT, op1=ADD)
              # RoPE: out = st*cos + swap(st)*sin_rot   (tables pre-scaled by QSC)
              sw = stg.tile([128, SLAB], BF16, tag="sw")
              nc.vector.tensor_copy(out=sw[0:64, :], in_=st[64:128, :])
              nc.vector.tensor_copy(out=sw[64:128, :], in_=st[0:64, :])
              m1 = stg.tile([128, SLAB], BF16, tag="m1")
              nc.vector.tensor_tensor(out=m1[:], in0=st[:], in1=tcos[:, sl], op=MULT)
              nc.vector.tensor_tensor(out=sw[:], in0=sw[:], in1=tsin[:, sl], op=MULT)
              if which == "q":
                  nc.vector.tensor_tensor(out=qf8[h][:, sl], in0=m1[:], in1=sw[:], op=ADD)
              else:
                  kb = stg.tile([128, SLAB], BF16, tag="kb")
                  nc.vector.tensor_tensor(out=kb[:], in0=m1[:], in1=sw[:], op=ADD)
                  nc.scalar.copy(out=kpair[h][:, 0, sl], in_=kb[:])
                  nc.gpsimd.tensor_tensor(out=kpair[h][:, 1, sl], in0=kb[:],
                                          in1=kpair[h][:, 0, sl], op=SUB)

          def v_tile(t):
              ns, sti = divmod(t, 4)
              s0 = sti * 128
              pv = mmp.tile([128, GQ], F32, tag="mm")
              passes = [(xres[ns], twvh[0]), (xlres[ns], twvh[0]), (xres[ns], twvl[0])]
              for pi, (xt_, wv_) in enumerate(passes):
                  for kc2 in range(8):
                      nc.tensor.matmul(pv[:], xt_[:, kc2, :, s0:s0 + 128],
                                       wv_[:, kc2, :, :], start=(pi == 0 and kc2 == 0),
                                       stop=(pi == 2 and kc2 == 7), perf_mode=DR)
              nc.scalar.activation(out=vres[t], in_=pv[:], func=IDENT,
                                   scale=INV_SC)

          def rec_group(h, gq):
              # ship recip(ell) for q-blocks 4g..4g+3 to DRAM and back as a row
              rt = smt[0:4, 0:128]
              nc.tensor.transpose(rt, lrec_cur[h][:, 4 * gq:4 * gq + 4], ident_f[:])
              rts = lrp.tile([4, 128], F32, tag="rts")
              nc.vector.tensor_copy(out=rts[:], in_=rt)
              nc.sync.dma_start(out=lrt[h, 0, 4 * gq:4 * gq + 4, :], in_=rts[:])
              nc.sync.dma_start(out=recrow[h][:, 4 * gq:4 * gq + 4, :],
                                in_=lrt[h, :, 4 * gq:4 * gq + 4, :])

          def denom(h, b):
              # ell[q] for q-block b: sum_k exp tiles via ap-1 matmuls, then recip
              lp = smt[:, 128 + (b % 4):129 + (b % 4)]
              for j in range(b + 1):
                  nc.tensor.matmul(lp, expT[h][j][:, (b - j) * 128:(b - j + 1) * 128],
                                   tones[:], start=(j == 0), stop=(j == b))
              nc.vector.reciprocal(out=lrec_cur[h][:, b:b + 1], in_=lp)

          def sweep2_group(h, gq, split_at=None, mid_fn=None):
              # ct = (sum_k V^T[k] expS^T[k]) * recip -> split into fp8 hi/lo
              rbs = rbp.tile([128, 512], F32, tag="rbs")
              nc.gpsimd.partition_broadcast(
                  rbs[:], recrow[h][:, 4 * gq:4 * gq + 4, :])
              ct = mmp.tile([128, 512], F32, tag="mm")
              last = 4 * gq + 3
              for j in range(last + 1):
                  if split_at is not None and j == split_at:
                      mid_fn()
                  if j <= 4 * gq:
                      nc.tensor.matmul(ct[:], vres[j][:, h * 128:(h + 1) * 128],
                                       expT[h][j][:, (4 * gq - j) * 128:(4 * gq - j) * 128 + 512],
                                       start=(j == 0), stop=(j == last))
                  else:
                      w = (4 * gq + 4 - j) * 128
                      nc.tensor.matmul(ct[:, 512 - w:512], vres[j][:, h * 128:(h + 1) * 128],
                                       expT[h][j][:, 0:w], start=False, stop=(j == last))
              ctb = ctsp.tile([128, 512], BF16, tag="ctb")
              nc.vector.tensor_tensor(out=ctb[:], in0=ct[:], in1=rbs[:], op=MULT)
              nc.gpsimd.tensor_scalar(out=cth[(h, gq)][:], in0=ctb[:],
                                      scalar1=tbvc[:, h:h + 1], scalar2=None,
                                      op0=ADD)
              nc.vector.scalar_tensor_tensor(
                  out=ctl[gq][:, h, :], in0=ctb[:], scalar=tbvc[:, h:h + 1],
                  in1=cth[(h, gq)][:], op0=ADD, op1=SUB)

          def scores_head(h, interleave):
              expT[h] = []
              lrec_cur[h] = lrp.tile([128, 16], F32, tag="lrec", name="lrec", bufs=1)
              recrow[h] = lrp.tile([1, 16, 128], F32, tag="recrow", name="recrow", bufs=1)
              kp = kpair[h]
              qf = qf8[h]
              for i in range(NB):
                  w = (NB - i) * 128
                  ex = expp.tile([128, w], BF16, tag=f"expT{i}", name=f"expT{i}")
                  expT[h].append(ex)
                  for c0 in range(0, w, 512):
                      cw = min(512, w - c0)
                      sp = sps.tile([128, 512], F32, tag="sp")
                      q0 = i * 128 + c0
                      nc.tensor.matmul(
                          sp[:, 0:cw], kp[:, :, i * 128:(i + 1) * 128],
                          qf[:, q0:q0 + cw].unsqueeze(1).broadcast_to((128, 2, cw)),
                          start=True, stop=True, perf_mode=DR)
                      nc.scalar.activation(out=ex[:, c0:c0 + cw], in_=sp[:, 0:cw],
                                           func=EXPF, scale=EXP_SCALE)
                      if c0 == 0:
                          nc.gpsimd.tensor_tensor(out=ex[:, 0:128], in0=ex[:, 0:128],
                                                  in1=tmask[:], op=MULT)
                  if i >= 2:
                      denom(h, i - 2)
                      if i % 4 == 1 and i >= 5:
                          rec_group(h, (i - 5) // 4)
                  interleave(h, i)
              denom(h, NB - 2)
              denom(h, NB - 1)
              rec_group(h, 3)

          def alloc_qk(h):
              qf8[h] = qkp.tile([128, S], F8, tag="qf8", name=f"qf8_{h}")
              kpair[h] = qkp.tile([128, 2, S], F8, tag="kpair", name=f"kpair_{h}")

          wo1t = []
          wo2t = []
          tailp = ExitStack()

          def open_wop():
              xps.close()
              wop = tailp.enter_context(tc.tile_pool(name="wop", bufs=1))
              for kh in range(HG):
                  wt_ = wop.tile([128, 2, D], F8, tag=f"wo1_{kh}", name=f"wo1_{kh}")
                  nc.sync.dma_start(out=wt_, in_=wo1[kh])
                  wo1t.append(wt_)
              for gp in range(2):
                  wt_ = wop.tile([128, 2, D], F8, tag=f"wo2_{gp}", name=f"wo2_{gp}")
                  nc.sync.dma_start(out=wt_, in_=wo2[gp])
                  wo2t.append(wt_)

          def p3_mt(gq, mt, tail=False):
              if tail and mt % 2 == 0:
                  op = sps.tile([128, 512], F32, tag="sp")
              else:
                  op = mmp.tile([128, 512], F32, tag="mm")
              for kh in range(HG):
                  nc.tensor.matmul(
                      op[:], wo1t[kh][:, :, mt * 128:(mt + 1) * 128],
                      cth[(kh, gq)][:].unsqueeze(1).broadcast_to((128, 2, 512)),
                      start=(kh == 0), stop=False, perf_mode=DR)
              for gp in range(2):
                  nc.tensor.matmul(
                      op[:], wo2t[gp][:, :, mt * 128:(mt + 1) * 128],
                      ctl[gq][:, 2 * gp:2 * gp + 2, :],
                      start=False, stop=(gp == 1), perf_mode=DR)
              ob = obp.tile([128, 512], BF16, tag="ob")
              if mt % 3 == 0:
                  nc.vector.tensor_copy(out=ob[:], in_=op[:])
              elif mt % 3 == 1:
                  nc.scalar.copy(out=ob[:], in_=op[:])
              else:
                  nc.gpsimd.tensor_copy(out=ob[:], in_=op[:])
              nc.sync.dma_start(out=outt[mt, :, gq * 512:(gq + 1) * 512], in_=ob[:])

          def mk_sched(h):
              # schedule of extra PE work per score block of head h
              sched = {i: [] for i in range(NB)}
              if h == 0:
                  sched[2].append(lambda: alloc_qk(1))
                  for idx, (which, ns) in enumerate(
                          (w, n) for n in range(4) for w in ("q", "k")):
                      sched[2 + idx].append(
                          lambda w=which, n=ns: chain_qk(1, w, n))
                  sched[2].append(lambda: xl_dma(2))
                  sched[5].append(lambda: xl_dma(3))
                  for t in range(13):           # v tiles 0-12 at blocks 3-15
                      sched[3 + t].append(lambda t=t: v_tile(t))
                  sched[10].append(lambda: weights_dma(2))
              else:
                  if h == 1:
                      def _mid():
                          for t in (13, 14, 15):
                              v_tile(t)
                          sec0.close()
                      sched[0].append(
                          lambda: sweep2_group(0, 3, split_at=13, mid_fn=_mid))
                  else:
                      sched[0].append(lambda: sweep2_group(h - 1, 3))
                  if h < 3:
                      sched[2].append(lambda: alloc_qk(h + 1))
                      for idx, (which, ns) in enumerate(
                              (w, n) for n in range(4) for w in ("q", "k")):
                          sched[2 + idx].append(
                              lambda w=which, n=ns: chain_qk(h + 1, w, n))
                      if h + 2 < HG:
                          sched[10].append(lambda: weights_dma(h + 2))
                  if h == 2:
                      sched[6].append(open_wop)
                  if h == 3:
                      for j in range(16):       # p3 gq0 at blocks 8-15
                          sched[8 + j // 2].append(
                              lambda mt=j: p3_mt(0, mt))
              # own sweeps (recip rows for gq land 2 blocks after rec_group)
              sched[7].append(lambda: sweep2_group(h, 0))
              sched[11].append(lambda: sweep2_group(h, 1))
              sched[15].append(lambda: sweep2_group(h, 2))
              return sched

          def run_sched(sched, h, i):
              for fn in sched[i]:
                  fn()

          # ---- emit sections ----
          alloc_qk(0)
          for ns in range(4):
              chain_qk(0, "q", ns)
              chain_qk(0, "k", ns)
          weights_dma(1)
          wv_dmas("h")
          xl_dma(0)
          wv_dmas("l")
          xl_dma(1)
          for h in range(HG):
              sched = mk_sched(h)
              scores_head(h, lambda hh, i, sched=sched: run_sched(sched, hh, i))

          # ---- tail: rest of the output projection ----
          for mt in range(16):
              p3_mt(1, mt, tail=True)
          sweep2_group(3, 3)
          for mt in range(16):
              p3_mt(2, mt, tail=True)
          for mt in range(16):
              p3_mt(3, mt, tail=True)
          tailp.close()
          phA.close()
          phB.close()
    nc.finalize()
    return nc


_NC_CACHE = {}


def _get_nc(reps=1):
    if reps not in _NC_CACHE:
        _NC_CACHE[reps] = build_nc(reps)
    return _NC_CACHE[reps]


def _rope_tables(position_ids_b):
    pos = position_ids_b.astype(np.float32)
    inv_freq = (1.0 / (ROPE_THETA ** (np.arange(0, DH, 2, dtype=np.float32) / np.float32(DH))))
    ang = pos[:, None] * inv_freq[None, :]          # [S, 64]
    emb = np.concatenate([ang, ang], axis=-1)       # [S, 128]
    cosT = np.ascontiguousarray(np.cos(emb).T) * np.float32(QSC)   # [128, S]
    sinT = np.sin(emb).T * np.float32(QSC)
    sin_rot = np.concatenate([-sinT[0:64], sinT[64:128]], axis=0)
    return cosT.astype(ml_dtypes.bfloat16), np.ascontiguousarray(sin_rot).astype(ml_dtypes.bfloat16)


def _make_in_maps(inputs):
    hidden_states = np.asarray(inputs["hidden_states"], dtype=np.float32)
    position_ids = np.asarray(inputs["position_ids"])
    Wqkv = np.asarray(inputs["Wqkv"], dtype=np.float32)
    bqkv = np.asarray(inputs["bqkv"], dtype=np.float32)
    Wo = np.asarray(inputs["Wo"], dtype=np.float32)

    mask = np.triu(np.ones((128, 128), dtype=np.float32)).astype(ml_dtypes.bfloat16)
    tones = np.full((128, 1), 1.0 / CSC, dtype=ml_dtypes.bfloat16)
    tabs = [_rope_tables(np.asarray(position_ids)[b]) for b in range(B)]

    def _hilo(M, sc):
        Ms = M * np.float32(sc)
        hi = Ms.astype(ml_dtypes.float8_e4m3)
        lo = (Ms - hi.astype(np.float32)).astype(ml_dtypes.float8_e4m3)
        return hi, lo

    def _pack_pairs(M):
        # [D, C] -> [8, 128, 2, C] with row r = kc2*256 + i*128 + p
        C = M.shape[1]
        return np.ascontiguousarray(M.reshape(8, 2, 128, C).transpose(0, 2, 1, 3))

    def _pack_x(M):
        # [D, S] -> [128, 4, 8, 2, SLAB] partition-major, slab-major free
        return np.ascontiguousarray(
            M.reshape(8, 2, 128, 4, SLAB).transpose(2, 3, 0, 1, 4))

    xts = []
    for b in range(B):
        hi, lo = _hilo(np.ascontiguousarray(hidden_states[b].T), XSC)
        xts.append((_pack_x(hi.astype(np.float32)).astype(ml_dtypes.float8_e4m3),
                    _pack_x(lo.astype(np.float32)).astype(ml_dtypes.float8_e4m3)))

    in_maps = []
    for c in range(NCORES):
        b, hg = divmod(c, HG)
        qcols = slice(hg * GQ, (hg + 1) * GQ)
        kcols = slice(D + hg * GQ, D + (hg + 1) * GQ)
        vcols = slice(2 * D + hg * GQ, 2 * D + (hg + 1) * GQ)
        wqk_c = np.ascontiguousarray(np.concatenate([Wqkv[:, qcols], Wqkv[:, kcols]], axis=1))
        qk_h, qk_l = _hilo(wqk_c, WSC)
        # per-mt packing: [8(mt), 128(p), 8(kc2), 2(i), 128(m)]
        def _pack_mt(M8):
            P = _pack_pairs(M8.astype(np.float32))          # [8, 128, 2, 1024]
            P = P.reshape(8, 128, 2, 8, 128)                 # [kc2, p, i, mt, m]
            return np.ascontiguousarray(P.transpose(3, 1, 0, 2, 4)).astype(ml_dtypes.float8_e4m3)
        wqkh_c = _pack_mt(qk_h)
        wqkl_c = _pack_mt(qk_l)
        wv_c = np.ascontiguousarray(Wqkv[:, vcols])
        v_h, v_l = _hilo(wv_c, WSC)
        wvh_c = _pack_pairs(v_h.astype(np.float32)).astype(ml_dtypes.float8_e4m3)
        wvl_c = _pack_pairs(v_l.astype(np.float32)).astype(ml_dtypes.float8_e4m3)
        # out-proj fp8 packs
        wo_c = np.ascontiguousarray(Wo[hg * GQ:(hg + 1) * GQ, :])   # [512, D]
        wo_h, wo_l = _hilo(wo_c, WSC)
        wo_h = wo_h.astype(np.float32)
        wo_l = wo_l.astype(np.float32)
        wo1_c = np.empty((HG, 128, 2, D), np.float32)
        for kh in range(HG):
            wo1_c[kh, :, 0, :] = wo_h[kh * 128:(kh + 1) * 128, :]
            wo1_c[kh, :, 1, :] = wo_l[kh * 128:(kh + 1) * 128, :]
        wo2_c = np.empty((2, 128, 2, D), np.float32)
        for gp in range(2):
            wo2_c[gp, :, 0, :] = wo_h[gp * 256:gp * 256 + 128, :]
            wo2_c[gp, :, 1, :] = wo_h[gp * 256 + 128:gp * 256 + 256, :]
        bqk_c = np.concatenate([bqkv[qcols], bqkv[kcols]]).reshape(8, 128).T
        bvc_c = np.ascontiguousarray(bqkv[vcols].reshape(HG, 128).T * np.float32(CSC))
        cosT, sin_rot = tabs[b]
        in_maps.append({
            "xh": xts[b][0], "xl": xts[b][1],
            "wqkh": wqkh_c, "wqkl": wqkl_c, "wvh": wvh_c, "wvl": wvl_c,
            "wo1": wo1_c.astype(ml_dtypes.float8_e4m3),
            "wo2": wo2_c.astype(ml_dtypes.float8_e4m3),
            "bqkt": np.ascontiguousarray(bqk_c),
            "bvc": bvc_c,
            "cost": cosT, "sinrt": sin_rot, "maskd": mask,
            "tonesd": tones,
        })
    return in_maps


def kernel(hidden_states, position_ids, Wqkv, bqkv, Wo, bo, _reps=1):
    bo = np.asarray(bo, dtype=np.float32)
    in_maps = _make_in_maps({
        "hidden_states": hidden_states, "position_ids": position_ids,
        "Wqkv": Wqkv, "bqkv": bqkv, "Wo": Wo, "bo": bo,
    })
    nc = _get_nc(_reps)
    res = run_bass_kernel_spmd(nc, in_maps, core_ids=list(range(NCORES)))

    out = np.empty((B, S, D), dtype=np.float32)
    for b in range(B):
        acc = res.results[b * HG]["outt"].reshape(D, S).astype(np.float32).copy()
        for hg in range(1, HG):
            acc += res.results[b * HG + hg]["outt"].reshape(D, S).astype(np.float32)
        out[b] = acc.T * np.float32(P3_SCALE) + bo[None, :]
    return out


# revision 12
# speedup vs baseline: 1.2632x; 1.0006x over previous
"""Trainium2 Bass kernel for CustomRoPEAttention (B=2, S=2048, H=16, Dh=128).

Sharding: 8 cores = 2 batches x 4 head-groups (4 heads/core).

Head-pipelined structure: per head h, QKV^T projection (fp8 hi/lo DoubleRow
matmuls) + RoPE, then transposed-layout causal attention for that head while
later heads' projections stream -- this overlaps the ACT-engine exp work with
PE-engine matmul work across the whole kernel instead of serializing phases.

fp8 DoubleRow "dup trick": scores use stationary (k_hi,k_lo) pairs against a
broadcast (step-0) fp8 q moving operand, and the output projection uses
(wo_hi,wo_lo) pairs against broadcast ct_hi plus a wo_hi x ct_lo correction --
half / 0.75x the bf16 PE time at first-order-exact precision.

Host sums the 4 partial (transposed) output projections per batch.

Self-contained: hardcodes shapes from the problem spec.
"""
import math
from contextlib import ExitStack

import numpy as np
import ml_dtypes

import concourse.mybir as mybir
import concourse.tile as tile
from concourse import bacc
from concourse.bass_utils import run_bass_kernel_spmd
from concourse.masks import make_identity

S = 2048            # sequence
D = 2048            # hidden
NH = 16             # total heads
DH = 128            # head dim
HG = 4              # heads per core
GQ = HG * DH        # 512: per-core q/k/v feature width
B = 2
NCORES = 8
ROPE_THETA = 10000.0
SCALE = 1.0 / math.sqrt(DH)
SLAB = 512          # qkv sequence slab width
XSC = 16.0          # fp8 pre-scale for x
WSC = 512.0         # fp8 pre-scale for Wqkv / Wo
QSC = 16.0          # fp8 pre-scale for roped q/k (folded into cos/sin tables)
CSC = 16.0          # fp8 pre-scale for attention-out ct (folded into tones)
INV_SC = 1.0 / (XSC * WSC)
EXP_SCALE = SCALE / (QSC * QSC)
P3_SCALE = 1.0 / (WSC * CSC)   # applied host-side
F32 = mybir.dt.float32
BF16 = mybir.dt.bfloat16
F8 = mybir.dt.float8e4
MULT = mybir.AluOpType.mult
ADD = mybir.AluOpType.add
SUB = mybir.AluOpType.subtract
DR = mybir.MatmulPerfMode.DoubleRow
NB = S // 128       # 16 k/q blocks
IDENT = mybir.ActivationFunctionType.Identity
EXPF = mybir.ActivationFunctionType.Exp


def build_nc(reps=1, knobs=None):
    kn = {"sps": 4, "mmp": 3, "expb": 1, "wqb": 2, "qkb": 2, "stg": 2, "obp": 6}
    if knobs:
        kn.update(knobs)
    nc = bacc.Bacc(None, target_bir_lowering=False)
    # x^T hi/lo, slab-major pack: [p, ns, kc2, i, s]
    xh = nc.dram_tensor("xh", [128, 4, 8, 2, SLAB], F8, kind="ExternalInput")
    xl = nc.dram_tensor("xl", [128, 4, 8, 2, SLAB], F8, kind="ExternalInput")
    # per-mt packed qk weights: [mt, p, kc2, i, m]
    wqkh = nc.dram_tensor("wqkh", [8, 128, 8, 2, 128], F8, kind="ExternalInput")
    wvh = nc.dram_tensor("wvh", [8, 128, 2, GQ], F8, kind="ExternalInput")
    wvl = nc.dram_tensor("wvl", [8, 128, 2, GQ], F8, kind="ExternalInput")
    # out-proj fp8 packs: wo1[kh] = (hi,lo) pairs; wo2[g] = hi head-pair packs
    wo1 = nc.dram_tensor("wo1", [HG, 128, 2, D], F8, kind="ExternalInput")
    wo2 = nc.dram_tensor("wo2", [2, 128, 2, D], F8, kind="ExternalInput")
    bqkt = nc.dram_tensor("bqkt", [128, 8], F32, kind="ExternalInput")
    bvc = nc.dram_tensor("bvc", [128, HG], F32, kind="ExternalInput")  # v bias * CSC
    cost = nc.dram_tensor("cost", [128, S], BF16, kind="ExternalInput")    # cos^T * QSC
    sinrt = nc.dram_tensor("sinrt", [128, S], BF16, kind="ExternalInput")  # sin^T * QSC, rot sign
    maskd = nc.dram_tensor("maskd", [128, 128], BF16, kind="ExternalInput")  # triu 0/1 keep-mask
    tonesd = nc.dram_tensor("tonesd", [128, 1], BF16, kind="ExternalInput")  # 1/CSC
    outt = nc.dram_tensor("outt", [16, 128, S], BF16, kind="ExternalOutput")
    lrt = nc.dram_tensor("lrt", [HG, 1, 16, 128], F32)  # recip bounce: [16,128] -> [1,2048]

    with tile.TileContext(nc) as tc, ExitStack() as top:
        g = top.enter_context(tc.tile_pool(name="glob", bufs=1))
        tcos = g.tile([128, S], BF16)
        tsin = g.tile([128, S], BF16)
        tmask = g.tile([128, 128], BF16)
        ident_f = g.tile([128, 128], F32)
        make_identity(nc, ident_f[:])
        tbqkt = g.tile([128, 8], F32)
        tbvc = g.tile([128, HG], F32)
        tones = g.tile([128, 1], BF16)
        tinv = g.tile([128, 1], F32)
        nc.vector.memset(tinv[:], INV_SC)

        # Whole-kernel residents
        res = top.enter_context(tc.tile_pool(name="res", bufs=1))
        vres = []  # 16 V k-block tiles [128(seq), GQ] bf16
        for t in range(NB):
            vres.append(res.tile([128, GQ], BF16, tag=f"v{t}", name=f"v{t}"))
        cth = {}
        for h in range(HG):
            for gq in range(4):
                cth[(h, gq)] = res.tile([128, 512], F8, tag=f"cth_{h}_{gq}",
                                        name=f"cth_{h}_{gq}")
        ctl = [res.tile([128, HG, 512], F8, tag=f"ctl{gq}", name=f"ctl{gq}")
               for gq in range(4)]

        for _rep in range(reps):
          phB = ExitStack()
          mmp = phB.enter_context(tc.tile_pool(name="mmp", bufs=kn["mmp"], space="PSUM"))
          sps = phB.enter_context(tc.tile_pool(name="sps", bufs=kn["sps"], space="PSUM"))
          smps = phB.enter_context(tc.tile_pool(name="smps", bufs=1, space="PSUM"))
          expp = phB.enter_context(tc.tile_pool(name="expp", bufs=kn["expb"]))
          lrp = phB.enter_context(tc.tile_pool(name="lrp", bufs=2))
          rbp = phB.enter_context(tc.tile_pool(name="rbp", bufs=2))
          ctsp = phB.enter_context(tc.tile_pool(name="ctsp", bufs=2))
          obp = phB.enter_context(tc.tile_pool(name="obp", bufs=kn["obp"]))
          smt = smps.tile([128, 132], F32, tag="sm", name="smt")

          phA = ExitStack()
          wqp = phA.enter_context(tc.tile_pool(name="wqp", bufs=kn["wqb"]))
          qkp = phA.enter_context(tc.tile_pool(name="qkp", bufs=kn["qkb"]))
          stg = phA.enter_context(tc.tile_pool(name="stg", bufs=kn["stg"]))
          xps = ExitStack()
          xp = xps.enter_context(tc.tile_pool(name="xp", bufs=1))
          sec0 = ExitStack()
          xsl = sec0.enter_context(tc.tile_pool(name="xsl", bufs=3))
          wvp = sec0.enter_context(tc.tile_pool(name="wvp", bufs=1))

          # ---- initial DMA order (startup-critical) ----
          wq_tiles = {}  # (h) -> (wqh, wql, wkh, wkl)

          def weights_dma(h):
              tl = []
              for mt, tag in ((h, "wqh"), (4 + h, "wkh")):
                  wt = wqp.tile([128, 8, 2, 128], F8, tag=tag, name=f"{tag}{h}")
                  nc.sync.dma_start(out=wt, in_=wqkh[mt])
                  tl.append(wt)
              wq_tiles[h] = tl

          # head-0 weights + first x slab first
          wt = wqp.tile([128, 8, 2, 128], F8, tag="wqh", name="wqh0")
          nc.sync.dma_start(out=wt[:, 0:2], in_=wqkh[0, :, 0:2])
          xres = [xp.tile([128, 8, 2, SLAB], F8, tag=f"x{ns}", name=f"x{ns}")
                  for ns in range(4)]
          nc.sync.dma_start(out=xres[0][:, 0:2], in_=xh[:, 0, 0:2])
          nc.sync.dma_start(out=wt[:, 2:8], in_=wqkh[0, :, 2:8])
          nc.sync.dma_start(out=xres[0][:, 2:5], in_=xh[:, 0, 2:5])
          wt2 = wqp.tile([128, 8, 2, 128], F8, tag="wkh", name="wkh0")
          nc.sync.dma_start(out=wt2, in_=wqkh[4])
          wq_tiles[0] = [wt, wt2]
          nc.sync.dma_start(out=xres[0][:, 5:8], in_=xh[:, 0, 5:8])
          # small consts needed by first psum copies / rope
          nc.sync.dma_start(out=tbqkt, in_=bqkt[:])
          nc.sync.dma_start(out=xres[1], in_=xh[:, 1])
          nc.sync.dma_start(out=tcos, in_=cost[:])
          nc.sync.dma_start(out=tsin, in_=sinrt[:])
          nc.sync.dma_start(out=xres[2], in_=xh[:, 2])
          nc.sync.dma_start(out=tones, in_=tonesd[:])
          nc.sync.dma_start(out=tmask, in_=maskd[:])
          nc.sync.dma_start(out=xres[3], in_=xh[:, 3])
          nc.sync.dma_start(out=tbvc, in_=bvc[:])
          twvh, twvl = [], []
          xlres = {}

          def wv_dmas(which):
              if which == "h":
                  wvht = wvp.tile([128, 8, 2, GQ], F8, tag="wvh", name="wvht")
                  nc.sync.dma_start(out=wvht, in_=wvh.rearrange("a p b c -> p a b c"))
                  twvh.append(wvht)
              else:
                  wvlt = wvp.tile([128, 8, 2, GQ], F8, tag="wvl", name="wvlt")
                  nc.sync.dma_start(out=wvlt, in_=wvl.rearrange("a p b c -> p a b c"))
                  twvl.append(wvlt)

          def xl_dma(ns):
              xt = xsl.tile([128, 8, 2, SLAB], F8, tag="xl", name=f"xl{ns}")
              nc.sync.dma_start(out=xt, in_=xl[:, ns])
              xlres[ns] = xt

          # ---- per-head state ----
          qf8 = {}
          kpair = {}
          expT = {}
          lrec_cur = {}
          recrow = {}

          def chain_qk(h, which, ns):
              wqh_, wkh_ = wq_tiles[h]
              whi = wqh_ if which == "q" else wkh_
              sl = slice(ns * SLAB, (ns + 1) * SLAB)
              ps = mmp.tile([128, SLAB], F32, tag="mm")
              for kc2 in range(8):
                  nc.tensor.matmul(ps[:], whi[:, kc2, :, :], xres[ns][:, kc2, :, :],
                                   start=(kc2 == 0), stop=(kc2 == 7), perf_mode=DR)
              st = stg.tile([128, SLAB], BF16, tag="st")
              mt = h if which == "q" else 4 + h
              nc.vector.tensor_scalar(out=st[:], in0=ps[:], scalar1=INV_SC,
                                      scalar2=tbqkt[:, mt:mt + 1],
                                      op0=MULT, op1=ADD)
              # RoPE: out = st*cos + swap(st)*sin_rot   (tables pre-scaled by QSC)
              sw = stg.tile([128, SLAB], BF16, tag="sw")
              nc.vector.tensor_copy(out=sw[0:64, :], in_=st[64:128, :])
              nc.vector.tensor_copy(out=sw[64:128, :], in_=st[0:64, :])
              m1 = stg.tile([128, SLAB], BF16, tag="m1")
              nc.vector.tensor_tensor(out=m1[:], in0=st[:], in1=tcos[:, sl], op=MULT)
              nc.vector.tensor_tensor(out=sw[:], in0=sw[:], in1=tsin[:, sl], op=MULT)
              if which == "q":
                  nc.vector.tensor_tensor(out=qf8[h][:, sl], in0=m1[:], in1=sw[:], op=ADD)
              else:
                  kb = stg.tile([128, SLAB], BF16, tag="kb")
                  nc.vector.tensor_tensor(out=kb[:], in0=m1[:], in1=sw[:], op=ADD)
                  nc.scalar.copy(out=kpair[h][:, 0, sl], in_=kb[:])
                  nc.gpsimd.tensor_tensor(out=kpair[h][:, 1, sl], in0=kb[:],
                                          in1=kpair[h][:, 0, sl], op=SUB)

          def v_tile(t):
              ns, sti = divmod(t, 4)
              s0 = sti * 128
              pv = mmp.tile([128, GQ], F32, tag="mm")
              passes = [(xres[ns], twvh[0]), (xlres[ns], twvh[0]), (xres[ns], twvl[0])]
              for pi, (xt_, wv_) in enumerate(passes):
                  for kc2 in range(8):
                      nc.tensor.matmul(pv[:], xt_[:, kc2, :, s0:s0 + 128],
                                       wv_[:, kc2, :, :], start=(pi == 0 and kc2 == 0),
                                       stop=(pi == 2 and kc2 == 7), perf_mode=DR)
              nc.scalar.activation(out=vres[t], in_=pv[:], func=IDENT,
                                   scale=INV_SC)

          def rec_group(h, gq):
              # ship recip(ell) for q-blocks 4g..4g+3 to DRAM and back as a row
              rt = smt[0:4, 0:128]
              nc.tensor.transpose(rt, lrec_cur[h][:, 4 * gq:4 * gq + 4], ident_f[:])
              rts = lrp.tile([4, 128], F32, tag="rts")
              nc.vector.tensor_copy(out=rts[:], in_=rt)
              nc.sync.dma_start(out=lrt[h, 0, 4 * gq:4 * gq + 4, :], in_=rts[:])
              nc.sync.dma_start(out=recrow[h][:, 4 * gq:4 * gq + 4, :],
                                in_=lrt[h, :, 4 * gq:4 * gq + 4, :])

          def denom(h, b):
              # ell[q] for q-block b: sum_k exp tiles via ap-1 matmuls, then recip
              lp = smt[:, 128 + (b % 4):129 + (b % 4)]
              for j in range(b + 1):
                  nc.tensor.matmul(lp, expT[h][j][:, (b - j) * 128:(b - j + 1) * 128],
                                   tones[:], start=(j == 0), stop=(j == b))
              nc.vector.reciprocal(out=lrec_cur[h][:, b:b + 1], in_=lp)

          def sweep2_group(h, gq, split_at=None, mid_fn=None):
              # ct = (sum_k V^T[k] expS^T[k]) * recip -> split into fp8 hi/lo
              rbs = rbp.tile([128, 512], F32, tag="rbs")
              nc.gpsimd.partition_broadcast(
                  rbs[:], recrow[h][:, 4 * gq:4 * gq + 4, :])
              ct = mmp.tile([128, 512], F32, tag="mm")
              last = 4 * gq + 3
              for j in range(last + 1):
                  if split_at is not None and j == split_at:
                      mid_fn()
                  if j <= 4 * gq:
                      nc.tensor.matmul(ct[:], vres[j][:, h * 128:(h + 1) * 128],
                                       expT[h][j][:, (4 * gq - j) * 128:(4 * gq - j) * 128 + 512],
                                       start=(j == 0), stop=(j == last))
                  else:
                      w = (4 * gq + 4 - j) * 128
                      nc.tensor.matmul(ct[:, 512 - w:512], vres[j][:, h * 128:(h + 1) * 128],
                                       expT[h][j][:, 0:w], start=False, stop=(j == last))
              ctb = ctsp.tile([128, 512], BF16, tag="ctb")
              nc.vector.tensor_tensor(out=ctb[:], in0=ct[:], in1=rbs[:], op=MULT)
              nc.gpsimd.tensor_scalar(out=cth[(h, gq)][:], in0=ctb[:],
                                      scalar1=tbvc[:, h:h + 1], scalar2=None,
                                      op0=ADD)
              nc.vector.scalar_tensor_tensor(
                  out=ctl[gq][:, h, :], in0=ctb[:], scalar=tbvc[:, h:h + 1],
                  in1=cth[(h, gq)][:], op0=ADD, op1=SUB)

          def scores_head(h, interleave):
              expT[h] = []
              lrec_cur[h] = lrp.tile([128, 16], F32, tag="lrec", name="lrec", bufs=1)
              recrow[h] = lrp.tile([1, 16, 128], F32, tag="recrow", name="recrow", bufs=1)
              kp = kpair[h]
              qf = qf8[h]
              for i in range(NB):
                  w = (NB - i) * 128
                  ex = expp.tile([128, w], BF16, tag=f"expT{i}", name=f"expT{i}")
                  expT[h].append(ex)
                  for c0 in range(0, w, 512):
                      cw = min(512, w - c0)
                      sp = sps.tile([128, 512], F32, tag="sp")
                      q0 = i * 128 + c0
                      nc.tensor.matmul(
                          sp[:, 0:cw], kp[:, :, i * 128:(i + 1) * 128],
                          qf[:, q0:q0 + cw].unsqueeze(1).broadcast_to((128, 2, cw)),
                          start=True, stop=True, perf_mode=DR)
                      nc.scalar.activation(out=ex[:, c0:c0 + cw], in_=sp[:, 0:cw],
                                           func=EXPF, scale=EXP_SCALE)
                      if c0 == 0:
                          nc.gpsimd.tensor_tensor(out=ex[:, 0:128], in0=ex[:, 0:128],
                                                  in1=tmask[:], op=MULT)
                  if i >= 2:
                      denom(h, i - 2)
                      if i % 4 == 1 and i >= 5:
                          rec_group(h, (i - 5) // 4)
                  interleave(h, i)
              denom(h, NB - 2)
              denom(h, NB - 1)
              rec_group(h, 3)

          def alloc_qk(h):
              qf8[h] = qkp.tile([128, S], F8, tag="qf8", name=f"qf8_{h}")
              kpair[h] = qkp.tile([128, 2, S], F8, tag="kpair", name=f"kpair_{h}")

          wo1t = []
          wo2t = []
          tailp = ExitStack()

          def open_wop():
              xps.close()
              wop = tailp.enter_context(tc.tile_pool(name="wop", bufs=1))
              for kh in range(HG):
                  wt_ = wop.tile([128, 2, D], F8, tag=f"wo1_{kh}", name=f"wo1_{kh}")
                  nc.sync.dma_start(out=wt_, in_=wo1[kh])
                  wo1t.append(wt_)
              for gp in range(2):
                  wt_ = wop.tile([128, 2, D], F8, tag=f"wo2_{gp}", name=f"wo2_{gp}")
                  nc.sync.dma_start(out=wt_, in_=wo2[gp])
                  wo2t.append(wt_)

          def p3_mt(gq, mt, tail=False):
              if tail and mt % 2 == 0:
                  op = sps.tile([128, 512], F32, tag="sp")
              else:
                  op = mmp.tile([128, 512], F32, tag="mm")
              for kh in range(HG):
                  nc.tensor.matmul(
                      op[:], wo1t[kh][:, :, mt * 128:(mt + 1) * 128],
                      cth[(kh, gq)][:].unsqueeze(1).broadcast_to((128, 2, 512)),
                      start=(kh == 0), stop=False, perf_mode=DR)
              for gp in range(2):
                  nc.tensor.matmul(
                      op[:], wo2t[gp][:, :, mt * 128:(mt + 1) * 128],
                      ctl[gq][:, 2 * gp:2 * gp + 2, :],
                      start=False, stop=(gp == 1), perf_mode=DR)
              ob = obp.tile([128, 512], BF16, tag="ob")
              if mt % 2 == 0:
                  nc.vector.tensor_copy(out=ob[:], in_=op[:])
              else:
                  nc.scalar.copy(out=ob[:], in_=op[:])
              nc.sync.dma_start(out=outt[mt, :, gq * 512:(gq + 1) * 512], in_=ob[:])

          def mk_sched(h):
              # schedule of extra PE work per score block of head h
              sched = {i: [] for i in range(NB)}
              if h == 0:
                  sched[2].append(lambda: alloc_qk(1))
                  for idx, (which, ns) in enumerate(
                          (w, n) for n in range(4) for w in ("q", "k")):
                      sched[2 + idx].append(
                          lambda w=which, n=ns: chain_qk(1, w, n))
                  sched[2].append(lambda: xl_dma(2))
                  sched[5].append(lambda: xl_dma(3))
                  for t in range(13):           # v tiles 0-12 at blocks 3-15
                      sched[3 + t].append(lambda t=t: v_tile(t))
                  sched[10].append(lambda: weights_dma(2))
              else:
                  if h == 1:
                      def _mid():
                          for t in (13, 14, 15):
                              v_tile(t)
                          sec0.close()
                      sched[0].append(
                          lambda: sweep2_group(0, 3, split_at=13, mid_fn=_mid))
                  else:
                      sched[0].append(lambda: sweep2_group(h - 1, 3))
                  if h < 3:
                      sched[2].append(lambda: alloc_qk(h + 1))
                      for idx, (which, ns) in enumerate(
                              (w, n) for n in range(4) for w in ("q", "k")):
                          sched[2 + idx].append(
                              lambda w=which, n=ns: chain_qk(h + 1, w, n))
                      if h + 2 < HG:
                          sched[10].append(lambda: weights_dma(h + 2))
                  if h == 2:
                      sched[6].append(open_wop)
                  if h == 3:
                      for j in range(16):       # p3 gq0 at blocks 8-15
                          sched[8 + j // 2].append(
                              lambda mt=j: p3_mt(0, mt))
              # own sweeps (recip rows for gq land 2 blocks after rec_group)
              sched[7].append(lambda: sweep2_group(h, 0))
              sched[11].append(lambda: sweep2_group(h, 1))
              sched[15].append(lambda: sweep2_group(h, 2))
              return sched

          def run_sched(sched, h, i):
              for fn in sched[i]:
                  fn()

          # ---- emit sections ----
          alloc_qk(0)
          for ns in range(4):
              chain_qk(0, "q", ns)
              chain_qk(0, "k", ns)
          weights_dma(1)
          wv_dmas("h")
          xl_dma(0)
          wv_dmas("l")
          xl_dma(1)
          for h in range(HG):
              sched = mk_sched(h)
              scores_head(h, lambda hh, i, sched=sched: run_sched(sched, hh, i))

          # ---- tail: rest of the output projection ----
          for mt in range(16):
              p3_mt(1, mt, tail=True)
          sweep2_group(3, 3)
          for mt in range(16):
              p3_mt(2, mt, tail=True)
          for mt in range(16):
              p3_mt(3, mt, tail=True)
          tailp.close()
          phA.close()
          phB.close()
    nc.finalize()
    return nc


_NC_CACHE = {}


def _get_nc(reps=1):
    if reps not in _NC_CACHE:
        _NC_CACHE[reps] = build_nc(reps)
    return _NC_CACHE[reps]


def _rope_tables(position_ids_b):
    pos = position_ids_b.astype(np.float32)
    inv_freq = (1.0 / (ROPE_THETA ** (np.arange(0, DH, 2, dtype=np.float32) / np.float32(DH))))
    ang = pos[:, None] * inv_freq[None, :]          # [S, 64]
    emb = np.concatenate([ang, ang], axis=-1)       # [S, 128]
    cosT = np.ascontiguousarray(np.cos(emb).T) * np.float32(QSC)   # [128, S]
    sinT = np.sin(emb).T * np.float32(QSC)
    sin_rot = np.concatenate([-sinT[0:64], sinT[64:128]], axis=0)
    return cosT.astype(ml_dtypes.bfloat16), np.ascontiguousarray(sin_rot).astype(ml_dtypes.bfloat16)


def _make_in_maps(inputs):
    hidden_states = np.asarray(inputs["hidden_states"], dtype=np.float32)
    position_ids = np.asarray(inputs["position_ids"])
    Wqkv = np.asarray(inputs["Wqkv"], dtype=np.float32)
    bqkv = np.asarray(inputs["bqkv"], dtype=np.float32)
    Wo = np.asarray(inputs["Wo"], dtype=np.float32)

    mask = np.triu(np.ones((128, 128), dtype=np.float32)).astype(ml_dtypes.bfloat16)
    tones = np.full((128, 1), 1.0 / CSC, dtype=ml_dtypes.bfloat16)
    tabs = [_rope_tables(np.asarray(position_ids)[b]) for b in range(B)]

    def _hilo(M, sc):
        Ms = M * np.float32(sc)
        hi = Ms.astype(ml_dtypes.float8_e4m3)
        lo = (Ms - hi.astype(np.float32)).astype(ml_dtypes.float8_e4m3)
        return hi, lo

    def _pack_pairs(M):
        # [D, C] -> [8, 128, 2, C] with row r = kc2*256 + i*128 + p
        C = M.shape[1]
        return np.ascontiguousarray(M.reshape(8, 2, 128, C).transpose(0, 2, 1, 3))

    def _pack_x(M):
        # [D, S] -> [128, 4, 8, 2, SLAB] partition-major, slab-major free
        return np.ascontiguousarray(
            M.reshape(8, 2, 128, 4, SLAB).transpose(2, 3, 0, 1, 4))

    xts = []
    for b in range(B):
        hi, lo = _hilo(np.ascontiguousarray(hidden_states[b].T), XSC)
        xts.append((_pack_x(hi.astype(np.float32)).astype(ml_dtypes.float8_e4m3),
                    _pack_x(lo.astype(np.float32)).astype(ml_dtypes.float8_e4m3)))

    in_maps = []
    for c in range(NCORES):
        b, hg = divmod(c, HG)
        qcols = slice(hg * GQ, (hg + 1) * GQ)
        kcols = slice(D + hg * GQ, D + (hg + 1) * GQ)
        vcols = slice(2 * D + hg * GQ, 2 * D + (hg + 1) * GQ)
        wqk_c = np.ascontiguousarray(np.concatenate([Wqkv[:, qcols], Wqkv[:, kcols]], axis=1))
        qk_h, qk_l = _hilo(wqk_c, WSC)
        # per-mt packing: [8(mt), 128(p), 8(kc2), 2(i), 128(m)]
        def _pack_mt(M8):
            P = _pack_pairs(M8.astype(np.float32))          # [8, 128, 2, 1024]
            P = P.reshape(8, 128, 2, 8, 128)                 # [kc2, p, i, mt, m]
            return np.ascontiguousarray(P.transpose(3, 1, 0, 2, 4)).astype(ml_dtypes.float8_e4m3)
        wqkh_c = _pack_mt(qk_h)
        wqkl_c = _pack_mt(qk_l)
        wv_c = np.ascontiguousarray(Wqkv[:, vcols])
        v_h, v_l = _hilo(wv_c, WSC)
        wvh_c = _pack_pairs(v_h.astype(np.float32)).astype(ml_dtypes.float8_e4m3)
        wvl_c = _pack_pairs(v_l.astype(np.float32)).astype(ml_dtypes.float8_e4m3)
        # out-proj fp8 packs
        wo_c = np.ascontiguousarray(Wo[hg * GQ:(hg + 1) * GQ, :])   # [512, D]
        wo_h, wo_l = _hilo(wo_c, WSC)
        wo_h = wo_h.astype(np.float32)
        wo_l = wo_l.astype(np.float32)
        wo1_c = np.empty((HG, 128, 2, D), np.float32)
        for kh in range(HG):
            wo1_c[kh, :, 0, :] = wo_h[kh * 128:(kh + 1) * 128, :]
            wo1_c[kh, :, 1, :] = wo_l[kh * 128:(kh + 1) * 128, :]
        wo2_c = np.empty((2, 128, 2, D), np.float32)
        for gp in range(2):
            wo2_c[gp, :, 0, :] = wo_h[gp * 256:gp * 256 + 128, :]
            wo2_c[gp, :, 1, :] = wo_h[gp * 256 + 128:gp * 256 + 256, :]
        bqk_c = np.concatenate([bqkv[qcols], bqkv[kcols]]).reshape(8, 128).T
        bvc_c = np.ascontiguousarray(bqkv[vcols].reshape(HG, 128).T * np.float32(CSC))
        cosT, sin_rot = tabs[b]
        in_maps.append({
            "xh": xts[b][0], "xl": xts[b][1],
            "wqkh": wqkh_c, "wqkl": wqkl_c, "wvh": wvh_c, "wvl": wvl_c,
            "wo1": wo1_c.astype(ml_dtypes.float8_e4m3),
            "wo2": wo2_c.astype(ml_dtypes.float8_e4m3),
            "bqkt": np.ascontiguousarray(bqk_c),
            "bvc": bvc_c,
            "cost": cosT, "sinrt": sin_rot, "maskd": mask,
            "tonesd": tones,
        })
    return in_maps


def kernel(hidden_states, position_ids, Wqkv, bqkv, Wo, bo, _reps=1):
    bo = np.asarray(bo, dtype=np.float32)
    in_maps = _make_in_maps({
        "hidden_states": hidden_states, "position_ids": position_ids,
        "Wqkv": Wqkv, "bqkv": bqkv, "Wo": Wo, "bo": bo,
    })
    nc = _get_nc(_reps)
    res = run_bass_kernel_spmd(nc, in_maps, core_ids=list(range(NCORES)))

    out = np.empty((B, S, D), dtype=np.float32)
    for b in range(B):
        acc = res.results[b * HG]["outt"].reshape(D, S).astype(np.float32).copy()
        for hg in range(1, HG):
            acc += res.results[b * HG + hg]["outt"].reshape(D, S).astype(np.float32)
        out[b] = acc.T * np.float32(P3_SCALE) + bo[None, :]
    return out


# revision 17
# speedup vs baseline: 1.2887x; 1.0202x over previous
"""Trainium2 Bass kernel for CustomRoPEAttention (B=2, S=2048, H=16, Dh=128).

Sharding: 8 cores = 2 batches x 4 head-groups (4 heads/core).

Head-pipelined structure: per head h, QKV^T projection (fp8 hi/lo DoubleRow
matmuls) + RoPE, then transposed-layout causal attention for that head while
later heads' projections stream -- this overlaps the ACT-engine exp work with
PE-engine matmul work across the whole kernel instead of serializing phases.

fp8 DoubleRow "dup trick": scores use stationary (k_hi,k_lo) pairs against a
broadcast (step-0) fp8 q moving operand, and the output projection uses
(wo_hi,wo_lo) pairs against broadcast ct_hi plus a wo_hi x ct_lo correction --
half / 0.75x the bf16 PE time at first-order-exact precision.

Host sums the 4 partial (transposed) output projections per batch.

Self-contained: hardcodes shapes from the problem spec.
"""
import math
from contextlib import ExitStack

import numpy as np
import ml_dtypes

import concourse.mybir as mybir
import concourse.tile as tile
from concourse import bacc
from concourse.bass_utils import run_bass_kernel_spmd
from concourse.masks import make_identity

S = 2048            # sequence
D = 2048            # hidden
NH = 16             # total heads
DH = 128            # head dim
HG = 4              # heads per core
GQ = HG * DH        # 512: per-core q/k/v feature width
B = 2
NCORES = 8
ROPE_THETA = 10000.0
SCALE = 1.0 / math.sqrt(DH)
SLAB = 512          # qkv sequence slab width
XSC = 16.0          # fp8 pre-scale for x
WSC = 512.0         # fp8 pre-scale for Wqkv / Wo
QSC = 16.0          # fp8 pre-scale for roped q/k (folded into cos/sin tables)
CSC = 16.0          # fp8 pre-scale for attention-out ct (folded into tones)
INV_SC = 1.0 / (XSC * WSC)
EXP_SCALE = SCALE / (QSC * QSC)
P3_SCALE = 1.0 / (WSC * CSC)   # applied host-side
F32 = mybir.dt.float32
BF16 = mybir.dt.bfloat16
F8 = mybir.dt.float8e4
MULT = mybir.AluOpType.mult
ADD = mybir.AluOpType.add
SUB = mybir.AluOpType.subtract
DR = mybir.MatmulPerfMode.DoubleRow
NB = S // 128       # 16 k/q blocks
IDENT = mybir.ActivationFunctionType.Identity
EXPF = mybir.ActivationFunctionType.Exp


def build_nc(reps=1, knobs=None):
    kn = {"sps": 4, "mmp": 3, "expb": 1, "wqb": 2, "qkb": 2, "stg": 2, "obp": 6}
    if knobs:
        kn.update(knobs)
    nc = bacc.Bacc(None, target_bir_lowering=False)
    # x^T hi/lo, slab-major pack: [p, ns, kc2, i, s]
    xh = nc.dram_tensor("xh", [128, 4, 8, 2, SLAB], F8, kind="ExternalInput")
    xl = nc.dram_tensor("xl", [128, 4, 8, 2, SLAB], F8, kind="ExternalInput")
    # per-mt packed qk weights: [mt, p, kc2, i, m]
    wqkh = nc.dram_tensor("wqkh", [8, 128, 8, 2, 128], F8, kind="ExternalInput")
    wvh = nc.dram_tensor("wvh", [8, 128, 2, GQ], F8, kind="ExternalInput")
    wvl = nc.dram_tensor("wvl", [8, 128, 2, GQ], F8, kind="ExternalInput")
    # out-proj fp8 packs: wo1[kh] = (hi,lo) pairs; wo2[g] = hi head-pair packs
    wo1 = nc.dram_tensor("wo1", [HG, 128, 2, D], F8, kind="ExternalInput")
    wo2 = nc.dram_tensor("wo2", [2, 128, 2, D], F8, kind="ExternalInput")
    bqkt = nc.dram_tensor("bqkt", [128, 8], F32, kind="ExternalInput")
    bvc = nc.dram_tensor("bvc", [128, HG], F32, kind="ExternalInput")  # v bias * CSC
    cost = nc.dram_tensor("cost", [128, S], BF16, kind="ExternalInput")    # cos^T * QSC
    sinrt = nc.dram_tensor("sinrt", [128, S], BF16, kind="ExternalInput")  # sin^T * QSC, rot sign
    maskd = nc.dram_tensor("maskd", [128, 128], BF16, kind="ExternalInput")  # triu 0/1 keep-mask
    tonesd = nc.dram_tensor("tonesd", [128, 1], BF16, kind="ExternalInput")  # 1/CSC
    outt = nc.dram_tensor("outt", [16, 128, S], BF16, kind="ExternalOutput")
    lrt = nc.dram_tensor("lrt", [HG, 1, 16, 128], F32)  # recip bounce: [16,128] -> [1,2048]

    with tile.TileContext(nc) as tc, ExitStack() as top:
        g = top.enter_context(tc.tile_pool(name="glob", bufs=1))
        tcos = g.tile([128, S], BF16)
        tsin = g.tile([128, S], BF16)
        tmask = g.tile([128, 128], BF16)
        ident_f = g.tile([128, 128], F32)
        make_identity(nc, ident_f[:])
        tbqkt = g.tile([128, 8], F32)
        tbvc = g.tile([128, HG], F32)
        tones = g.tile([128, 1], BF16)
        tinv = g.tile([128, 1], F32)
        nc.vector.memset(tinv[:], INV_SC)

        # Whole-kernel residents
        res = top.enter_context(tc.tile_pool(name="res", bufs=1))
        vres = []  # 16 V k-block tiles [128(seq), GQ] bf16
        for t in range(NB):
            vres.append(res.tile([128, GQ], BF16, tag=f"v{t}", name=f"v{t}"))
        cth = {}
        for h in range(HG):
            for gq in range(4):
                cth[(h, gq)] = res.tile([128, 512], F8, tag=f"cth_{h}_{gq}",
                                        name=f"cth_{h}_{gq}")
        ctl = [res.tile([128, HG, 512], F8, tag=f"ctl{gq}", name=f"ctl{gq}")
               for gq in range(4)]

        for _rep in range(reps):
          phB = ExitStack()
          mmp = phB.enter_context(tc.tile_pool(name="mmp", bufs=kn["mmp"], space="PSUM"))
          sps = phB.enter_context(tc.tile_pool(name="sps", bufs=kn["sps"], space="PSUM"))
          smps = phB.enter_context(tc.tile_pool(name="smps", bufs=1, space="PSUM"))
          expp = phB.enter_context(tc.tile_pool(name="expp", bufs=kn["expb"]))
          lrp = phB.enter_context(tc.tile_pool(name="lrp", bufs=2))
          rbp = phB.enter_context(tc.tile_pool(name="rbp", bufs=2))
          ctsp = phB.enter_context(tc.tile_pool(name="ctsp", bufs=2))
          obp = phB.enter_context(tc.tile_pool(name="obp", bufs=kn["obp"]))
          smt = smps.tile([128, 132], F32, tag="sm", name="smt")

          phA = ExitStack()
          wqp = phA.enter_context(tc.tile_pool(name="wqp", bufs=kn["wqb"]))
          qkp = phA.enter_context(tc.tile_pool(name="qkp", bufs=kn["qkb"]))
          stg = phA.enter_context(tc.tile_pool(name="stg", bufs=kn["stg"]))
          xps = ExitStack()
          xp = xps.enter_context(tc.tile_pool(name="xp", bufs=1))
          sec0 = ExitStack()
          xsl = sec0.enter_context(tc.tile_pool(name="xsl", bufs=3))
          wvp = sec0.enter_context(tc.tile_pool(name="wvp", bufs=1))

          # ---- initial DMA order (startup-critical) ----
          wq_tiles = {}  # (h) -> (wqh, wql, wkh, wkl)

          def weights_dma(h):
              tl = []
              for mt, tag in ((h, "wqh"), (4 + h, "wkh")):
                  wt = wqp.tile([128, 8, 2, 128], F8, tag=tag, name=f"{tag}{h}")
                  nc.sync.dma_start(out=wt, in_=wqkh[mt])
                  tl.append(wt)
              wq_tiles[h] = tl

          # head-0 weights + first x slab first
          wt = wqp.tile([128, 8, 2, 128], F8, tag="wqh", name="wqh0")
          nc.sync.dma_start(out=wt[:, 0:2], in_=wqkh[0, :, 0:2])
          xres = [xp.tile([128, 8, 2, SLAB], F8, tag=f"x{ns}", name=f"x{ns}")
                  for ns in range(4)]
          nc.sync.dma_start(out=xres[0][:, 0:2], in_=xh[:, 0, 0:2])
          nc.sync.dma_start(out=wt[:, 2:8], in_=wqkh[0, :, 2:8])
          nc.sync.dma_start(out=xres[0][:, 2:5], in_=xh[:, 0, 2:5])
          wt2 = wqp.tile([128, 8, 2, 128], F8, tag="wkh", name="wkh0")
          nc.sync.dma_start(out=wt2, in_=wqkh[4])
          wq_tiles[0] = [wt, wt2]
          nc.sync.dma_start(out=xres[0][:, 5:8], in_=xh[:, 0, 5:8])
          # small consts needed by first psum copies / rope
          nc.sync.dma_start(out=tbqkt, in_=bqkt[:])
          nc.sync.dma_start(out=xres[1], in_=xh[:, 1])
          nc.sync.dma_start(out=tcos, in_=cost[:])
          nc.sync.dma_start(out=tsin, in_=sinrt[:])
          nc.sync.dma_start(out=xres[2], in_=xh[:, 2])
          nc.sync.dma_start(out=tones, in_=tonesd[:])
          nc.sync.dma_start(out=tmask, in_=maskd[:])
          nc.sync.dma_start(out=xres[3], in_=xh[:, 3])
          nc.sync.dma_start(out=tbvc, in_=bvc[:])
          twvh, twvl = [], []
          xlres = {}

          def wv_dmas(which):
              if which == "h":
                  wvht = wvp.tile([128, 8, 2, GQ], F8, tag="wvh", name="wvht")
                  nc.sync.dma_start(out=wvht, in_=wvh.rearrange("a p b c -> p a b c"))
                  twvh.append(wvht)
              else:
                  wvlt = wvp.tile([128, 8, 2, GQ], F8, tag="wvl", name="wvlt")
                  nc.sync.dma_start(out=wvlt, in_=wvl.rearrange("a p b c -> p a b c"))
                  twvl.append(wvlt)

          def xl_dma(ns):
              xt = xsl.tile([128, 8, 2, SLAB], F8, tag="xl", name=f"xl{ns}")
              nc.sync.dma_start(out=xt, in_=xl[:, ns])
              xlres[ns] = xt

          # ---- per-head state ----
          qf8 = {}
          kpair = {}
          expT = {}
          lrec_cur = {}
          recrow = {}

          def chain_qk(h, which, ns):
              wqh_, wkh_ = wq_tiles[h]
              whi = wqh_ if which == "q" else wkh_
              sl = slice(ns * SLAB, (ns + 1) * SLAB)
              ps = mmp.tile([128, SLAB], F32, tag="mm")
              for kc2 in range(8):
                  nc.tensor.matmul(ps[:], whi[:, kc2, :, :], xres[ns][:, kc2, :, :],
                                   start=(kc2 == 0), stop=(kc2 == 7), perf_mode=DR)
              st = stg.tile([128, SLAB], BF16, tag="st")
              mt = h if which == "q" else 4 + h
              nc.vector.tensor_scalar(out=st[:], in0=ps[:], scalar1=INV_SC,
                                      scalar2=tbqkt[:, mt:mt + 1],
                                      op0=MULT, op1=ADD)
              # RoPE: out = st*cos + swap(st)*sin_rot   (tables pre-scaled by QSC)
              sw = stg.tile([128, SLAB], BF16, tag="sw")
              nc.vector.tensor_copy(out=sw[0:64, :], in_=st[64:128, :])
              nc.vector.tensor_copy(out=sw[64:128, :], in_=st[0:64, :])
              m1 = stg.tile([128, SLAB], BF16, tag="m1")
              nc.vector.tensor_tensor(out=m1[:], in0=st[:], in1=tcos[:, sl], op=MULT)
              nc.vector.tensor_tensor(out=sw[:], in0=sw[:], in1=tsin[:, sl], op=MULT)
              if which == "q":
                  nc.vector.tensor_tensor(out=qf8[h][:, sl], in0=m1[:], in1=sw[:], op=ADD)
              else:
                  kb = stg.tile([128, SLAB], BF16, tag="kb")
                  nc.vector.tensor_tensor(out=kb[:], in0=m1[:], in1=sw[:], op=ADD)
                  if h == 0:
                      nc.scalar.copy(out=kpair[h][:, 0, sl], in_=kb[:])
                      nc.vector.tensor_tensor(out=kpair[h][:, 1, sl], in0=kb[:],
                                              in1=kpair[h][:, 0, sl], op=SUB)
                  else:
                      nc.gpsimd.tensor_scalar(out=kpair[h][:, 0, sl], in0=kb[:],
                                              scalar1=0.0, scalar2=None, op0=ADD)
                      nc.gpsimd.tensor_tensor(out=kpair[h][:, 1, sl], in0=kb[:],
                                              in1=kpair[h][:, 0, sl], op=SUB)

          def v_tile(t):
              ns, sti = divmod(t, 4)
              s0 = sti * 128
              pv = mmp.tile([128, GQ], F32, tag="mm")
              passes = [(xres[ns], twvh[0]), (xlres[ns], twvh[0]), (xres[ns], twvl[0])]
              for pi, (xt_, wv_) in enumerate(passes):
                  for kc2 in range(8):
                      nc.tensor.matmul(pv[:], xt_[:, kc2, :, s0:s0 + 128],
                                       wv_[:, kc2, :, :], start=(pi == 0 and kc2 == 0),
                                       stop=(pi == 2 and kc2 == 7), perf_mode=DR)
              nc.scalar.activation(out=vres[t], in_=pv[:], func=IDENT,
                                   scale=INV_SC)

          def rec_group(h, gq):
              # ship recip(ell) for q-blocks 4g..4g+3 to DRAM and back as a row
              rt = smt[0:4, 0:128]
              nc.tensor.transpose(rt, lrec_cur[h][:, 4 * gq:4 * gq + 4], ident_f[:])
              rts = lrp.tile([4, 128], F32, tag="rts")
              nc.vector.tensor_copy(out=rts[:], in_=rt)
              nc.sync.dma_start(out=lrt[h, 0, 4 * gq:4 * gq + 4, :], in_=rts[:])
              nc.sync.dma_start(out=recrow[h][:, 4 * gq:4 * gq + 4, :],
                                in_=lrt[h, :, 4 * gq:4 * gq + 4, :])

          def denom(h, b):
              # ell[q] for q-block b: sum_k exp tiles via ap-1 matmuls, then recip
              lp = smt[:, 128 + (b % 4):129 + (b % 4)]
              for j in range(b + 1):
                  nc.tensor.matmul(lp, expT[h][j][:, (b - j) * 128:(b - j + 1) * 128],
                                   tones[:], start=(j == 0), stop=(j == b))
              nc.vector.reciprocal(out=lrec_cur[h][:, b:b + 1], in_=lp)

          def sweep2_group(h, gq, split_at=None, mid_fn=None):
              # ct = (sum_k V^T[k] expS^T[k]) * recip -> split into fp8 hi/lo
              rbs = rbp.tile([128, 512], F32, tag="rbs")
              nc.gpsimd.partition_broadcast(
                  rbs[:], recrow[h][:, 4 * gq:4 * gq + 4, :])
              ct = mmp.tile([128, 512], F32, tag="mm")
              last = 4 * gq + 3
              for j in range(last + 1):
                  if split_at is not None and j == split_at:
                      mid_fn()
                  if j <= 4 * gq:
                      nc.tensor.matmul(ct[:], vres[j][:, h * 128:(h + 1) * 128],
                                       expT[h][j][:, (4 * gq - j) * 128:(4 * gq - j) * 128 + 512],
                                       start=(j == 0), stop=(j == last))
                  else:
                      w = (4 * gq + 4 - j) * 128
                      nc.tensor.matmul(ct[:, 512 - w:512], vres[j][:, h * 128:(h + 1) * 128],
                                       expT[h][j][:, 0:w], start=False, stop=(j == last))
              ctb = ctsp.tile([128, 512], BF16, tag="ctb")
              nc.vector.tensor_tensor(out=ctb[:], in0=ct[:], in1=rbs[:], op=MULT)
              nc.gpsimd.tensor_scalar(out=cth[(h, gq)][:], in0=ctb[:],
                                      scalar1=tbvc[:, h:h + 1], scalar2=None,
                                      op0=ADD)
              nc.vector.scalar_tensor_tensor(
                  out=ctl[gq][:, h, :], in0=ctb[:], scalar=tbvc[:, h:h + 1],
                  in1=cth[(h, gq)][:], op0=ADD, op1=SUB)

          def scores_head(h, interleave):
              expT[h] = []
              lrec_cur[h] = lrp.tile([128, 16], F32, tag="lrec", name="lrec", bufs=1)
              recrow[h] = lrp.tile([1, 16, 128], F32, tag="recrow", name="recrow", bufs=1)
              kp = kpair[h]
              qf = qf8[h]
              for i in range(NB):
                  w = (NB - i) * 128
                  ex = expp.tile([128, w], BF16, tag=f"expT{i}", name=f"expT{i}")
                  expT[h].append(ex)
              # chunk list; head 0 emits in slab-availability wavefront order
              chunks = []
              for i in range(NB):
                  w = (NB - i) * 128
                  for c0 in range(0, w, 512):
                      cw = min(512, w - c0)
                      p = max((i * 128 + c0 + cw - 1) // 512, i // 4)
                      chunks.append((p, i, c0, cw))
              if h == 0:
                  chunks.sort()
              nch = len(chunks)
              per_block = [(((NB - i) * 128) + 511) // 512 for i in range(NB)]
              done = [0] * NB
              dfired = 0
              mi = 0
              if h == 0:
                  milestones = [((i + 1) * nch + NB - 1) // NB for i in range(NB)]
              else:
                  acc = 0
                  milestones = []
                  for i in range(NB):
                      acc += per_block[i]
                      milestones.append(acc)
              emitted_p = [-1]

              def on_block_complete(b):
                  # fire denoms (2-block lag), recips, and own sweeps
                  nonlocal dfired
                  while dfired <= b - 2:
                      d = dfired
                      denom(h, d)
                      dfired += 1
                      if d >= 3 and d % 4 == 3:
                          rec_group(h, d // 4)
                      if d >= 5 and (d - 5) % 4 == 0 and (d - 5) // 4 <= 2:
                          sweep2_group(h, (d - 5) // 4)

              for n, (p, i, c0, cw) in enumerate(chunks, 1):
                  if h == 0 and p > emitted_p[0]:
                      for ns_ in range(emitted_p[0] + 1, p + 1):
                          chain_qk(0, "q", ns_)
                          chain_qk(0, "k", ns_)
                      emitted_p[0] = p
                  ex = expT[h][i]
                  sp = sps.tile([128, 512], F32, tag="sp")
                  q0 = i * 128 + c0
                  nc.tensor.matmul(
                      sp[:, 0:cw], kp[:, :, i * 128:(i + 1) * 128],
                      qf[:, q0:q0 + cw].unsqueeze(1).broadcast_to((128, 2, cw)),
                      start=True, stop=True, perf_mode=DR)
                  nc.scalar.activation(out=ex[:, c0:c0 + cw], in_=sp[:, 0:cw],
                                       func=EXPF, scale=EXP_SCALE)
                  if c0 == 0:
                      nc.gpsimd.tensor_tensor(out=ex[:, 0:128], in0=ex[:, 0:128],
                                              in1=tmask[:], op=MULT)
                  done[i] += 1
                  if done[i] == per_block[i]:
                      on_block_complete(i)
                  while mi < NB and n >= milestones[mi]:
                      interleave(h, mi)
                      mi += 1
              while dfired < NB:
                  d = dfired
                  denom(h, d)
                  dfired += 1
                  if d >= 3 and d % 4 == 3:
                      rec_group(h, d // 4)
                  if d >= 5 and (d - 5) % 4 == 0 and (d - 5) // 4 <= 2:
                      sweep2_group(h, (d - 5) // 4)

          def alloc_qk(h):
              qf8[h] = qkp.tile([128, S], F8, tag="qf8", name=f"qf8_{h}")
              kpair[h] = qkp.tile([128, 2, S], F8, tag="kpair", name=f"kpair_{h}")

          wo1t = []
          wo2t = []
          tailp = ExitStack()

          def open_wop():
              xps.close()
              wop = tailp.enter_context(tc.tile_pool(name="wop", bufs=1))
              for kh in range(HG):
                  wt_ = wop.tile([128, 2, D], F8, tag=f"wo1_{kh}", name=f"wo1_{kh}")
                  nc.sync.dma_start(out=wt_, in_=wo1[kh])
                  wo1t.append(wt_)
              for gp in range(2):
                  wt_ = wop.tile([128, 2, D], F8, tag=f"wo2_{gp}", name=f"wo2_{gp}")
                  nc.sync.dma_start(out=wt_, in_=wo2[gp])
                  wo2t.append(wt_)

          ob_cur = [None]

          def p3_mt(gq, mt, tail=False, insec=False):
              if tail and mt % 2 == 0:
                  op = sps.tile([128, 512], F32, tag="sp")
              else:
                  op = mmp.tile([128, 512], F32, tag="mm")
              for kh in range(HG):
                  nc.tensor.matmul(
                      op[:], wo1t[kh][:, :, mt * 128:(mt + 1) * 128],
                      cth[(kh, gq)][:].unsqueeze(1).broadcast_to((128, 2, 512)),
                      start=(kh == 0), stop=False, perf_mode=DR)
              for gp in range(2):
                  nc.tensor.matmul(
                      op[:], wo2t[gp][:, :, mt * 128:(mt + 1) * 128],
                      ctl[gq][:, 2 * gp:2 * gp + 2, :],
                      start=False, stop=(gp == 1), perf_mode=DR)
              if mt % 2 == 0:
                  ob_cur[0] = obp.tile([128, 2, 512], BF16, tag="ob", name="ob")
              ob = ob_cur[0]
              half = ob[:, mt % 2, :]
              if mt % 2 == 0:
                  nc.vector.tensor_copy(out=half, in_=op[:])
              else:
                  nc.scalar.copy(out=half, in_=op[:])
              if mt % 2 == 1:
                  nc.sync.dma_start(
                      out=outt[mt - 1:mt + 1, :, gq * 512:(gq + 1) * 512]
                      .rearrange("m p s -> p m s"),
                      in_=ob[:])

          def mk_sched(h):
              # schedule of extra PE work per score block of head h
              sched = {i: [] for i in range(NB)}
              if h == 0:
                  sched[2].append(lambda: alloc_qk(1))
                  for idx, (which, ns) in enumerate(
                          (w, n) for n in range(4) for w in ("q", "k")):
                      sched[2 + idx].append(
                          lambda w=which, n=ns: chain_qk(1, w, n))
                  sched[2].append(lambda: xl_dma(2))
                  sched[5].append(lambda: xl_dma(3))
                  for t in range(13):           # v tiles 0-12 at blocks 3-15
                      sched[3 + t].append(lambda t=t: v_tile(t))
                  sched[10].append(lambda: weights_dma(2))
              else:
                  if h == 1:
                      def _mid():
                          for t in (13, 14, 15):
                              v_tile(t)
                          sec0.close()
                      sched[0].append(
                          lambda: sweep2_group(0, 3, split_at=13, mid_fn=_mid))
                  else:
                      sched[0].append(lambda: sweep2_group(h - 1, 3))
                  if h < 3:
                      sched[2].append(lambda: alloc_qk(h + 1))
                      for idx, (which, ns) in enumerate(
                              (w, n) for n in range(4) for w in ("q", "k")):
                          sched[2 + idx].append(
                              lambda w=which, n=ns: chain_qk(h + 1, w, n))
                      if h + 2 < HG:
                          sched[10].append(lambda: weights_dma(h + 2))
                  if h == 2:
                      sched[6].append(open_wop)
                  if h == 3:
                      for j in range(16):       # p3 gq0 at blocks 8-15
                          sched[8 + j // 2].append(
                              lambda mt=j: p3_mt(0, mt, insec=True))
              return sched

          def run_sched(sched, h, i):
              for fn in sched[i]:
                  fn()

          # ---- emit sections ----
          alloc_qk(0)
          weights_dma(1)
          wv_dmas("h")
          xl_dma(0)
          wv_dmas("l")
          xl_dma(1)
          for h in range(HG):
              sched = mk_sched(h)
              scores_head(h, lambda hh, i, sched=sched: run_sched(sched, hh, i))

          # ---- tail: rest of the output projection ----
          for mt in range(16):
              p3_mt(1, mt, tail=True)
          sweep2_group(3, 3)
          for mt in range(16):
              p3_mt(2, mt, tail=True)
          for mt in range(16):
              p3_mt(3, mt, tail=True)
          tailp.close()
          phA.close()
          phB.close()
    nc.finalize()
    return nc


_NC_CACHE = {}


def _get_nc(reps=1):
    if reps not in _NC_CACHE:
        _NC_CACHE[reps] = build_nc(reps)
    return _NC_CACHE[reps]


def _rope_tables(position_ids_b):
    pos = position_ids_b.astype(np.float32)
    inv_freq = (1.0 / (ROPE_THETA ** (np.arange(0, DH, 2, dtype=np.float32) / np.float32(DH))))
    ang = pos[:, None] * inv_freq[None, :]          # [S, 64]
    emb = np.concatenate([ang, ang], axis=-1)       # [S, 128]
    cosT = np.ascontiguousarray(np.cos(emb).T) * np.float32(QSC)   # [128, S]
    sinT = np.sin(emb).T * np.float32(QSC)
    sin_rot = np.concatenate([-sinT[0:64], sinT[64:128]], axis=0)
    return cosT.astype(ml_dtypes.bfloat16), np.ascontiguousarray(sin_rot).astype(ml_dtypes.bfloat16)


def _make_in_maps(inputs):
    hidden_states = np.asarray(inputs["hidden_states"], dtype=np.float32)
    position_ids = np.asarray(inputs["position_ids"])
    Wqkv = np.asarray(inputs["Wqkv"], dtype=np.float32)
    bqkv = np.asarray(inputs["bqkv"], dtype=np.float32)
    Wo = np.asarray(inputs["Wo"], dtype=np.float32)

    mask = np.triu(np.ones((128, 128), dtype=np.float32)).astype(ml_dtypes.bfloat16)
    tones = np.full((128, 1), 1.0 / CSC, dtype=ml_dtypes.bfloat16)
    tabs = [_rope_tables(np.asarray(position_ids)[b]) for b in range(B)]

    def _hilo(M, sc):
        Ms = M * np.float32(sc)
        hi = Ms.astype(ml_dtypes.float8_e4m3)
        lo = (Ms - hi.astype(np.float32)).astype(ml_dtypes.float8_e4m3)
        return hi, lo

    def _pack_pairs(M):
        # [D, C] -> [8, 128, 2, C] with row r = kc2*256 + i*128 + p
        C = M.shape[1]
        return np.ascontiguousarray(M.reshape(8, 2, 128, C).transpose(0, 2, 1, 3))

    def _pack_x(M):
        # [D, S] -> [128, 4, 8, 2, SLAB] partition-major, slab-major free
        return np.ascontiguousarray(
            M.reshape(8, 2, 128, 4, SLAB).transpose(2, 3, 0, 1, 4))

    xts = []
    for b in range(B):
        hi, lo = _hilo(np.ascontiguousarray(hidden_states[b].T), XSC)
        xts.append((_pack_x(hi.astype(np.float32)).astype(ml_dtypes.float8_e4m3),
                    _pack_x(lo.astype(np.float32)).astype(ml_dtypes.float8_e4m3)))

    in_maps = []
    for c in range(NCORES):
        b, hg = divmod(c, HG)
        qcols = slice(hg * GQ, (hg + 1) * GQ)
        kcols = slice(D + hg * GQ, D + (hg + 1) * GQ)
        vcols = slice(2 * D + hg * GQ, 2 * D + (hg + 1) * GQ)
        wqk_c = np.ascontiguousarray(np.concatenate([Wqkv[:, qcols], Wqkv[:, kcols]], axis=1))
        qk_h, qk_l = _hilo(wqk_c, WSC)
        # per-mt packing: [8(mt), 128(p), 8(kc2), 2(i), 128(m)]
        def _pack_mt(M8):
            P = _pack_pairs(M8.astype(np.float32))          # [8, 128, 2, 1024]
            P = P.reshape(8, 128, 2, 8, 128)                 # [kc2, p, i, mt, m]
            return np.ascontiguousarray(P.transpose(3, 1, 0, 2, 4)).astype(ml_dtypes.float8_e4m3)
        wqkh_c = _pack_mt(qk_h)
        wqkl_c = _pack_mt(qk_l)
        wv_c = np.ascontiguousarray(Wqkv[:, vcols])
        v_h, v_l = _hilo(wv_c, WSC)
        wvh_c = _pack_pairs(v_h.astype(np.float32)).astype(ml_dtypes.float8_e4m3)
        wvl_c = _pack_pairs(v_l.astype(np.float32)).astype(ml_dtypes.float8_e4m3)
        # out-proj fp8 packs
        wo_c = np.ascontiguousarray(Wo[hg * GQ:(hg + 1) * GQ, :])   # [512, D]
        wo_h, wo_l = _hilo(wo_c, WSC)
        wo_h = wo_h.astype(np.float32)
        wo_l = wo_l.astype(np.float32)
        wo1_c = np.empty((HG, 128, 2, D), np.float32)
        for kh in range(HG):
            wo1_c[kh, :, 0, :] = wo_h[kh * 128:(kh + 1) * 128, :]
            wo1_c[kh, :, 1, :] = wo_l[kh * 128:(kh + 1) * 128, :]
        wo2_c = np.empty((2, 128, 2, D), np.float32)
        for gp in range(2):
            wo2_c[gp, :, 0, :] = wo_h[gp * 256:gp * 256 + 128, :]
            wo2_c[gp, :, 1, :] = wo_h[gp * 256 + 128:gp * 256 + 256, :]
        bqk_c = np.concatenate([bqkv[qcols], bqkv[kcols]]).reshape(8, 128).T
        bvc_c = np.ascontiguousarray(bqkv[vcols].reshape(HG, 128).T * np.float32(CSC))
        cosT, sin_rot = tabs[b]
        in_maps.append({
            "xh": xts[b][0], "xl": xts[b][1],
            "wqkh": wqkh_c, "wqkl": wqkl_c, "wvh": wvh_c, "wvl": wvl_c,
            "wo1": wo1_c.astype(ml_dtypes.float8_e4m3),
            "wo2": wo2_c.astype(ml_dtypes.float8_e4m3),
            "bqkt": np.ascontiguousarray(bqk_c),
            "bvc": bvc_c,
            "cost": cosT, "sinrt": sin_rot, "maskd": mask,
            "tonesd": tones,
        })
    return in_maps


def kernel(hidden_states, position_ids, Wqkv, bqkv, Wo, bo, _reps=1):
    bo = np.asarray(bo, dtype=np.float32)
    in_maps = _make_in_maps({
        "hidden_states": hidden_states, "position_ids": position_ids,
        "Wqkv": Wqkv, "bqkv": bqkv, "Wo": Wo, "bo": bo,
    })
    nc = _get_nc(_reps)
    res = run_bass_kernel_spmd(nc, in_maps, core_ids=list(range(NCORES)))

    out = np.empty((B, S, D), dtype=np.float32)
    for b in range(B):
        acc = res.results[b * HG]["outt"].reshape(D, S).astype(np.float32).copy()
        for hg in range(1, HG):
            acc += res.results[b * HG + hg]["outt"].reshape(D, S).astype(np.float32)
        out[b] = acc.T * np.float32(P3_SCALE) + bo[None, :]
    return out


# revision 35
# speedup vs baseline: 1.3239x; 1.0273x over previous
"""Trainium2 Bass kernel for CustomRoPEAttention (B=2, S=2048, H=16, Dh=128).

Sharding: 8 cores = 2 batches x 4 head-groups (4 heads/core).

Head-pipelined structure: per head h, QKV^T projection (fp8 hi/lo DoubleRow
matmuls) + RoPE, then transposed-layout causal attention for that head while
later heads' projections stream -- this overlaps the ACT-engine exp work with
PE-engine matmul work across the whole kernel instead of serializing phases.

fp8 DoubleRow "dup trick": scores use stationary (k_hi,k_lo) pairs against a
broadcast (step-0) fp8 q moving operand, and the output projection uses
(wo_hi,wo_lo) pairs against broadcast ct_hi plus a wo_hi x ct_lo correction --
half / 0.75x the bf16 PE time at first-order-exact precision.

Host sums the 4 partial (transposed) output projections per batch.

Self-contained: hardcodes shapes from the problem spec.
"""
import math
from contextlib import ExitStack

import numpy as np
import ml_dtypes

import concourse.mybir as mybir
import concourse.tile as tile
from concourse import bacc
from concourse.bass_utils import run_bass_kernel_spmd
from concourse.masks import make_identity

S = 2048            # sequence
D = 2048            # hidden
NH = 16             # total heads
DH = 128            # head dim
HG = 4              # heads per core
GQ = HG * DH        # 512: per-core q/k/v feature width
B = 2
NCORES = 8
ROPE_THETA = 10000.0
SCALE = 1.0 / math.sqrt(DH)
SLAB = 512          # qkv sequence slab width
XSC = 16.0          # fp8 pre-scale for x
WSC = 512.0         # fp8 pre-scale for Wqkv / Wo
QSC = 16.0          # fp8 pre-scale for roped q/k (folded into cos/sin tables)
CSC = 16.0          # fp8 pre-scale for attention-out ct (folded into tones)
INV_SC = 1.0 / (XSC * WSC)
EXP_SCALE = SCALE / (QSC * QSC)
P3_SCALE = 1.0 / (WSC * CSC)   # applied host-side
F32 = mybir.dt.float32
BF16 = mybir.dt.bfloat16
F8 = mybir.dt.float8e4
MULT = mybir.AluOpType.mult
ADD = mybir.AluOpType.add
SUB = mybir.AluOpType.subtract
DR = mybir.MatmulPerfMode.DoubleRow
NB = S // 128       # 16 k/q blocks
IDENT = mybir.ActivationFunctionType.Identity
EXPF = mybir.ActivationFunctionType.Exp


def build_nc(reps=1, knobs=None):
    kn = {"sps": 3, "mmp": 4, "expb": 1, "wqb": 2, "qkb": 2, "stg": 2, "obp": 4}
    if knobs:
        kn.update(knobs)
    nc = bacc.Bacc(None, target_bir_lowering=False)
    # x^T hi/lo, slab-major pack: [p, ns, kc2, i, s]
    xh = nc.dram_tensor("xh", [128, 4, 8, 2, SLAB], F8, kind="ExternalInput")
    xl = nc.dram_tensor("xl", [128, 4, 8, 2, SLAB], F8, kind="ExternalInput")
    # per-mt packed qk weights: [mt, p, kc2, i, m]
    wqkh = nc.dram_tensor("wqkh", [8, 128, 8, 2, 128], F8, kind="ExternalInput")
    wvh = nc.dram_tensor("wvh", [8, 128, 2, GQ], F8, kind="ExternalInput")
    wvl = nc.dram_tensor("wvl", [8, 128, 2, GQ], F8, kind="ExternalInput")
    # out-proj fp8 packs: wo1[kh] = (hi,lo) pairs; wo2[g] = hi head-pair packs
    wo1 = nc.dram_tensor("wo1", [HG, 128, 2, D], F8, kind="ExternalInput")
    wo2 = nc.dram_tensor("wo2", [2, 128, 2, D], F8, kind="ExternalInput")
    bqkt = nc.dram_tensor("bqkt", [128, 8], F32, kind="ExternalInput")
    bvc = nc.dram_tensor("bvc", [128, HG], F32, kind="ExternalInput")  # v bias * CSC
    cost = nc.dram_tensor("cost", [128, S], BF16, kind="ExternalInput")    # cos^T * QSC
    sinrt = nc.dram_tensor("sinrt", [128, S], BF16, kind="ExternalInput")  # sin^T * QSC, rot sign
    maskd = nc.dram_tensor("maskd", [128, 128], BF16, kind="ExternalInput")  # triu 0/1 keep-mask
    tonesd = nc.dram_tensor("tonesd", [128, 1], BF16, kind="ExternalInput")  # 1/CSC
    outt = nc.dram_tensor("outt", [16, 128, S], BF16, kind="ExternalOutput")
    lrt = nc.dram_tensor("lrt", [HG, 1, 16, 128], F32)  # recip bounce: [16,128] -> [1,2048]

    with tile.TileContext(nc) as tc, ExitStack() as top:
        g = top.enter_context(tc.tile_pool(name="glob", bufs=1))
        tcos = g.tile([128, S], BF16)
        tsin = g.tile([128, S], BF16)
        tmask = g.tile([128, 128], BF16)
        ident_f = g.tile([128, 128], F32)
        make_identity(nc, ident_f[:])
        tbqkt = g.tile([128, 8], F32)
        tbvc = g.tile([128, HG], F32)
        tones = g.tile([128, 1], BF16)
        tinv = g.tile([128, 1], F32)
        nc.vector.memset(tinv[:], INV_SC)

        # Whole-kernel residents
        res = top.enter_context(tc.tile_pool(name="res", bufs=1))
        vres = []  # 16 V k-block tiles [128(seq), GQ] bf16
        for t in range(NB):
            vres.append(res.tile([128, GQ], BF16, tag=f"v{t}", name=f"v{t}"))
        cth = {}
        for h in range(HG):
            for gq in range(4):
                cth[(h, gq)] = res.tile([128, 512], F8, tag=f"cth_{h}_{gq}",
                                        name=f"cth_{h}_{gq}")
        ctl = [res.tile([128, HG, 512], F8, tag=f"ctl{gq}", name=f"ctl{gq}")
               for gq in range(4)]

        for _rep in range(reps):
          phB = ExitStack()
          mmp = phB.enter_context(tc.tile_pool(name="mmp", bufs=kn["mmp"], space="PSUM"))
          sps = phB.enter_context(tc.tile_pool(name="sps", bufs=kn["sps"], space="PSUM"))
          smps = phB.enter_context(tc.tile_pool(name="smps", bufs=1, space="PSUM"))
          expp = phB.enter_context(tc.tile_pool(name="expp", bufs=kn["expb"]))
          lrp = phB.enter_context(tc.tile_pool(name="lrp", bufs=2))
          rbp = phB.enter_context(tc.tile_pool(name="rbp", bufs=2))
          ctsp = phB.enter_context(tc.tile_pool(name="ctsp", bufs=2))
          obp = phB.enter_context(tc.tile_pool(name="obp", bufs=kn["obp"]))
          smt = smps.tile([128, 132], F32, tag="sm", name="smt")

          phA = ExitStack()
          wqp = phA.enter_context(tc.tile_pool(name="wqp", bufs=kn["wqb"]))
          qkp = phA.enter_context(tc.tile_pool(name="qkp", bufs=kn["qkb"]))
          stg = phA.enter_context(tc.tile_pool(name="stg", bufs=kn["stg"]))
          xps = ExitStack()
          xp = xps.enter_context(tc.tile_pool(name="xp", bufs=1))
          sec0 = ExitStack()
          xsl = sec0.enter_context(tc.tile_pool(name="xsl", bufs=2))
          wvp = sec0.enter_context(tc.tile_pool(name="wvp", bufs=1))

          # ---- initial DMA order (startup-critical) ----
          wq_tiles = {}  # (h) -> (wqh, wql, wkh, wkl)

          def weights_dma(h):
              tl = []
              for mt, tag in ((h, "wqh"), (4 + h, "wkh")):
                  wt = wqp.tile([128, 8, 2, 128], F8, tag=tag, name=f"{tag}{h}")
                  nc.sync.dma_start(out=wt, in_=wqkh[mt])
                  tl.append(wt)
              wq_tiles[h] = tl

          # head-0 weights + first x slab first
          wt = wqp.tile([128, 8, 2, 128], F8, tag="wqh", name="wqh0")
          nc.sync.dma_start(out=wt[:, 0:2], in_=wqkh[0, :, 0:2])
          xres = [xp.tile([128, 8, 2, SLAB], F8, tag=f"x{ns}", name=f"x{ns}")
                  for ns in range(4)]
          nc.sync.dma_start(out=xres[0][:, 0:2], in_=xh[:, 0, 0:2])
          nc.sync.dma_start(out=wt[:, 2:8], in_=wqkh[0, :, 2:8])
          nc.sync.dma_start(out=xres[0][:, 2:5], in_=xh[:, 0, 2:5])
          wt2 = wqp.tile([128, 8, 2, 128], F8, tag="wkh", name="wkh0")
          nc.sync.dma_start(out=wt2, in_=wqkh[4])
          wq_tiles[0] = [wt, wt2]
          nc.sync.dma_start(out=xres[0][:, 5:8], in_=xh[:, 0, 5:8])
          # small consts needed by first psum copies / rope
          nc.sync.dma_start(out=tbqkt, in_=bqkt[:])
          nc.sync.dma_start(out=xres[1][:, 0:4], in_=xh[:, 1, 0:4])
          nc.sync.dma_start(out=tcos, in_=cost[:])
          nc.sync.dma_start(out=xres[1][:, 4:8], in_=xh[:, 1, 4:8])
          nc.sync.dma_start(out=tsin, in_=sinrt[:])
          nc.sync.dma_start(out=xres[2][:, 0:4], in_=xh[:, 2, 0:4])
          nc.sync.dma_start(out=xres[2][:, 4:8], in_=xh[:, 2, 4:8])
          nc.sync.dma_start(out=tones, in_=tonesd[:])
          nc.sync.dma_start(out=xres[3][:, 0:4], in_=xh[:, 3, 0:4])
          nc.sync.dma_start(out=tmask, in_=maskd[:])
          nc.sync.dma_start(out=xres[3][:, 4:8], in_=xh[:, 3, 4:8])
          nc.sync.dma_start(out=tbvc, in_=bvc[:])
          twvh, twvl = [], []
          xlres = {}

          def wv_dmas(which):
              if which == "h":
                  wvht = wvp.tile([128, 8, 2, GQ], F8, tag="wvh", name="wvht")
                  nc.sync.dma_start(out=wvht, in_=wvh.rearrange("a p b c -> p a b c"))
                  twvh.append(wvht)
              else:
                  wvlt = wvp.tile([128, 8, 2, GQ], F8, tag="wvl", name="wvlt")
                  nc.sync.dma_start(out=wvlt, in_=wvl.rearrange("a p b c -> p a b c"))
                  twvl.append(wvlt)

          def xl_dma(ns):
              xt = xsl.tile([128, 8, 2, SLAB], F8, tag="xl", name=f"xl{ns}")
              nc.sync.dma_start(out=xt, in_=xl[:, ns])
              xlres[ns] = xt

          # ---- per-head state ----
          qf8 = {}
          kpair = {}
          expT = {}
          lrec_cur = {}
          recrow = {}

          def chain_qk(h, which, ns):
              wqh_, wkh_ = wq_tiles[h]
              whi = wqh_ if which == "q" else wkh_
              sl = slice(ns * SLAB, (ns + 1) * SLAB)
              ps = mmp.tile([128, SLAB], F32, tag="mm")
              for kc2 in range(8):
                  nc.tensor.matmul(ps[:], whi[:, kc2, :, :], xres[ns][:, kc2, :, :],
                                   start=(kc2 == 0), stop=(kc2 == 7), perf_mode=DR)
              st = stg.tile([128, SLAB], BF16, tag="st")
              mt = h if which == "q" else 4 + h
              nc.vector.tensor_scalar(out=st[:], in0=ps[:], scalar1=INV_SC,
                                      scalar2=tbqkt[:, mt:mt + 1],
                                      op0=MULT, op1=ADD)
              # RoPE: out = st*cos + swap(st)*sin_rot   (tables pre-scaled by QSC)
              sw = stg.tile([128, SLAB], BF16, tag="sw")
              nc.vector.tensor_copy(out=sw[0:64, :], in_=st[64:128, :])
              nc.vector.tensor_copy(out=sw[64:128, :], in_=st[0:64, :])
              m1 = stg.tile([128, SLAB], BF16, tag="m1")
              nc.vector.tensor_tensor(out=m1[:], in0=st[:], in1=tcos[:, sl], op=MULT)
              nc.vector.tensor_tensor(out=sw[:], in0=sw[:], in1=tsin[:, sl], op=MULT)
              if which == "q":
                  nc.vector.tensor_tensor(out=qf8[h][:, sl], in0=m1[:], in1=sw[:], op=ADD)
              else:
                  kb = m1
                  nc.vector.tensor_tensor(out=kb[:], in0=m1[:], in1=sw[:], op=ADD)
                  if h == 0:
                      nc.scalar.copy(out=kpair[h][:, 0, sl], in_=kb[:])
                      nc.vector.tensor_tensor(out=kpair[h][:, 1, sl], in0=kb[:],
                                              in1=kpair[h][:, 0, sl], op=SUB)
                  else:
                      nc.gpsimd.tensor_scalar(out=kpair[h][:, 0, sl], in0=kb[:],
                                              scalar1=0.0, scalar2=None, op0=ADD)
                      nc.gpsimd.tensor_tensor(out=kpair[h][:, 1, sl], in0=kb[:],
                                              in1=kpair[h][:, 0, sl], op=SUB)

          def v_tile(t):
              ns, sti = divmod(t, 4)
              s0 = sti * 128
              pv = mmp.tile([128, GQ], F32, tag="mm")
              passes = [(xres[ns], twvh[0]), (xlres[ns], twvh[0]), (xres[ns], twvl[0])]
              for pi, (xt_, wv_) in enumerate(passes):
                  for kc2 in range(8):
                      nc.tensor.matmul(pv[:], xt_[:, kc2, :, s0:s0 + 128],
                                       wv_[:, kc2, :, :], start=(pi == 0 and kc2 == 0),
                                       stop=(pi == 2 and kc2 == 7), perf_mode=DR)
              nc.scalar.activation(out=vres[t], in_=pv[:], func=IDENT,
                                   scale=INV_SC)

          def rec_group(h, gq):
              # ship recip(ell) for q-blocks 4g..4g+3 to DRAM and back as a row
              rt = smt[0:4, 0:128]
              nc.tensor.transpose(rt, lrec_cur[h][:, 4 * gq:4 * gq + 4], ident_f[:])
              rts = lrp.tile([4, 128], F32, tag="rts")
              nc.vector.tensor_copy(out=rts[:], in_=rt)
              nc.sync.dma_start(out=lrt[h, 0, 4 * gq:4 * gq + 4, :], in_=rts[:])
              nc.sync.dma_start(out=recrow[h][:, 4 * gq:4 * gq + 4, :],
                                in_=lrt[h, :, 4 * gq:4 * gq + 4, :])

          def denom(h, b):
              # ell[q] for q-block b: sum_k exp tiles via ap-1 matmuls, then recip
              lp = smt[:, 128 + (b % 4):129 + (b % 4)]
              for j in range(b + 1):
                  nc.tensor.matmul(lp, expT[h][j][:, (b - j) * 128:(b - j + 1) * 128],
                                   tones[:], start=(j == 0), stop=(j == b))
              nc.vector.reciprocal(out=lrec_cur[h][:, b:b + 1], in_=lp)

          def sweep2_group(h, gq, split_at=None, mid_fn=None):
              # ct = (sum_k V^T[k] expS^T[k]) * recip -> split into fp8 hi/lo
              rbs = rbp.tile([128, 512], F32, tag="rbs")
              nc.gpsimd.partition_broadcast(
                  rbs[:], recrow[h][:, 4 * gq:4 * gq + 4, :])
              ct = mmp.tile([128, 512], F32, tag="mm")
              last = 4 * gq + 3
              for j in range(last + 1):
                  if split_at is not None and j == split_at:
                      mid_fn()
                  if j <= 4 * gq:
                      nc.tensor.matmul(ct[:], vres[j][:, h * 128:(h + 1) * 128],
                                       expT[h][j][:, (4 * gq - j) * 128:(4 * gq - j) * 128 + 512],
                                       start=(j == 0), stop=(j == last))
                  else:
                      w = (4 * gq + 4 - j) * 128
                      nc.tensor.matmul(ct[:, 512 - w:512], vres[j][:, h * 128:(h + 1) * 128],
                                       expT[h][j][:, 0:w], start=False, stop=(j == last))
              ctb = ctsp.tile([128, 512], BF16, tag="ctb")
              nc.vector.tensor_tensor(out=ctb[:], in0=ct[:], in1=rbs[:], op=MULT)
              nc.gpsimd.tensor_scalar(out=cth[(h, gq)][:], in0=ctb[:],
                                      scalar1=tbvc[:, h:h + 1], scalar2=None,
                                      op0=ADD)
              nc.vector.scalar_tensor_tensor(
                  out=ctl[gq][:, h, :], in0=ctb[:], scalar=tbvc[:, h:h + 1],
                  in1=cth[(h, gq)][:], op0=ADD, op1=SUB)

          def scores_head(h, interleave):
              expT[h] = []
              lrec_cur[h] = lrp.tile([128, 16], F32, tag="lrec", name="lrec", bufs=1)
              recrow[h] = lrp.tile([1, 16, 128], F32, tag="recrow", name="recrow", bufs=1)
              kp = kpair[h]
              qf = qf8[h]
              for i in range(NB):
                  w = (NB - i) * 128
                  ex = expp.tile([128, w], BF16, tag=f"expT{i}", name=f"expT{i}",
                                 bufs=2 if i < 4 else 1)
                  expT[h].append(ex)
              # chunk list; head 0 emits in slab-availability wavefront order
              chunks = []
              for i in range(NB):
                  w = (NB - i) * 128
                  for c0 in range(0, w, 512):
                      cw = min(512, w - c0)
                      p = max((i * 128 + c0 + cw - 1) // 512, i // 4)
                      chunks.append((p, i, c0, cw))
              if h == 0:
                  chunks.sort()
              nch = len(chunks)
              per_block = [0] * NB
              for _p, i_, _c, _w in chunks:
                  per_block[i_] += 1
              done = [0] * NB
              dfired = 0
              mi = 0
              if h == 0:
                  milestones = [((i + 1) * nch + NB - 1) // NB for i in range(NB)]
              else:
                  acc = 0
                  milestones = []
                  for i in range(NB):
                      acc += per_block[i]
                      milestones.append(acc)
              emitted_p = [-1]

              def on_block_complete(b):
                  # fire denoms (2-block lag), recips, and own sweeps
                  nonlocal dfired
                  while dfired <= b - 2:
                      d = dfired
                      denom(h, d)
                      dfired += 1
                      if d >= 3 and d % 4 == 3:
                          rec_group(h, d // 4)
                      if d >= 5 and (d - 5) % 4 == 0 and (d - 5) // 4 <= 2:
                          sweep2_group(h, (d - 5) // 4)

              for n, (p, i, c0, cw) in enumerate(chunks, 1):
                  if h == 0 and p > emitted_p[0]:
                      for ns_ in range(emitted_p[0] + 1, p + 1):
                          chain_qk(0, "q", ns_)
                          chain_qk(0, "k", ns_)
                      emitted_p[0] = p
                  ex = expT[h][i]
                  sp = sps.tile([128, 512], F32, tag="sp")
                  q0 = i * 128 + c0
                  nc.tensor.matmul(
                      sp[:, 0:cw], kp[:, :, i * 128:(i + 1) * 128],
                      qf[:, q0:q0 + cw].unsqueeze(1).broadcast_to((128, 2, cw)),
                      start=True, stop=True, perf_mode=DR)
                  nc.scalar.activation(out=ex[:, c0:c0 + cw], in_=sp[:, 0:cw],
                                       func=EXPF, scale=EXP_SCALE)
                  if c0 == 0:
                      nc.gpsimd.tensor_tensor(out=ex[:, 0:128], in0=ex[:, 0:128],
                                              in1=tmask[:], op=MULT)
                  done[i] += 1
                  if done[i] == per_block[i]:
                      on_block_complete(i)
                  while mi < NB and n >= milestones[mi]:
                      interleave(h, mi)
                      mi += 1
              while dfired < NB:
                  d = dfired
                  denom(h, d)
                  dfired += 1
                  if d >= 3 and d % 4 == 3:
                      rec_group(h, d // 4)
                  if d >= 5 and (d - 5) % 4 == 0 and (d - 5) // 4 <= 2:
                      sweep2_group(h, (d - 5) // 4)

          def alloc_qk(h):
              qf8[h] = qkp.tile([128, S], F8, tag="qf8", name=f"qf8_{h}")
              kpair[h] = qkp.tile([128, 2, S], F8, tag="kpair", name=f"kpair_{h}")

          wo1t = []
          wo2t = []
          tailp = ExitStack()

          def open_wop():
              xps.close()
              wop = tailp.enter_context(tc.tile_pool(name="wop", bufs=1))
              for kh in range(HG):
                  wt_ = wop.tile([128, 2, D], F8, tag=f"wo1_{kh}", name=f"wo1_{kh}")
                  nc.sync.dma_start(out=wt_, in_=wo1[kh])
                  wo1t.append(wt_)
              for gp in range(2):
                  wt_ = wop.tile([128, 2, D], F8, tag=f"wo2_{gp}", name=f"wo2_{gp}")
                  nc.sync.dma_start(out=wt_, in_=wo2[gp])
                  wo2t.append(wt_)

          ob_cur = [None]

          def p3_mt(gq, mt, tail=False, insec=False):
              if tail and mt % 2 == 0:
                  op = sps.tile([128, 512], F32, tag="sp", name="op")
              else:
                  op = mmp.tile([128, 512], F32, tag="mm")
              for kh in range(HG):
                  nc.tensor.matmul(
                      op[:], wo1t[kh][:, :, mt * 128:(mt + 1) * 128],
                      cth[(kh, gq)][:].unsqueeze(1).broadcast_to((128, 2, 512)),
                      start=(kh == 0), stop=False, perf_mode=DR)
              for gp in range(2):
                  nc.tensor.matmul(
                      op[:], wo2t[gp][:, :, mt * 128:(mt + 1) * 128],
                      ctl[gq][:, 2 * gp:2 * gp + 2, :],
                      start=False, stop=(gp == 1), perf_mode=DR)
              if mt % 2 == 0:
                  ob_cur[0] = obp.tile([128, 2, 512], BF16, tag="ob", name="ob")
              ob = ob_cur[0]
              half = ob[:, mt % 2, :]
              if mt % 2 == 0:
                  nc.vector.tensor_copy(out=half, in_=op[:])
              else:
                  nc.scalar.copy(out=half, in_=op[:])
              if mt % 2 == 1:
                  nc.sync.dma_start(
                      out=outt[mt - 1:mt + 1, :, gq * 512:(gq + 1) * 512]
                      .rearrange("m p s -> p m s"),
                      in_=ob[:])

          def mk_sched(h):
              # schedule of extra PE work per score block of head h
              sched = {i: [] for i in range(NB)}
              if h == 0:
                  sched[2].append(lambda: alloc_qk(1))
                  for idx, (which, ns) in enumerate(
                          (w, n) for n in range(4) for w in ("q", "k")):
                      sched[2 + idx].append(
                          lambda w=which, n=ns: chain_qk(1, w, n))
                  sched[2].append(lambda: xl_dma(2))
                  sched[5].append(lambda: xl_dma(3))
                  for t in range(13):           # v tiles 0-12 at blocks 3-15
                      sched[3 + t].append(lambda t=t: v_tile(t))
                  sched[10].append(lambda: weights_dma(2))
              else:
                  if h == 1:
                      def _mid():
                          for t in (13, 14, 15):
                              v_tile(t)
                          sec0.close()
                      sched[0].append(
                          lambda: sweep2_group(0, 3, split_at=13, mid_fn=_mid))
                  else:
                      sched[0].append(lambda: sweep2_group(h - 1, 3))
                  if h < 3:
                      sched[2].append(lambda: alloc_qk(h + 1))
                      for idx, (which, ns) in enumerate(
                              (w, n) for n in range(4) for w in ("q", "k")):
                          sched[2 + idx].append(
                              lambda w=which, n=ns: chain_qk(h + 1, w, n))
                      if h + 2 < HG:
                          sched[10].append(lambda: weights_dma(h + 2))
                  if h == 2:
                      sched[6].append(open_wop)
                  if h == 3:
                      for j in range(16):       # p3 gq0 at blocks 8-15
                          sched[8 + j // 2].append(
                              lambda mt=j: p3_mt(0, mt, insec=True))
              return sched

          def run_sched(sched, h, i):
              for fn in sched[i]:
                  fn()

          # ---- emit sections ----
          alloc_qk(0)
          weights_dma(1)
          wv_dmas("h")
          xl_dma(0)
          wv_dmas("l")
          xl_dma(1)
          for h in range(HG):
              sched = mk_sched(h)
              scores_head(h, lambda hh, i, sched=sched: run_sched(sched, hh, i))

          # ---- tail: rest of the output projection ----
          for mt in range(16):
              p3_mt(1, mt, tail=True)
          sweep2_group(3, 3)
          for mt in range(16):
              p3_mt(2, mt, tail=True)
          for mt in range(16):
              p3_mt(3, mt, tail=True)
          tailp.close()
          phA.close()
          phB.close()
    nc.finalize()
    return nc


_NC_CACHE = {}


def _get_nc(reps=1):
    if reps not in _NC_CACHE:
        _NC_CACHE[reps] = build_nc(reps)
    return _NC_CACHE[reps]


def _rope_tables(position_ids_b):
    pos = position_ids_b.astype(np.float32)
    inv_freq = (1.0 / (ROPE_THETA ** (np.arange(0, DH, 2, dtype=np.float32) / np.float32(DH))))
    ang = pos[:, None] * inv_freq[None, :]          # [S, 64]
    emb = np.concatenate([ang, ang], axis=-1)       # [S, 128]
    cosT = np.ascontiguousarray(np.cos(emb).T) * np.float32(QSC)   # [128, S]
    sinT = np.sin(emb).T * np.float32(QSC)
    sin_rot = np.concatenate([-sinT[0:64], sinT[64:128]], axis=0)
    return cosT.astype(ml_dtypes.bfloat16), np.ascontiguousarray(sin_rot).astype(ml_dtypes.bfloat16)


def _make_in_maps(inputs):
    hidden_states = np.asarray(inputs["hidden_states"], dtype=np.float32)
    position_ids = np.asarray(inputs["position_ids"])
    Wqkv = np.asarray(inputs["Wqkv"], dtype=np.float32)
    bqkv = np.asarray(inputs["bqkv"], dtype=np.float32)
    Wo = np.asarray(inputs["Wo"], dtype=np.float32)

    mask = np.triu(np.ones((128, 128), dtype=np.float32)).astype(ml_dtypes.bfloat16)
    tones = np.full((128, 1), 1.0 / CSC, dtype=ml_dtypes.bfloat16)
    tabs = [_rope_tables(np.asarray(position_ids)[b]) for b in range(B)]

    def _hilo(M, sc):
        Ms = M * np.float32(sc)
        hi = Ms.astype(ml_dtypes.float8_e4m3)
        lo = (Ms - hi.astype(np.float32)).astype(ml_dtypes.float8_e4m3)
        return hi, lo

    def _pack_pairs(M):
        # [D, C] -> [8, 128, 2, C] with row r = kc2*256 + i*128 + p
        C = M.shape[1]
        return np.ascontiguousarray(M.reshape(8, 2, 128, C).transpose(0, 2, 1, 3))

    def _pack_x(M):
        # [D, S] -> [128, 4, 8, 2, SLAB] partition-major, slab-major free
        return np.ascontiguousarray(
            M.reshape(8, 2, 128, 4, SLAB).transpose(2, 3, 0, 1, 4))

    xts = []
    for b in range(B):
        hi, lo = _hilo(np.ascontiguousarray(hidden_states[b].T), XSC)
        xts.append((_pack_x(hi.astype(np.float32)).astype(ml_dtypes.float8_e4m3),
                    _pack_x(lo.astype(np.float32)).astype(ml_dtypes.float8_e4m3)))

    in_maps = []
    for c in range(NCORES):
        b, hg = divmod(c, HG)
        qcols = slice(hg * GQ, (hg + 1) * GQ)
        kcols = slice(D + hg * GQ, D + (hg + 1) * GQ)
        vcols = slice(2 * D + hg * GQ, 2 * D + (hg + 1) * GQ)
        wqk_c = np.ascontiguousarray(np.concatenate([Wqkv[:, qcols], Wqkv[:, kcols]], axis=1))
        qk_h, qk_l = _hilo(wqk_c, WSC)
        # per-mt packing: [8(mt), 128(p), 8(kc2), 2(i), 128(m)]
        def _pack_mt(M8):
            P = _pack_pairs(M8.astype(np.float32))          # [8, 128, 2, 1024]
            P = P.reshape(8, 128, 2, 8, 128)                 # [kc2, p, i, mt, m]
            return np.ascontiguousarray(P.transpose(3, 1, 0, 2, 4)).astype(ml_dtypes.float8_e4m3)
        wqkh_c = _pack_mt(qk_h)
        wqkl_c = _pack_mt(qk_l)
        wv_c = np.ascontiguousarray(Wqkv[:, vcols])
        v_h, v_l = _hilo(wv_c, WSC)
        wvh_c = _pack_pairs(v_h.astype(np.float32)).astype(ml_dtypes.float8_e4m3)
        wvl_c = _pack_pairs(v_l.astype(np.float32)).astype(ml_dtypes.float8_e4m3)
        # out-proj fp8 packs
        wo_c = np.ascontiguousarray(Wo[hg * GQ:(hg + 1) * GQ, :])   # [512, D]
        wo_h, wo_l = _hilo(wo_c, WSC)
        wo_h = wo_h.astype(np.float32)
        wo_l = wo_l.astype(np.float32)
        wo1_c = np.empty((HG, 128, 2, D), np.float32)
        for kh in range(HG):
            wo1_c[kh, :, 0, :] = wo_h[kh * 128:(kh + 1) * 128, :]
            wo1_c[kh, :, 1, :] = wo_l[kh * 128:(kh + 1) * 128, :]
        wo2_c = np.empty((2, 128, 2, D), np.float32)
        for gp in range(2):
            wo2_c[gp, :, 0, :] = wo_h[gp * 256:gp * 256 + 128, :]
            wo2_c[gp, :, 1, :] = wo_h[gp * 256 + 128:gp * 256 + 256, :]
        bqk_c = np.concatenate([bqkv[qcols], bqkv[kcols]]).reshape(8, 128).T
        bvc_c = np.ascontiguousarray(bqkv[vcols].reshape(HG, 128).T * np.float32(CSC))
        cosT, sin_rot = tabs[b]
        in_maps.append({
            "xh": xts[b][0], "xl": xts[b][1],
            "wqkh": wqkh_c, "wqkl": wqkl_c, "wvh": wvh_c, "wvl": wvl_c,
            "wo1": wo1_c.astype(ml_dtypes.float8_e4m3),
            "wo2": wo2_c.astype(ml_dtypes.float8_e4m3),
            "bqkt": np.ascontiguousarray(bqk_c),
            "bvc": bvc_c,
            "cost": cosT, "sinrt": sin_rot, "maskd": mask,
            "tonesd": tones,
        })
    return in_maps


def kernel(hidden_states, position_ids, Wqkv, bqkv, Wo, bo, _reps=1):
    bo = np.asarray(bo, dtype=np.float32)
    in_maps = _make_in_maps({
        "hidden_states": hidden_states, "position_ids": position_ids,
        "Wqkv": Wqkv, "bqkv": bqkv, "Wo": Wo, "bo": bo,
    })
    nc = _get_nc(_reps)
    res = run_bass_kernel_spmd(nc, in_maps, core_ids=list(range(NCORES)))

    out = np.empty((B, S, D), dtype=np.float32)
    for b in range(B):
        acc = res.results[b * HG]["outt"].reshape(D, S).astype(np.float32).copy()
        for hg in range(1, HG):
            acc += res.results[b * HG + hg]["outt"].reshape(D, S).astype(np.float32)
        out[b] = acc.T * np.float32(P3_SCALE) + bo[None, :]
    return out


# revision 49
# speedup vs baseline: 1.3243x; 1.0003x over previous
"""Trainium2 Bass kernel for CustomRoPEAttention (B=2, S=2048, H=16, Dh=128).

Sharding: 8 cores = 2 batches x 4 head-groups (4 heads/core).

Head-pipelined structure: per head h, QKV^T projection (fp8 hi/lo DoubleRow
matmuls) + RoPE, then transposed-layout causal attention for that head while
later heads' projections stream -- this overlaps the ACT-engine exp work with
PE-engine matmul work across the whole kernel instead of serializing phases.

fp8 DoubleRow "dup trick": scores use stationary (k_hi,k_lo) pairs against a
broadcast (step-0) fp8 q moving operand, and the output projection uses
(wo_hi,wo_lo) pairs against broadcast ct_hi plus a wo_hi x ct_lo correction --
half / 0.75x the bf16 PE time at first-order-exact precision.

Host sums the 4 partial (transposed) output projections per batch.

Self-contained: hardcodes shapes from the problem spec.
"""
import math
from contextlib import ExitStack

import numpy as np
import ml_dtypes

import concourse.mybir as mybir
import concourse.tile as tile
from concourse import bacc
from concourse.bass_utils import run_bass_kernel_spmd
from concourse.masks import make_identity

S = 2048            # sequence
D = 2048            # hidden
NH = 16             # total heads
DH = 128            # head dim
HG = 4              # heads per core
GQ = HG * DH        # 512: per-core q/k/v feature width
B = 2
NCORES = 8
ROPE_THETA = 10000.0
SCALE = 1.0 / math.sqrt(DH)
SLAB = 512          # qkv sequence slab width
XSC = 16.0          # fp8 pre-scale for x
WSC = 512.0         # fp8 pre-scale for Wqkv / Wo
QSC = 16.0          # fp8 pre-scale for roped q/k (folded into cos/sin tables)
CSC = 16.0          # fp8 pre-scale for attention-out ct (folded into tones)
INV_SC = 1.0 / (XSC * WSC)
EXP_SCALE = SCALE / (QSC * QSC)
P3_SCALE = 1.0 / (WSC * CSC)   # applied host-side
F32 = mybir.dt.float32
BF16 = mybir.dt.bfloat16
F8 = mybir.dt.float8e4
MULT = mybir.AluOpType.mult
ADD = mybir.AluOpType.add
SUB = mybir.AluOpType.subtract
DR = mybir.MatmulPerfMode.DoubleRow
NB = S // 128       # 16 k/q blocks
IDENT = mybir.ActivationFunctionType.Identity
EXPF = mybir.ActivationFunctionType.Exp


def build_nc(reps=1, knobs=None):
    kn = {"sps": 3, "mmp": 4, "expb": 1, "wqb": 2, "qkb": 2, "stg": 2, "obp": 6}
    if knobs:
        kn.update(knobs)
    nc = bacc.Bacc(None, target_bir_lowering=False)
    # x^T hi/lo, slab-major pack: [p, ns, kc2, i, s]
    xh = nc.dram_tensor("xh", [128, 4, 8, 2, SLAB], F8, kind="ExternalInput")
    xl = nc.dram_tensor("xl", [128, 4, 8, 2, SLAB], F8, kind="ExternalInput")
    # per-mt packed qk weights: [mt, p, kc2, i, m]
    wqkh = nc.dram_tensor("wqkh", [8, 128, 8, 2, 128], F8, kind="ExternalInput")
    wvh = nc.dram_tensor("wvh", [8, 128, 2, GQ], F8, kind="ExternalInput")
    wvl = nc.dram_tensor("wvl", [8, 128, 2, GQ], F8, kind="ExternalInput")
    # out-proj fp8 packs: wo1[kh] = (hi,lo) pairs; wo2[g] = hi head-pair packs
    wo1 = nc.dram_tensor("wo1", [HG, 128, 2, D], F8, kind="ExternalInput")
    wo2 = nc.dram_tensor("wo2", [2, 128, 2, D], F8, kind="ExternalInput")
    bqkt = nc.dram_tensor("bqkt", [128, 8], F32, kind="ExternalInput")
    bvc = nc.dram_tensor("bvc", [128, HG], F32, kind="ExternalInput")  # v bias * CSC
    cost = nc.dram_tensor("cost", [128, S], BF16, kind="ExternalInput")    # cos^T * QSC
    sinrt = nc.dram_tensor("sinrt", [128, S], BF16, kind="ExternalInput")  # sin^T * QSC, rot sign
    maskd = nc.dram_tensor("maskd", [128, 128], BF16, kind="ExternalInput")  # triu 0/1 keep-mask
    tonesd = nc.dram_tensor("tonesd", [128, 1], BF16, kind="ExternalInput")  # 1/CSC
    outt = nc.dram_tensor("outt", [16, 128, S], BF16, kind="ExternalOutput")
    lrt = nc.dram_tensor("lrt", [HG, 1, 16, 128], F32)  # recip bounce: [16,128] -> [1,2048]

    with tile.TileContext(nc) as tc, ExitStack() as top:
        g = top.enter_context(tc.tile_pool(name="glob", bufs=1))
        tcos = g.tile([128, S], BF16)
        tsin = g.tile([128, S], BF16)
        tmask = g.tile([128, 128], BF16)
        ident_f = g.tile([128, 128], F32)
        make_identity(nc, ident_f[:])
        tbqkt = g.tile([128, 8], F32)
        tbvc = g.tile([128, HG], F32)
        tones = g.tile([128, 1], BF16)
        tinv = g.tile([128, 1], F32)
        nc.vector.memset(tinv[:], INV_SC)

        # Whole-kernel residents
        res = top.enter_context(tc.tile_pool(name="res", bufs=1))
        vres = []  # 16 V k-block tiles [128(seq), GQ] bf16
        for t in range(NB):
            vres.append(res.tile([128, GQ], BF16, tag=f"v{t}", name=f"v{t}"))
        cth = {}
        for h in range(HG):
            for gq in range(4):
                cth[(h, gq)] = res.tile([128, 512], F8, tag=f"cth_{h}_{gq}",
                                        name=f"cth_{h}_{gq}")
        ctl = [res.tile([128, HG, 512], F8, tag=f"ctl{gq}", name=f"ctl{gq}")
               for gq in range(4)]

        for _rep in range(reps):
          phB = ExitStack()
          mmp = phB.enter_context(tc.tile_pool(name="mmp", bufs=kn["mmp"], space="PSUM"))
          sps = phB.enter_context(tc.tile_pool(name="sps", bufs=kn["sps"], space="PSUM"))
          smps = phB.enter_context(tc.tile_pool(name="smps", bufs=1, space="PSUM"))
          expp = phB.enter_context(tc.tile_pool(name="expp", bufs=kn["expb"]))
          lrp = phB.enter_context(tc.tile_pool(name="lrp", bufs=2))
          rbp = phB.enter_context(tc.tile_pool(name="rbp", bufs=2))
          ctsp = phB.enter_context(tc.tile_pool(name="ctsp", bufs=2))
          obp = phB.enter_context(tc.tile_pool(name="obp", bufs=kn["obp"]))
          smt = smps.tile([128, 132], F32, tag="sm", name="smt")

          phA = ExitStack()
          wqp = phA.enter_context(tc.tile_pool(name="wqp", bufs=kn["wqb"]))
          qkp = phA.enter_context(tc.tile_pool(name="qkp", bufs=kn["qkb"]))
          stg = phA.enter_context(tc.tile_pool(name="stg", bufs=kn["stg"]))
          xps = ExitStack()
          xp = xps.enter_context(tc.tile_pool(name="xp", bufs=1))
          sec0 = ExitStack()
          xsl = sec0.enter_context(tc.tile_pool(name="xsl", bufs=2))
          wvp = sec0.enter_context(tc.tile_pool(name="wvp", bufs=1))

          # ---- initial DMA order (startup-critical) ----
          wq_tiles = {}  # (h) -> (wqh, wql, wkh, wkl)

          def weights_dma(h):
              tl = []
              for mt, tag in ((h, "wqh"), (4 + h, "wkh")):
                  wt = wqp.tile([128, 8, 2, 128], F8, tag=tag, name=f"{tag}{h}")
                  nc.sync.dma_start(out=wt, in_=wqkh[mt])
                  tl.append(wt)
              wq_tiles[h] = tl

          # head-0 weights + first x slab first
          wt = wqp.tile([128, 8, 2, 128], F8, tag="wqh", name="wqh0")
          nc.sync.dma_start(out=wt[:, 0:2], in_=wqkh[0, :, 0:2])
          xres = [xp.tile([128, 8, 2, SLAB], F8, tag=f"x{ns}", name=f"x{ns}")
                  for ns in range(4)]
          nc.sync.dma_start(out=xres[0][:, 0:2], in_=xh[:, 0, 0:2])
          nc.sync.dma_start(out=wt[:, 2:8], in_=wqkh[0, :, 2:8])
          nc.sync.dma_start(out=xres[0][:, 2:5], in_=xh[:, 0, 2:5])
          wt2 = wqp.tile([128, 8, 2, 128], F8, tag="wkh", name="wkh0")
          nc.sync.dma_start(out=wt2, in_=wqkh[4])
          wq_tiles[0] = [wt, wt2]
          nc.sync.dma_start(out=xres[0][:, 5:8], in_=xh[:, 0, 5:8])
          # small consts needed by first psum copies / rope
          nc.sync.dma_start(out=tbqkt, in_=bqkt[:])
          nc.sync.dma_start(out=xres[1][:, 0:4], in_=xh[:, 1, 0:4])
          nc.sync.dma_start(out=tcos, in_=cost[:])
          nc.sync.dma_start(out=xres[1][:, 4:8], in_=xh[:, 1, 4:8])
          nc.sync.dma_start(out=tsin, in_=sinrt[:])
          nc.sync.dma_start(out=xres[2][:, 0:4], in_=xh[:, 2, 0:4])
          nc.sync.dma_start(out=xres[2][:, 4:8], in_=xh[:, 2, 4:8])
          nc.sync.dma_start(out=tones, in_=tonesd[:])
          nc.sync.dma_start(out=xres[3][:, 0:4], in_=xh[:, 3, 0:4])
          nc.sync.dma_start(out=tmask, in_=maskd[:])
          nc.sync.dma_start(out=xres[3][:, 4:8], in_=xh[:, 3, 4:8])
          nc.sync.dma_start(out=tbvc, in_=bvc[:])
          twvh, twvl = [], []
          xlres = {}

          def wv_dmas(which):
              if which == "h":
                  wvht = wvp.tile([128, 8, 2, GQ], F8, tag="wvh", name="wvht")
                  nc.sync.dma_start(out=wvht, in_=wvh.rearrange("a p b c -> p a b c"))
                  twvh.append(wvht)
              else:
                  wvlt = wvp.tile([128, 8, 2, GQ], F8, tag="wvl", name="wvlt")
                  nc.sync.dma_start(out=wvlt, in_=wvl.rearrange("a p b c -> p a b c"))
                  twvl.append(wvlt)

          def xl_dma(ns):
              xt = xsl.tile([128, 8, 2, SLAB], F8, tag="xl", name=f"xl{ns}")
              nc.sync.dma_start(out=xt, in_=xl[:, ns])
              xlres[ns] = xt

          # ---- per-head state ----
          qf8 = {}
          kpair = {}
          expT = {}
          lrec_cur = {}
          recrow = {}

          def chain_qk(h, which, ns):
              wqh_, wkh_ = wq_tiles[h]
              whi = wqh_ if which == "q" else wkh_
              sl = slice(ns * SLAB, (ns + 1) * SLAB)
              ps = mmp.tile([128, SLAB], F32, tag="mm")
              for kc2 in range(8):
                  nc.tensor.matmul(ps[:], whi[:, kc2, :, :], xres[ns][:, kc2, :, :],
                                   start=(kc2 == 0), stop=(kc2 == 7), perf_mode=DR)
              st = stg.tile([128, SLAB], BF16, tag="st")
              mt = h if which == "q" else 4 + h
              nc.vector.tensor_scalar(out=st[:], in0=ps[:], scalar1=INV_SC,
                                      scalar2=tbqkt[:, mt:mt + 1],
                                      op0=MULT, op1=ADD)
              # RoPE: out = st*cos + swap(st)*sin_rot   (tables pre-scaled by QSC)
              sw = stg.tile([128, SLAB], BF16, tag="sw")
              nc.vector.tensor_copy(out=sw[0:64, :], in_=st[64:128, :])
              nc.vector.tensor_copy(out=sw[64:128, :], in_=st[0:64, :])
              m1 = stg.tile([128, SLAB], BF16, tag="m1")
              nc.vector.tensor_tensor(out=m1[:], in0=st[:], in1=tcos[:, sl], op=MULT)
              nc.vector.tensor_tensor(out=sw[:], in0=sw[:], in1=tsin[:, sl], op=MULT)
              if which == "q":
                  nc.vector.tensor_tensor(out=qf8[h][:, sl], in0=m1[:], in1=sw[:], op=ADD)
              else:
                  kb = m1
                  nc.vector.tensor_tensor(out=kb[:], in0=m1[:], in1=sw[:], op=ADD)
                  if h == 0:
                      nc.scalar.copy(out=kpair[h][:, 0, sl], in_=kb[:])
                      nc.vector.tensor_tensor(out=kpair[h][:, 1, sl], in0=kb[:],
                                              in1=kpair[h][:, 0, sl], op=SUB)
                  else:
                      nc.gpsimd.tensor_scalar(out=kpair[h][:, 0, sl], in0=kb[:],
                                              scalar1=0.0, scalar2=None, op0=ADD)
                      nc.gpsimd.tensor_tensor(out=kpair[h][:, 1, sl], in0=kb[:],
                                              in1=kpair[h][:, 0, sl], op=SUB)

          def v_tile(t):
              ns, sti = divmod(t, 4)
              s0 = sti * 128
              pv = mmp.tile([128, GQ], F32, tag="mm")
              passes = [(xres[ns], twvh[0]), (xlres[ns], twvh[0]), (xres[ns], twvl[0])]
              for pi, (xt_, wv_) in enumerate(passes):
                  for kc2 in range(8):
                      nc.tensor.matmul(pv[:], xt_[:, kc2, :, s0:s0 + 128],
                                       wv_[:, kc2, :, :], start=(pi == 0 and kc2 == 0),
                                       stop=(pi == 2 and kc2 == 7), perf_mode=DR)
              nc.scalar.activation(out=vres[t], in_=pv[:], func=IDENT,
                                   scale=INV_SC)

          def rec_group(h, gq):
              # ship recip(ell) for q-blocks 4g..4g+3 to DRAM and back as a row
              rt = smt[0:4, 0:128]
              nc.tensor.transpose(rt, lrec_cur[h][:, 4 * gq:4 * gq + 4], ident_f[:])
              rts = lrp.tile([4, 128], F32, tag="rts")
              nc.vector.tensor_copy(out=rts[:], in_=rt)
              nc.sync.dma_start(out=lrt[h, 0, 4 * gq:4 * gq + 4, :], in_=rts[:])
              nc.sync.dma_start(out=recrow[h][:, 4 * gq:4 * gq + 4, :],
                                in_=lrt[h, :, 4 * gq:4 * gq + 4, :])

          def denom(h, b):
              # ell[q] for q-block b: sum_k exp tiles via ap-1 matmuls, then recip
              lp = smt[:, 128 + (b % 4):129 + (b % 4)]
              for j in range(b + 1):
                  nc.tensor.matmul(lp, expT[h][j][:, (b - j) * 128:(b - j + 1) * 128],
                                   tones[:], start=(j == 0), stop=(j == b))
              nc.vector.reciprocal(out=lrec_cur[h][:, b:b + 1], in_=lp)

          def sweep2_group(h, gq, split_at=None, mid_fn=None):
              # ct = (sum_k V^T[k] expS^T[k]) * recip -> split into fp8 hi/lo
              rbs = rbp.tile([128, 512], F32, tag="rbs")
              nc.gpsimd.partition_broadcast(
                  rbs[:], recrow[h][:, 4 * gq:4 * gq + 4, :])
              ct = mmp.tile([128, 512], F32, tag="mm")
              last = 4 * gq + 3
              for j in range(last + 1):
                  if split_at is not None and j == split_at:
                      mid_fn()
                  if j <= 4 * gq:
                      nc.tensor.matmul(ct[:], vres[j][:, h * 128:(h + 1) * 128],
                                       expT[h][j][:, (4 * gq - j) * 128:(4 * gq - j) * 128 + 512],
                                       start=(j == 0), stop=(j == last))
                  else:
                      w = (4 * gq + 4 - j) * 128
                      nc.tensor.matmul(ct[:, 512 - w:512], vres[j][:, h * 128:(h + 1) * 128],
                                       expT[h][j][:, 0:w], start=False, stop=(j == last))
              ctb = ctsp.tile([128, 512], BF16, tag="ctb")
              nc.vector.tensor_tensor(out=ctb[:], in0=ct[:], in1=rbs[:], op=MULT)
              nc.gpsimd.tensor_scalar(out=cth[(h, gq)][:], in0=ctb[:],
                                      scalar1=tbvc[:, h:h + 1], scalar2=None,
                                      op0=ADD)
              nc.vector.scalar_tensor_tensor(
                  out=ctl[gq][:, h, :], in0=ctb[:], scalar=tbvc[:, h:h + 1],
                  in1=cth[(h, gq)][:], op0=ADD, op1=SUB)

          def scores_head(h, interleave):
              expT[h] = []
              lrec_cur[h] = lrp.tile([128, 16], F32, tag="lrec", name="lrec", bufs=1)
              recrow[h] = lrp.tile([1, 16, 128], F32, tag="recrow", name="recrow", bufs=1)
              kp = kpair[h]
              qf = qf8[h]
              for i in range(NB):
                  w = (NB - i) * 128
                  ex = expp.tile([128, w], BF16, tag=f"expT{i}", name=f"expT{i}",
                                 bufs=2 if i < 4 else 1)
                  expT[h].append(ex)
              # chunk list; head 0 emits in slab-availability wavefront order
              chunks = []
              for i in range(NB):
                  w = (NB - i) * 128
                  for c0 in range(0, w, 512):
                      cw = min(512, w - c0)
                      p = max((i * 128 + c0 + cw - 1) // 512, i // 4)
                      chunks.append((p, i, c0, cw))
              if h == 0:
                  chunks.sort()
              nch = len(chunks)
              per_block = [0] * NB
              for _p, i_, _c, _w in chunks:
                  per_block[i_] += 1
              done = [0] * NB
              dfired = 0
              mi = 0
              if h == 0:
                  milestones = [((i + 1) * nch + NB - 1) // NB for i in range(NB)]
              else:
                  acc = 0
                  milestones = []
                  for i in range(NB):
                      acc += per_block[i]
                      milestones.append(acc)
              emitted_p = [-1]

              def on_block_complete(b):
                  # fire denoms (2-block lag), recips, and own sweeps
                  nonlocal dfired
                  while dfired <= b - 2:
                      d = dfired
                      denom(h, d)
                      dfired += 1
                      if d >= 3 and d % 4 == 3:
                          rec_group(h, d // 4)
                      if d >= 5 and (d - 5) % 4 == 0 and (d - 5) // 4 <= 2:
                          sweep2_group(h, (d - 5) // 4)

              for n, (p, i, c0, cw) in enumerate(chunks, 1):
                  if h == 0 and p > emitted_p[0]:
                      for ns_ in range(emitted_p[0] + 1, p + 1):
                          chain_qk(0, "q", ns_)
                          chain_qk(0, "k", ns_)
                      emitted_p[0] = p
                  ex = expT[h][i]
                  sp = sps.tile([128, 512], F32, tag="sp")
                  q0 = i * 128 + c0
                  nc.tensor.matmul(
                      sp[:, 0:cw], kp[:, :, i * 128:(i + 1) * 128],
                      qf[:, q0:q0 + cw].unsqueeze(1).broadcast_to((128, 2, cw)),
                      start=True, stop=True, perf_mode=DR)
                  nc.scalar.activation(out=ex[:, c0:c0 + cw], in_=sp[:, 0:cw],
                                       func=EXPF, scale=EXP_SCALE)
                  if c0 == 0:
                      nc.gpsimd.tensor_tensor(out=ex[:, 0:128], in0=ex[:, 0:128],
                                              in1=tmask[:], op=MULT)
                  done[i] += 1
                  if done[i] == per_block[i]:
                      on_block_complete(i)
                  while mi < NB and n >= milestones[mi]:
                      interleave(h, mi)
                      mi += 1
              while dfired < NB:
                  d = dfired
                  denom(h, d)
                  dfired += 1
                  if d >= 3 and d % 4 == 3:
                      rec_group(h, d // 4)
                  if d >= 5 and (d - 5) % 4 == 0 and (d - 5) // 4 <= 2:
                      sweep2_group(h, (d - 5) // 4)

          def alloc_qk(h):
              qf8[h] = qkp.tile([128, S], F8, tag="qf8", name=f"qf8_{h}")
              kpair[h] = qkp.tile([128, 2, S], F8, tag="kpair", name=f"kpair_{h}")

          wo1t = []
          wo2t = []
          tailp = ExitStack()

          def open_wop():
              xps.close()
              wop = tailp.enter_context(tc.tile_pool(name="wop", bufs=1))
              for kh in range(HG):
                  wt_ = wop.tile([128, 2, D], F8, tag=f"wo1_{kh}", name=f"wo1_{kh}")
                  nc.sync.dma_start(out=wt_, in_=wo1[kh])
                  wo1t.append(wt_)
              for gp in range(2):
                  wt_ = wop.tile([128, 2, D], F8, tag=f"wo2_{gp}", name=f"wo2_{gp}")
                  nc.sync.dma_start(out=wt_, in_=wo2[gp])
                  wo2t.append(wt_)

          ob_cur = [None]

          def p3_mt(gq, mt, tail=False, insec=False):
              if tail and mt % 2 == 0:
                  op = sps.tile([128, 512], F32, tag="sp", name="op")
              else:
                  op = mmp.tile([128, 512], F32, tag="mm")
              for kh in range(HG):
                  nc.tensor.matmul(
                      op[:], wo1t[kh][:, :, mt * 128:(mt + 1) * 128],
                      cth[(kh, gq)][:].unsqueeze(1).broadcast_to((128, 2, 512)),
                      start=(kh == 0), stop=False, perf_mode=DR)
              for gp in range(2):
                  nc.tensor.matmul(
                      op[:], wo2t[gp][:, :, mt * 128:(mt + 1) * 128],
                      ctl[gq][:, 2 * gp:2 * gp + 2, :],
                      start=False, stop=(gp == 1), perf_mode=DR)
              if mt % 2 == 0:
                  ob_cur[0] = obp.tile([128, 2, 512], BF16, tag="ob", name="ob")
              ob = ob_cur[0]
              half = ob[:, mt % 2, :]
              if mt % 2 == 0:
                  nc.vector.tensor_copy(out=half, in_=op[:])
              else:
                  nc.scalar.copy(out=half, in_=op[:])
              if mt % 2 == 1:
                  nc.sync.dma_start(
                      out=outt[mt - 1:mt + 1, :, gq * 512:(gq + 1) * 512]
                      .rearrange("m p s -> p m s"),
                      in_=ob[:])

          def mk_sched(h):
              # schedule of extra PE work per score block of head h
              sched = {i: [] for i in range(NB)}
              if h == 0:
                  sched[2].append(lambda: alloc_qk(1))
                  for idx, (which, ns) in enumerate(
                          (w, n) for n in range(4) for w in ("q", "k")):
                      sched[2 + idx].append(
                          lambda w=which, n=ns: chain_qk(1, w, n))
                  sched[2].append(lambda: xl_dma(2))
                  sched[5].append(lambda: xl_dma(3))
                  for t in range(13):           # v tiles 0-12 at blocks 3-15
                      sched[3 + t].append(lambda t=t: v_tile(t))
                  sched[10].append(lambda: weights_dma(2))
              else:
                  if h == 1:
                      def _mid():
                          for t in (13, 14, 15):
                              v_tile(t)
                          sec0.close()
                      sched[0].append(
                          lambda: sweep2_group(0, 3, split_at=13, mid_fn=_mid))
                  else:
                      sched[0].append(lambda: sweep2_group(h - 1, 3))
                  if h < 3:
                      sched[2].append(lambda: alloc_qk(h + 1))
                      for idx, (which, ns) in enumerate(
                              (w, n) for n in range(4) for w in ("q", "k")):
                          sched[2 + idx].append(
                              lambda w=which, n=ns: chain_qk(h + 1, w, n))
                      if h + 2 < HG:
                          sched[10].append(lambda: weights_dma(h + 2))
                  if h == 2:
                      sched[6].append(open_wop)
                  if h == 3:
                      for j in range(16):       # p3 gq0 at blocks 8-15
                          sched[8 + j // 2].append(
                              lambda mt=j: p3_mt(0, mt, insec=True))
              return sched

          def run_sched(sched, h, i):
              for fn in sched[i]:
                  fn()

          # ---- emit sections ----
          alloc_qk(0)
          weights_dma(1)
          wv_dmas("h")
          xl_dma(0)
          wv_dmas("l")
          xl_dma(1)
          for h in range(HG):
              sched = mk_sched(h)
              scores_head(h, lambda hh, i, sched=sched: run_sched(sched, hh, i))

          # ---- tail: rest of the output projection ----
          for mt in range(16):
              p3_mt(1, mt, tail=True)
          sweep2_group(3, 3)
          for mt in range(16):
              p3_mt(2, mt, tail=True)
          for mt in range(16):
              p3_mt(3, mt, tail=True)
          tailp.close()
          phA.close()
          phB.close()
    nc.finalize()
    return nc


_NC_CACHE = {}


def _get_nc(reps=1):
    if reps not in _NC_CACHE:
        _NC_CACHE[reps] = build_nc(reps)
    return _NC_CACHE[reps]


def _rope_tables(position_ids_b):
    pos = position_ids_b.astype(np.float32)
    inv_freq = (1.0 / (ROPE_THETA ** (np.arange(0, DH, 2, dtype=np.float32) / np.float32(DH))))
    ang = pos[:, None] * inv_freq[None, :]          # [S, 64]
    emb = np.concatenate([ang, ang], axis=-1)       # [S, 128]
    cosT = np.ascontiguousarray(np.cos(emb).T) * np.float32(QSC)   # [128, S]
    sinT = np.sin(emb).T * np.float32(QSC)
    sin_rot = np.concatenate([-sinT[0:64], sinT[64:128]], axis=0)
    return cosT.astype(ml_dtypes.bfloat16), np.ascontiguousarray(sin_rot).astype(ml_dtypes.bfloat16)


def _make_in_maps(inputs):
    hidden_states = np.asarray(inputs["hidden_states"], dtype=np.float32)
    position_ids = np.asarray(inputs["position_ids"])
    Wqkv = np.asarray(inputs["Wqkv"], dtype=np.float32)
    bqkv = np.asarray(inputs["bqkv"], dtype=np.float32)
    Wo = np.asarray(inputs["Wo"], dtype=np.float32)

    mask = np.triu(np.ones((128, 128), dtype=np.float32)).astype(ml_dtypes.bfloat16)
    tones = np.full((128, 1), 1.0 / CSC, dtype=ml_dtypes.bfloat16)
    tabs = [_rope_tables(np.asarray(position_ids)[b]) for b in range(B)]

    def _hilo(M, sc):
        Ms = M * np.float32(sc)
        hi = Ms.astype(ml_dtypes.float8_e4m3)
        lo = (Ms - hi.astype(np.float32)).astype(ml_dtypes.float8_e4m3)
        return hi, lo

    def _pack_pairs(M):
        # [D, C] -> [8, 128, 2, C] with row r = kc2*256 + i*128 + p
        C = M.shape[1]
        return np.ascontiguousarray(M.reshape(8, 2, 128, C).transpose(0, 2, 1, 3))

    def _pack_x(M):
        # [D, S] -> [128, 4, 8, 2, SLAB] partition-major, slab-major free
        return np.ascontiguousarray(
            M.reshape(8, 2, 128, 4, SLAB).transpose(2, 3, 0, 1, 4))

    xts = []
    for b in range(B):
        hi, lo = _hilo(np.ascontiguousarray(hidden_states[b].T), XSC)
        xts.append((_pack_x(hi.astype(np.float32)).astype(ml_dtypes.float8_e4m3),
                    _pack_x(lo.astype(np.float32)).astype(ml_dtypes.float8_e4m3)))

    in_maps = []
    for c in range(NCORES):
        b, hg = divmod(c, HG)
        qcols = slice(hg * GQ, (hg + 1) * GQ)
        kcols = slice(D + hg * GQ, D + (hg + 1) * GQ)
        vcols = slice(2 * D + hg * GQ, 2 * D + (hg + 1) * GQ)
        wqk_c = np.ascontiguousarray(np.concatenate([Wqkv[:, qcols], Wqkv[:, kcols]], axis=1))
        qk_h, qk_l = _hilo(wqk_c, WSC)
        # per-mt packing: [8(mt), 128(p), 8(kc2), 2(i), 128(m)]
        def _pack_mt(M8):
            P = _pack_pairs(M8.astype(np.float32))          # [8, 128, 2, 1024]
            P = P.reshape(8, 128, 2, 8, 128)                 # [kc2, p, i, mt, m]
            return np.ascontiguousarray(P.transpose(3, 1, 0, 2, 4)).astype(ml_dtypes.float8_e4m3)
        wqkh_c = _pack_mt(qk_h)
        wqkl_c = _pack_mt(qk_l)
        wv_c = np.ascontiguousarray(Wqkv[:, vcols])
        v_h, v_l = _hilo(wv_c, WSC)
        wvh_c = _pack_pairs(v_h.astype(np.float32)).astype(ml_dtypes.float8_e4m3)
        wvl_c = _pack_pairs(v_l.astype(np.float32)).astype(ml_dtypes.float8_e4m3)
        # out-proj fp8 packs
        wo_c = np.ascontiguousarray(Wo[hg * GQ:(hg + 1) * GQ, :])   # [512, D]
        wo_h, wo_l = _hilo(wo_c, WSC)
        wo_h = wo_h.astype(np.float32)
        wo_l = wo_l.astype(np.float32)
        wo1_c = np.empty((HG, 128, 2, D), np.float32)
        for kh in range(HG):
            wo1_c[kh, :, 0, :] = wo_h[kh * 128:(kh + 1) * 128, :]
            wo1_c[kh, :, 1, :] = wo_l[kh * 128:(kh + 1) * 128, :]
        wo2_c = np.empty((2, 128, 2, D), np.float32)
        for gp in range(2):
            wo2_c[gp, :, 0, :] = wo_h[gp * 256:gp * 256 + 128, :]
            wo2_c[gp, :, 1, :] = wo_h[gp * 256 + 128:gp * 256 + 256, :]
        bqk_c = np.concatenate([bqkv[qcols], bqkv[kcols]]).reshape(8, 128).T
        bvc_c = np.ascontiguousarray(bqkv[vcols].reshape(HG, 128).T * np.float32(CSC))
        cosT, sin_rot = tabs[b]
        in_maps.append({
            "xh": xts[b][0], "xl": xts[b][1],
            "wqkh": wqkh_c, "wqkl": wqkl_c, "wvh": wvh_c, "wvl": wvl_c,
            "wo1": wo1_c.astype(ml_dtypes.float8_e4m3),
            "wo2": wo2_c.astype(ml_dtypes.float8_e4m3),
            "bqkt": np.ascontiguousarray(bqk_c),
            "bvc": bvc_c,
            "cost": cosT, "sinrt": sin_rot, "maskd": mask,
            "tonesd": tones,
        })
    return in_maps


def kernel(hidden_states, position_ids, Wqkv, bqkv, Wo, bo, _reps=1):
    bo = np.asarray(bo, dtype=np.float32)
    in_maps = _make_in_maps({
        "hidden_states": hidden_states, "position_ids": position_ids,
        "Wqkv": Wqkv, "bqkv": bqkv, "Wo": Wo, "bo": bo,
    })
    nc = _get_nc(_reps)
    res = run_bass_kernel_spmd(nc, in_maps, core_ids=list(range(NCORES)))

    out = np.empty((B, S, D), dtype=np.float32)
    for b in range(B):
        acc = res.results[b * HG]["outt"].reshape(D, S).astype(np.float32).copy()
        for hg in range(1, HG):
            acc += res.results[b * HG + hg]["outt"].reshape(D, S).astype(np.float32)
        out[b] = acc.T * np.float32(P3_SCALE) + bo[None, :]
    return out


# revision 52
# speedup vs baseline: 1.3251x; 1.0006x over previous
"""Trainium2 Bass kernel for CustomRoPEAttention (B=2, S=2048, H=16, Dh=128).

Sharding: 8 cores = 2 batches x 4 head-groups (4 heads/core).

Head-pipelined structure: per head h, QKV^T projection (fp8 hi/lo DoubleRow
matmuls) + RoPE, then transposed-layout causal attention for that head while
later heads' projections stream -- this overlaps the ACT-engine exp work with
PE-engine matmul work across the whole kernel instead of serializing phases.

fp8 DoubleRow "dup trick": scores use stationary (k_hi,k_lo) pairs against a
broadcast (step-0) fp8 q moving operand, and the output projection uses
(wo_hi,wo_lo) pairs against broadcast ct_hi plus a wo_hi x ct_lo correction --
half / 0.75x the bf16 PE time at first-order-exact precision.

Host sums the 4 partial (transposed) output projections per batch.

Self-contained: hardcodes shapes from the problem spec.
"""
import math
from contextlib import ExitStack

import numpy as np
import ml_dtypes

import concourse.mybir as mybir
import concourse.tile as tile
from concourse import bacc
from concourse.bass_utils import run_bass_kernel_spmd
from concourse.masks import make_identity

S = 2048            # sequence
D = 2048            # hidden
NH = 16             # total heads
DH = 128            # head dim
HG = 4              # heads per core
GQ = HG * DH        # 512: per-core q/k/v feature width
B = 2
NCORES = 8
ROPE_THETA = 10000.0
SCALE = 1.0 / math.sqrt(DH)
SLAB = 512          # qkv sequence slab width
XSC = 16.0          # fp8 pre-scale for x
WSC = 512.0         # fp8 pre-scale for Wqkv / Wo
QSC = 16.0          # fp8 pre-scale for roped q/k (folded into cos/sin tables)
CSC = 16.0          # fp8 pre-scale for attention-out ct (folded into tones)
INV_SC = 1.0 / (XSC * WSC)
EXP_SCALE = SCALE / (QSC * QSC)
P3_SCALE = 1.0 / (WSC * CSC)   # applied host-side
F32 = mybir.dt.float32
BF16 = mybir.dt.bfloat16
F8 = mybir.dt.float8e4
MULT = mybir.AluOpType.mult
ADD = mybir.AluOpType.add
SUB = mybir.AluOpType.subtract
DR = mybir.MatmulPerfMode.DoubleRow
NB = S // 128       # 16 k/q blocks
IDENT = mybir.ActivationFunctionType.Identity
EXPF = mybir.ActivationFunctionType.Exp


def build_nc(reps=1, knobs=None):
    kn = {"sps": 3, "mmp": 4, "expb": 1, "wqb": 2, "qkb": 2, "stg": 2, "obp": 6}
    if knobs:
        kn.update(knobs)
    nc = bacc.Bacc(None, target_bir_lowering=False)
    # x^T hi/lo, slab-major pack: [p, ns, kc2, i, s]
    xh = nc.dram_tensor("xh", [128, 4, 8, 2, SLAB], F8, kind="ExternalInput")
    xl = nc.dram_tensor("xl", [128, 4, 8, 2, SLAB], F8, kind="ExternalInput")
    # per-mt packed qk weights: [mt, p, kc2, i, m]
    wqkh = nc.dram_tensor("wqkh", [8, 128, 8, 2, 128], F8, kind="ExternalInput")
    wvh = nc.dram_tensor("wvh", [8, 128, 2, GQ], F8, kind="ExternalInput")
    wvl = nc.dram_tensor("wvl", [8, 128, 2, GQ], F8, kind="ExternalInput")
    # out-proj fp8 packs: wo1[kh] = (hi,lo) pairs; wo2[g] = hi head-pair packs
    wo1 = nc.dram_tensor("wo1", [HG, 128, 2, D], F8, kind="ExternalInput")
    wo2 = nc.dram_tensor("wo2", [2, 128, 2, D], F8, kind="ExternalInput")
    bqkt = nc.dram_tensor("bqkt", [128, 8], F32, kind="ExternalInput")
    bvc = nc.dram_tensor("bvc", [128, HG], F32, kind="ExternalInput")  # v bias * CSC
    cost = nc.dram_tensor("cost", [128, S], BF16, kind="ExternalInput")    # cos^T * QSC
    sinrt = nc.dram_tensor("sinrt", [128, S], BF16, kind="ExternalInput")  # sin^T * QSC, rot sign
    maskd = nc.dram_tensor("maskd", [128, 128], BF16, kind="ExternalInput")  # triu 0/1 keep-mask
    tonesd = nc.dram_tensor("tonesd", [128, 1], BF16, kind="ExternalInput")  # 1/CSC
    outt = nc.dram_tensor("outt", [16, 128, S], BF16, kind="ExternalOutput")
    lrt = nc.dram_tensor("lrt", [HG, 1, 16, 128], F32)  # recip bounce: [16,128] -> [1,2048]

    with tile.TileContext(nc) as tc, ExitStack() as top:
        g = top.enter_context(tc.tile_pool(name="glob", bufs=1))
        tcos = g.tile([128, S], BF16)
        tsin = g.tile([128, S], BF16)
        tmask = g.tile([128, 128], BF16)
        ident_f = g.tile([128, 128], F32)
        make_identity(nc, ident_f[:])
        tbqkt = g.tile([128, 8], F32)
        tbvc = g.tile([128, HG], F32)
        tones = g.tile([128, 1], BF16)
        tinv = g.tile([128, 1], F32)
        nc.vector.memset(tinv[:], INV_SC)

        # Whole-kernel residents
        res = top.enter_context(tc.tile_pool(name="res", bufs=1))
        vres = []  # 16 V k-block tiles [128(seq), GQ] bf16
        for t in range(NB):
            vres.append(res.tile([128, GQ], BF16, tag=f"v{t}", name=f"v{t}"))
        cth = {}
        for h in range(HG):
            for gq in range(4):
                cth[(h, gq)] = res.tile([128, 512], F8, tag=f"cth_{h}_{gq}",
                                        name=f"cth_{h}_{gq}")
        ctl = [res.tile([128, HG, 512], F8, tag=f"ctl{gq}", name=f"ctl{gq}")
               for gq in range(4)]

        for _rep in range(reps):
          phB = ExitStack()
          mmp = phB.enter_context(tc.tile_pool(name="mmp", bufs=kn["mmp"], space="PSUM"))
          sps = phB.enter_context(tc.tile_pool(name="sps", bufs=kn["sps"], space="PSUM"))
          smps = phB.enter_context(tc.tile_pool(name="smps", bufs=1, space="PSUM"))
          expp = phB.enter_context(tc.tile_pool(name="expp", bufs=kn["expb"]))
          lrp = phB.enter_context(tc.tile_pool(name="lrp", bufs=2))
          rbp = phB.enter_context(tc.tile_pool(name="rbp", bufs=2))
          ctsp = phB.enter_context(tc.tile_pool(name="ctsp", bufs=2))
          obp = phB.enter_context(tc.tile_pool(name="obp", bufs=kn["obp"]))
          smt = smps.tile([128, 132], F32, tag="sm", name="smt")

          phA = ExitStack()
          wqp = phA.enter_context(tc.tile_pool(name="wqp", bufs=kn["wqb"]))
          qkp = phA.enter_context(tc.tile_pool(name="qkp", bufs=kn["qkb"]))
          stg = phA.enter_context(tc.tile_pool(name="stg", bufs=kn["stg"]))
          xps = ExitStack()
          xp = xps.enter_context(tc.tile_pool(name="xp", bufs=1))
          sec0 = ExitStack()
          xsl = sec0.enter_context(tc.tile_pool(name="xsl", bufs=2))
          wvp = sec0.enter_context(tc.tile_pool(name="wvp", bufs=1))

          # ---- initial DMA order (startup-critical) ----
          wq_tiles = {}  # (h) -> (wqh, wql, wkh, wkl)

          def weights_dma(h):
              tl = []
              for mt, tag in ((h, "wqh"), (4 + h, "wkh")):
                  wt = wqp.tile([128, 8, 2, 128], F8, tag=tag, name=f"{tag}{h}")
                  nc.sync.dma_start(out=wt, in_=wqkh[mt])
                  tl.append(wt)
              wq_tiles[h] = tl

          # head-0 weights + first x slab first
          wt = wqp.tile([128, 8, 2, 128], F8, tag="wqh", name="wqh0")
          nc.sync.dma_start(out=wt[:, 0:2], in_=wqkh[0, :, 0:2])
          xres = [xp.tile([128, 8, 2, SLAB], F8, tag=f"x{ns}", name=f"x{ns}")
                  for ns in range(4)]
          nc.sync.dma_start(out=xres[0][:, 0:2], in_=xh[:, 0, 0:2])
          nc.sync.dma_start(out=wt[:, 2:8], in_=wqkh[0, :, 2:8])
          nc.sync.dma_start(out=xres[0][:, 2:5], in_=xh[:, 0, 2:5])
          wt2 = wqp.tile([128, 8, 2, 128], F8, tag="wkh", name="wkh0")
          nc.sync.dma_start(out=wt2, in_=wqkh[4])
          wq_tiles[0] = [wt, wt2]
          nc.sync.dma_start(out=xres[0][:, 5:8], in_=xh[:, 0, 5:8])
          # small consts needed by first psum copies / rope
          nc.sync.dma_start(out=tbqkt, in_=bqkt[:])
          nc.sync.dma_start(out=xres[1][:, 0:4], in_=xh[:, 1, 0:4])
          nc.sync.dma_start(out=tcos, in_=cost[:])
          nc.sync.dma_start(out=xres[1][:, 4:8], in_=xh[:, 1, 4:8])
          nc.sync.dma_start(out=tsin, in_=sinrt[:])
          nc.sync.dma_start(out=xres[2][:, 0:4], in_=xh[:, 2, 0:4])
          nc.sync.dma_start(out=xres[2][:, 4:8], in_=xh[:, 2, 4:8])
          nc.sync.dma_start(out=tones, in_=tonesd[:])
          nc.sync.dma_start(out=xres[3][:, 0:4], in_=xh[:, 3, 0:4])
          nc.sync.dma_start(out=tmask, in_=maskd[:])
          nc.sync.dma_start(out=xres[3][:, 4:8], in_=xh[:, 3, 4:8])
          nc.sync.dma_start(out=tbvc, in_=bvc[:])
          twvh, twvl = [], []
          xlres = {}

          def wv_dmas(which):
              if which == "h":
                  wvht = wvp.tile([128, 8, 2, GQ], F8, tag="wvh", name="wvht")
                  nc.sync.dma_start(out=wvht, in_=wvh.rearrange("a p b c -> p a b c"))
                  twvh.append(wvht)
              else:
                  wvlt = wvp.tile([128, 8, 2, GQ], F8, tag="wvl", name="wvlt")
                  nc.sync.dma_start(out=wvlt, in_=wvl.rearrange("a p b c -> p a b c"))
                  twvl.append(wvlt)

          def xl_dma(ns):
              xt = xsl.tile([128, 8, 2, SLAB], F8, tag="xl", name=f"xl{ns}")
              nc.sync.dma_start(out=xt, in_=xl[:, ns])
              xlres[ns] = xt

          # ---- per-head state ----
          qf8 = {}
          kpair = {}
          expT = {}
          lrec_cur = {}
          recrow = {}

          def chain_qk(h, which, ns):
              wqh_, wkh_ = wq_tiles[h]
              whi = wqh_ if which == "q" else wkh_
              sl = slice(ns * SLAB, (ns + 1) * SLAB)
              ps = mmp.tile([128, SLAB], F32, tag="mm")
              for kc2 in range(8):
                  nc.tensor.matmul(ps[:], whi[:, kc2, :, :], xres[ns][:, kc2, :, :],
                                   start=(kc2 == 0), stop=(kc2 == 7), perf_mode=DR)
              st = stg.tile([128, SLAB], BF16, tag="st")
              mt = h if which == "q" else 4 + h
              nc.vector.tensor_scalar(out=st[:], in0=ps[:], scalar1=INV_SC,
                                      scalar2=tbqkt[:, mt:mt + 1],
                                      op0=MULT, op1=ADD)
              # RoPE: out = st*cos + swap(st)*sin_rot   (tables pre-scaled by QSC)
              sw = stg.tile([128, SLAB], BF16, tag="sw")
              nc.vector.tensor_copy(out=sw[0:64, :], in_=st[64:128, :])
              nc.vector.tensor_copy(out=sw[64:128, :], in_=st[0:64, :])
              m1 = stg.tile([128, SLAB], BF16, tag="m1")
              nc.vector.tensor_tensor(out=m1[:], in0=st[:], in1=tcos[:, sl], op=MULT)
              nc.vector.tensor_tensor(out=sw[:], in0=sw[:], in1=tsin[:, sl], op=MULT)
              if which == "q":
                  nc.vector.tensor_tensor(out=qf8[h][:, sl], in0=m1[:], in1=sw[:], op=ADD)
              else:
                  kb = m1
                  nc.vector.tensor_tensor(out=kb[:], in0=m1[:], in1=sw[:], op=ADD)
                  if h == 0:
                      nc.scalar.copy(out=kpair[h][:, 0, sl], in_=kb[:])
                      nc.vector.tensor_tensor(out=kpair[h][:, 1, sl], in0=kb[:],
                                              in1=kpair[h][:, 0, sl], op=SUB)
                  else:
                      nc.gpsimd.tensor_scalar(out=kpair[h][:, 0, sl], in0=kb[:],
                                              scalar1=0.0, scalar2=None, op0=ADD)
                      nc.gpsimd.tensor_tensor(out=kpair[h][:, 1, sl], in0=kb[:],
                                              in1=kpair[h][:, 0, sl], op=SUB)

          def v_tile(t):
              ns, sti = divmod(t, 4)
              s0 = sti * 128
              pv = mmp.tile([128, GQ], F32, tag="mm")
              passes = [(xres[ns], twvh[0]), (xlres[ns], twvh[0]), (xres[ns], twvl[0])]
              for pi, (xt_, wv_) in enumerate(passes):
                  for kc2 in range(8):
                      nc.tensor.matmul(pv[:], xt_[:, kc2, :, s0:s0 + 128],
                                       wv_[:, kc2, :, :], start=(pi == 0 and kc2 == 0),
                                       stop=(pi == 2 and kc2 == 7), perf_mode=DR)
              nc.scalar.activation(out=vres[t], in_=pv[:], func=IDENT,
                                   scale=INV_SC)

          def rec_group(h, gq):
              # ship recip(ell) for q-blocks 4g..4g+3 to DRAM and back as a row
              rt = smt[0:4, 0:128]
              nc.tensor.transpose(rt, lrec_cur[h][:, 4 * gq:4 * gq + 4], ident_f[:])
              rts = lrp.tile([4, 128], F32, tag="rts")
              nc.vector.tensor_copy(out=rts[:], in_=rt)
              nc.sync.dma_start(out=lrt[h, 0, 4 * gq:4 * gq + 4, :], in_=rts[:])
              nc.sync.dma_start(out=recrow[h][:, 4 * gq:4 * gq + 4, :],
                                in_=lrt[h, :, 4 * gq:4 * gq + 4, :])

          def denom(h, b):
              # ell[q] for q-block b: sum_k exp tiles via ap-1 matmuls, then recip
              lp = smt[:, 128 + (b % 4):129 + (b % 4)]
              for j in range(b + 1):
                  nc.tensor.matmul(lp, expT[h][j][:, (b - j) * 128:(b - j + 1) * 128],
                                   tones[:], start=(j == 0), stop=(j == b))
              nc.vector.reciprocal(out=lrec_cur[h][:, b:b + 1], in_=lp)

          def sweep2_group(h, gq, split_at=None, mid_fn=None):
              # ct = (sum_k V^T[k] expS^T[k]) * recip -> split into fp8 hi/lo
              rbs = rbp.tile([128, 512], F32, tag="rbs")
              nc.gpsimd.partition_broadcast(
                  rbs[:], recrow[h][:, 4 * gq:4 * gq + 4, :])
              ct = mmp.tile([128, 512], F32, tag="mm")
              last = 4 * gq + 3
              for j in range(last + 1):
                  if split_at is not None and j == split_at:
                      mid_fn()
                  if j <= 4 * gq:
                      nc.tensor.matmul(ct[:], vres[j][:, h * 128:(h + 1) * 128],
                                       expT[h][j][:, (4 * gq - j) * 128:(4 * gq - j) * 128 + 512],
                                       start=(j == 0), stop=(j == last))
                  else:
                      w = (4 * gq + 4 - j) * 128
                      nc.tensor.matmul(ct[:, 512 - w:512], vres[j][:, h * 128:(h + 1) * 128],
                                       expT[h][j][:, 0:w], start=False, stop=(j == last))
              ctb = ctsp.tile([128, 512], BF16, tag="ctb")
              nc.vector.tensor_tensor(out=ctb[:], in0=ct[:], in1=rbs[:], op=MULT)
              nc.gpsimd.tensor_scalar(out=cth[(h, gq)][:], in0=ctb[:],
                                      scalar1=tbvc[:, h:h + 1], scalar2=None,
                                      op0=ADD)
              nc.vector.scalar_tensor_tensor(
                  out=ctl[gq][:, h, :], in0=ctb[:], scalar=tbvc[:, h:h + 1],
                  in1=cth[(h, gq)][:], op0=ADD, op1=SUB)

          def scores_head(h, interleave):
              expT[h] = []
              lrec_cur[h] = lrp.tile([128, 16], F32, tag="lrec", name="lrec", bufs=1)
              recrow[h] = lrp.tile([1, 16, 128], F32, tag="recrow", name="recrow", bufs=1)
              kp = kpair[h]
              qf = qf8[h]
              for i in range(NB):
                  w = (NB - i) * 128
                  ex = expp.tile([128, w], BF16, tag=f"expT{i}", name=f"expT{i}",
                                 bufs=2 if i < 4 else 1)
                  expT[h].append(ex)
              # chunk list; head 0 emits in slab-availability wavefront order
              chunks = []
              for i in range(NB):
                  w = (NB - i) * 128
                  for c0 in range(0, w, 512):
                      cw = min(512, w - c0)
                      p = max((i * 128 + c0 + cw - 1) // 512, i // 4)
                      chunks.append((p, i, c0, cw))
              if h == 0:
                  chunks.sort()
              nch = len(chunks)
              per_block = [0] * NB
              for _p, i_, _c, _w in chunks:
                  per_block[i_] += 1
              done = [0] * NB
              dfired = 0
              mi = 0
              if h == 0:
                  milestones = [((i + 1) * nch + NB - 1) // NB for i in range(NB)]
              else:
                  acc = 0
                  milestones = []
                  for i in range(NB):
                      acc += per_block[i]
                      milestones.append(acc)
              emitted_p = [-1]

              def on_block_complete(b):
                  # fire denoms (2-block lag), recips, and own sweeps
                  nonlocal dfired
                  while dfired <= b - 2:
                      d = dfired
                      denom(h, d)
                      dfired += 1
                      if d >= 3 and d % 4 == 3:
                          rec_group(h, d // 4)
                      if d >= 5 and (d - 5) % 4 == 0 and (d - 5) // 4 <= 2:
                          sweep2_group(h, (d - 5) // 4)

              for n, (p, i, c0, cw) in enumerate(chunks, 1):
                  if h == 0 and p > emitted_p[0]:
                      for ns_ in range(emitted_p[0] + 1, p + 1):
                          chain_qk(0, "q", ns_)
                          chain_qk(0, "k", ns_)
                      emitted_p[0] = p
                  ex = expT[h][i]
                  sp = sps.tile([128, 512], F32, tag="sp")
                  q0 = i * 128 + c0
                  nc.tensor.matmul(
                      sp[:, 0:cw], kp[:, :, i * 128:(i + 1) * 128],
                      qf[:, q0:q0 + cw].unsqueeze(1).broadcast_to((128, 2, cw)),
                      start=True, stop=True, perf_mode=DR)
                  nc.scalar.activation(out=ex[:, c0:c0 + cw], in_=sp[:, 0:cw],
                                       func=EXPF, scale=EXP_SCALE)
                  if c0 == 0:
                      nc.gpsimd.tensor_tensor(out=ex[:, 0:128], in0=ex[:, 0:128],
                                              in1=tmask[:], op=MULT)
                  done[i] += 1
                  if done[i] == per_block[i]:
                      on_block_complete(i)
                  while mi < NB and n >= milestones[mi]:
                      interleave(h, mi)
                      mi += 1
              while dfired < NB:
                  d = dfired
                  denom(h, d)
                  dfired += 1
                  if d >= 3 and d % 4 == 3:
                      rec_group(h, d // 4)
                  if d >= 5 and (d - 5) % 4 == 0 and (d - 5) // 4 <= 2:
                      sweep2_group(h, (d - 5) // 4)

          def alloc_qk(h):
              qf8[h] = qkp.tile([128, S], F8, tag="qf8", name=f"qf8_{h}")
              kpair[h] = qkp.tile([128, 2, S], F8, tag="kpair", name=f"kpair_{h}")

          wo1t = []
          wo2t = []
          tailp = ExitStack()

          def open_wop():
              xps.close()
              wop = tailp.enter_context(tc.tile_pool(name="wop", bufs=1))
              for kh in range(HG):
                  wt_ = wop.tile([128, 2, D], F8, tag=f"wo1_{kh}", name=f"wo1_{kh}")
                  nc.sync.dma_start(out=wt_, in_=wo1[kh])
                  wo1t.append(wt_)
              for gp in range(2):
                  wt_ = wop.tile([128, 2, D], F8, tag=f"wo2_{gp}", name=f"wo2_{gp}")
                  nc.sync.dma_start(out=wt_, in_=wo2[gp])
                  wo2t.append(wt_)

          ob_cur = [None]

          def p3_mt(gq, mt, tail=False, insec=False):
              if tail and mt % 2 == 0:
                  op = sps.tile([128, 512], F32, tag="sp", name="op")
              else:
                  op = mmp.tile([128, 512], F32, tag="mm")
              for kh in range(HG):
                  nc.tensor.matmul(
                      op[:], wo1t[kh][:, :, mt * 128:(mt + 1) * 128],
                      cth[(kh, gq)][:].unsqueeze(1).broadcast_to((128, 2, 512)),
                      start=(kh == 0), stop=False, perf_mode=DR)
              for gp in range(2):
                  nc.tensor.matmul(
                      op[:], wo2t[gp][:, :, mt * 128:(mt + 1) * 128],
                      ctl[gq][:, 2 * gp:2 * gp + 2, :],
                      start=False, stop=(gp == 1), perf_mode=DR)
              if mt % 2 == 0:
                  ob_cur[0] = obp.tile([128, 2, 512], BF16, tag="ob", name="ob")
              ob = ob_cur[0]
              half = ob[:, mt % 2, :]
              if mt % 2 == 0:
                  nc.vector.tensor_copy(out=half, in_=op[:])
              else:
                  nc.scalar.copy(out=half, in_=op[:])
              if mt % 2 == 1:
                  nc.sync.dma_start(
                      out=outt[mt - 1:mt + 1, :, gq * 512:(gq + 1) * 512]
                      .rearrange("m p s -> p m s"),
                      in_=ob[:])

          def mk_sched(h):
              # schedule of extra PE work per score block of head h
              sched = {i: [] for i in range(NB)}
              if h == 0:
                  sched[2].append(lambda: alloc_qk(1))
                  for idx, (which, ns) in enumerate(
                          (w, n) for n in range(4) for w in ("q", "k")):
                      sched[2 + idx].append(
                          lambda w=which, n=ns: chain_qk(1, w, n))
                  sched[2].append(lambda: xl_dma(2))
                  sched[5].append(lambda: xl_dma(3))
                  for t in range(13):           # v tiles 0-12 at blocks 3-15
                      sched[3 + t].append(lambda t=t: v_tile(t))
                  sched[10].append(lambda: weights_dma(2))
              else:
                  if h == 1:
                      def _mid():
                          for t in (13, 14, 15):
                              v_tile(t)
                          sec0.close()
                      sched[0].append(
                          lambda: sweep2_group(0, 3, split_at=13, mid_fn=_mid))
                  else:
                      sched[0].append(lambda: sweep2_group(h - 1, 3))
                  if h < 3:
                      sched[2].append(lambda: alloc_qk(h + 1))
                      for idx, (which, ns) in enumerate(
                              (w, n) for n in range(4) for w in ("q", "k")):
                          sched[2 + idx].append(
                              lambda w=which, n=ns: chain_qk(h + 1, w, n))
                      if h + 2 < HG:
                          sched[10].append(lambda: weights_dma(h + 2))
                  if h == 2:
                      sched[6].append(open_wop)
                  if h == 3:
                      for j in range(16):       # p3 gq0 at blocks 8-15
                          sched[8 + j // 2].append(
                              lambda mt=j: p3_mt(0, mt, insec=True))
                      for j in range(4):        # first p3 gq1 pieces
                          sched[14 + j // 2].append(
                              lambda mt=j: p3_mt(1, mt, insec=True))
              return sched

          def run_sched(sched, h, i):
              for fn in sched[i]:
                  fn()

          # ---- emit sections ----
          alloc_qk(0)
          weights_dma(1)
          wv_dmas("h")
          xl_dma(0)
          wv_dmas("l")
          xl_dma(1)
          for h in range(HG):
              sched = mk_sched(h)
              scores_head(h, lambda hh, i, sched=sched: run_sched(sched, hh, i))

          # ---- tail: rest of the output projection ----
          for mt in range(4, 16):
              p3_mt(1, mt, tail=True)
          sweep2_group(3, 3)
          for mt in range(16):
              p3_mt(2, mt, tail=True)
          for mt in range(16):
              p3_mt(3, mt, tail=True)
          tailp.close()
          phA.close()
          phB.close()
    nc.finalize()
    return nc


_NC_CACHE = {}


def _get_nc(reps=1):
    if reps not in _NC_CACHE:
        _NC_CACHE[reps] = build_nc(reps)
    return _NC_CACHE[reps]


def _rope_tables(position_ids_b):
    pos = position_ids_b.astype(np.float32)
    inv_freq = (1.0 / (ROPE_THETA ** (np.arange(0, DH, 2, dtype=np.float32) / np.float32(DH))))
    ang = pos[:, None] * inv_freq[None, :]          # [S, 64]
    emb = np.concatenate([ang, ang], axis=-1)       # [S, 128]
    cosT = np.ascontiguousarray(np.cos(emb).T) * np.float32(QSC)   # [128, S]
    sinT = np.sin(emb).T * np.float32(QSC)
    sin_rot = np.concatenate([-sinT[0:64], sinT[64:128]], axis=0)
    return cosT.astype(ml_dtypes.bfloat16), np.ascontiguousarray(sin_rot).astype(ml_dtypes.bfloat16)


def _make_in_maps(inputs):
    hidden_states = np.asarray(inputs["hidden_states"], dtype=np.float32)
    position_ids = np.asarray(inputs["position_ids"])
    Wqkv = np.asarray(inputs["Wqkv"], dtype=np.float32)
    bqkv = np.asarray(inputs["bqkv"], dtype=np.float32)
    Wo = np.asarray(inputs["Wo"], dtype=np.float32)

    mask = np.triu(np.ones((128, 128), dtype=np.float32)).astype(ml_dtypes.bfloat16)
    tones = np.full((128, 1), 1.0 / CSC, dtype=ml_dtypes.bfloat16)
    tabs = [_rope_tables(np.asarray(position_ids)[b]) for b in range(B)]

    def _hilo(M, sc):
        Ms = M * np.float32(sc)
        hi = Ms.astype(ml_dtypes.float8_e4m3)
        lo = (Ms - hi.astype(np.float32)).astype(ml_dtypes.float8_e4m3)
        return hi, lo

    def _pack_pairs(M):
        # [D, C] -> [8, 128, 2, C] with row r = kc2*256 + i*128 + p
        C = M.shape[1]
        return np.ascontiguousarray(M.reshape(8, 2, 128, C).transpose(0, 2, 1, 3))

    def _pack_x(M):
        # [D, S] -> [128, 4, 8, 2, SLAB] partition-major, slab-major free
        return np.ascontiguousarray(
            M.reshape(8, 2, 128, 4, SLAB).transpose(2, 3, 0, 1, 4))

    xts = []
    for b in range(B):
        hi, lo = _hilo(np.ascontiguousarray(hidden_states[b].T), XSC)
        xts.append((_pack_x(hi.astype(np.float32)).astype(ml_dtypes.float8_e4m3),
                    _pack_x(lo.astype(np.float32)).astype(ml_dtypes.float8_e4m3)))

    in_maps = []
    for c in range(NCORES):
        b, hg = divmod(c, HG)
        qcols = slice(hg * GQ, (hg + 1) * GQ)
        kcols = slice(D + hg * GQ, D + (hg + 1) * GQ)
        vcols = slice(2 * D + hg * GQ, 2 * D + (hg + 1) * GQ)
        wqk_c = np.ascontiguousarray(np.concatenate([Wqkv[:, qcols], Wqkv[:, kcols]], axis=1))
        qk_h, qk_l = _hilo(wqk_c, WSC)
        # per-mt packing: [8(mt), 128(p), 8(kc2), 2(i), 128(m)]
        def _pack_mt(M8):
            P = _pack_pairs(M8.astype(np.float32))          # [8, 128, 2, 1024]
            P = P.reshape(8, 128, 2, 8, 128)                 # [kc2, p, i, mt, m]
            return np.ascontiguousarray(P.transpose(3, 1, 0, 2, 4)).astype(ml_dtypes.float8_e4m3)
        wqkh_c = _pack_mt(qk_h)
        wqkl_c = _pack_mt(qk_l)
        wv_c = np.ascontiguousarray(Wqkv[:, vcols])
        v_h, v_l = _hilo(wv_c, WSC)
        wvh_c = _pack_pairs(v_h.astype(np.float32)).astype(ml_dtypes.float8_e4m3)
        wvl_c = _pack_pairs(v_l.astype(np.float32)).astype(ml_dtypes.float8_e4m3)
        # out-proj fp8 packs
        wo_c = np.ascontiguousarray(Wo[hg * GQ:(hg + 1) * GQ, :])   # [512, D]
        wo_h, wo_l = _hilo(wo_c, WSC)
        wo_h = wo_h.astype(np.float32)
        wo_l = wo_l.astype(np.float32)
        wo1_c = np.empty((HG, 128, 2, D), np.float32)
        for kh in range(HG):
            wo1_c[kh, :, 0, :] = wo_h[kh * 128:(kh + 1) * 128, :]
            wo1_c[kh, :, 1, :] = wo_l[kh * 128:(kh + 1) * 128, :]
        wo2_c = np.empty((2, 128, 2, D), np.float32)
        for gp in range(2):
            wo2_c[gp, :, 0, :] = wo_h[gp * 256:gp * 256 + 128, :]
            wo2_c[gp, :, 1, :] = wo_h[gp * 256 + 128:gp * 256 + 256, :]
        bqk_c = np.concatenate([bqkv[qcols], bqkv[kcols]]).reshape(8, 128).T
        bvc_c = np.ascontiguousarray(bqkv[vcols].reshape(HG, 128).T * np.float32(CSC))
        cosT, sin_rot = tabs[b]
        in_maps.append({
            "xh": xts[b][0], "xl": xts[b][1],
            "wqkh": wqkh_c, "wqkl": wqkl_c, "wvh": wvh_c, "wvl": wvl_c,
            "wo1": wo1_c.astype(ml_dtypes.float8_e4m3),
            "wo2": wo2_c.astype(ml_dtypes.float8_e4m3),
            "bqkt": np.ascontiguousarray(bqk_c),
            "bvc": bvc_c,
            "cost": cosT, "sinrt": sin_rot, "maskd": mask,
            "tonesd": tones,
        })
    return in_maps


def kernel(hidden_states, position_ids, Wqkv, bqkv, Wo, bo, _reps=1):
    bo = np.asarray(bo, dtype=np.float32)
    in_maps = _make_in_maps({
        "hidden_states": hidden_states, "position_ids": position_ids,
        "Wqkv": Wqkv, "bqkv": bqkv, "Wo": Wo, "bo": bo,
    })
    nc = _get_nc(_reps)
    res = run_bass_kernel_spmd(nc, in_maps, core_ids=list(range(NCORES)))

    out = np.empty((B, S, D), dtype=np.float32)
    for b in range(B):
        acc = res.results[b * HG]["outt"].reshape(D, S).astype(np.float32).copy()
        for hg in range(1, HG):
            acc += res.results[b * HG + hg]["outt"].reshape(D, S).astype(np.float32)
        out[b] = acc.T * np.float32(P3_SCALE) + bo[None, :]
    return out


# revision 59
# speedup vs baseline: 1.3298x; 1.0035x over previous
"""Trainium2 Bass kernel for CustomRoPEAttention (B=2, S=2048, H=16, Dh=128).

Sharding: 8 cores = 2 batches x 4 head-groups (4 heads/core).

Head-pipelined structure: per head h, QKV^T projection (fp8 hi/lo DoubleRow
matmuls) + RoPE, then transposed-layout causal attention for that head while
later heads' projections stream -- this overlaps the ACT-engine exp work with
PE-engine matmul work across the whole kernel instead of serializing phases.

fp8 DoubleRow "dup trick": scores use stationary (k_hi,k_lo) pairs against a
broadcast (step-0) fp8 q moving operand, and the output projection uses
(wo_hi,wo_lo) pairs against broadcast ct_hi plus a wo_hi x ct_lo correction --
half / 0.75x the bf16 PE time at first-order-exact precision.

Host sums the 4 partial (transposed) output projections per batch.

Self-contained: hardcodes shapes from the problem spec.
"""
import math
from contextlib import ExitStack

import numpy as np
import ml_dtypes

import concourse.mybir as mybir
import concourse.tile as tile
from concourse import bacc
from concourse.bass_utils import run_bass_kernel_spmd
from concourse.masks import make_identity

S = 2048            # sequence
D = 2048            # hidden
NH = 16             # total heads
DH = 128            # head dim
HG = 4              # heads per core
GQ = HG * DH        # 512: per-core q/k/v feature width
B = 2
NCORES = 8
ROPE_THETA = 10000.0
SCALE = 1.0 / math.sqrt(DH)
SLAB = 512          # qkv sequence slab width
XSC = 16.0          # fp8 pre-scale for x
WSC = 512.0         # fp8 pre-scale for Wqkv / Wo
QSC = 16.0          # fp8 pre-scale for roped q/k (folded into cos/sin tables)
CSC = 16.0          # fp8 pre-scale for attention-out ct (folded into tones)
INV_SC = 1.0 / (XSC * WSC)
EXP_SCALE = SCALE / (QSC * QSC)
P3_SCALE = 1.0 / (WSC * CSC)   # applied host-side
F32 = mybir.dt.float32
BF16 = mybir.dt.bfloat16
F8 = mybir.dt.float8e4
MULT = mybir.AluOpType.mult
ADD = mybir.AluOpType.add
SUB = mybir.AluOpType.subtract
DR = mybir.MatmulPerfMode.DoubleRow
NB = S // 128       # 16 k/q blocks
IDENT = mybir.ActivationFunctionType.Identity
EXPF = mybir.ActivationFunctionType.Exp


def build_nc(reps=1, knobs=None):
    kn = {"sps": 3, "mmp": 4, "expb": 1, "wqb": 2, "qkb": 2, "stg": 2, "obp": 6}
    if knobs:
        kn.update(knobs)
    nc = bacc.Bacc(None, target_bir_lowering=False)
    # x^T hi/lo, slab-major pack: [p, ns, kc2, i, s]
    xh = nc.dram_tensor("xh", [128, 4, 8, 2, SLAB], F8, kind="ExternalInput")
    xl = nc.dram_tensor("xl", [128, 4, 8, 2, SLAB], F8, kind="ExternalInput")
    # per-mt packed qk weights: [mt, p, kc2, i, m]
    wqkh = nc.dram_tensor("wqkh", [8, 128, 8, 2, 128], F8, kind="ExternalInput")
    wvh = nc.dram_tensor("wvh", [8, 128, 2, GQ], F8, kind="ExternalInput")
    wvl = nc.dram_tensor("wvl", [8, 128, 2, GQ], F8, kind="ExternalInput")
    # out-proj fp8 packs: wo1[kh] = (hi,lo) pairs; wo2[g] = hi head-pair packs
    wo1 = nc.dram_tensor("wo1", [HG, 128, 2, D], F8, kind="ExternalInput")
    wo2 = nc.dram_tensor("wo2", [2, 128, 2, D], F8, kind="ExternalInput")
    bqkt = nc.dram_tensor("bqkt", [128, 8], F32, kind="ExternalInput")
    bvc = nc.dram_tensor("bvc", [128, HG], F32, kind="ExternalInput")  # v bias * CSC
    cost = nc.dram_tensor("cost", [128, S], BF16, kind="ExternalInput")    # cos^T * QSC
    sinrt = nc.dram_tensor("sinrt", [128, S], BF16, kind="ExternalInput")  # sin^T * QSC, rot sign
    maskd = nc.dram_tensor("maskd", [128, 128], BF16, kind="ExternalInput")  # triu 0/1 keep-mask
    tonesd = nc.dram_tensor("tonesd", [128, 1], BF16, kind="ExternalInput")  # 1/CSC
    outt = nc.dram_tensor("outt", [16, 128, S], BF16, kind="ExternalOutput")
    lrt = nc.dram_tensor("lrt", [HG, 1, 16, 128], F32)  # recip bounce: [16,128] -> [1,2048]

    with tile.TileContext(nc) as tc, ExitStack() as top:
        g = top.enter_context(tc.tile_pool(name="glob", bufs=1))
        tcos = g.tile([128, S], BF16)
        tsin = g.tile([128, S], BF16)
        tmask = g.tile([128, 128], BF16)
        ident_f = g.tile([128, 128], F32)
        make_identity(nc, ident_f[:])
        tbqkt = g.tile([128, 8], F32)
        tbvc = g.tile([128, HG], F32)
        tones = g.tile([128, 1], BF16)
        tinv = g.tile([128, 1], F32)
        nc.vector.memset(tinv[:], INV_SC)

        # Whole-kernel residents
        res = top.enter_context(tc.tile_pool(name="res", bufs=1))
        vres = []  # 16 V k-block tiles [128(seq), GQ] bf16
        for t in range(NB):
            vres.append(res.tile([128, GQ], BF16, tag=f"v{t}", name=f"v{t}"))
        cth = {}
        for h in range(HG):
            for gq in range(4):
                cth[(h, gq)] = res.tile([128, 512], F8, tag=f"cth_{h}_{gq}",
                                        name=f"cth_{h}_{gq}")
        ctl = [res.tile([128, HG, 512], F8, tag=f"ctl{gq}", name=f"ctl{gq}")
               for gq in range(4)]

        for _rep in range(reps):
          phB = ExitStack()
          mmp = phB.enter_context(tc.tile_pool(name="mmp", bufs=kn["mmp"], space="PSUM"))
          sps = phB.enter_context(tc.tile_pool(name="sps", bufs=kn["sps"], space="PSUM"))
          smps = phB.enter_context(tc.tile_pool(name="smps", bufs=1, space="PSUM"))
          expp = phB.enter_context(tc.tile_pool(name="expp", bufs=kn["expb"]))
          lrp = phB.enter_context(tc.tile_pool(name="lrp", bufs=2))
          rbp = phB.enter_context(tc.tile_pool(name="rbp", bufs=2))
          ctsp = phB.enter_context(tc.tile_pool(name="ctsp", bufs=2))
          obp = phB.enter_context(tc.tile_pool(name="obp", bufs=kn["obp"]))
          smt = smps.tile([128, 132], F32, tag="sm", name="smt")

          phA = ExitStack()
          wqp = phA.enter_context(tc.tile_pool(name="wqp", bufs=kn["wqb"]))
          qkp = phA.enter_context(tc.tile_pool(name="qkp", bufs=kn["qkb"]))
          stg = phA.enter_context(tc.tile_pool(name="stg", bufs=kn["stg"]))
          xps = ExitStack()
          xp = xps.enter_context(tc.tile_pool(name="xp", bufs=1))
          sec0 = ExitStack()
          xsl = sec0.enter_context(tc.tile_pool(name="xsl", bufs=2))
          wvp = sec0.enter_context(tc.tile_pool(name="wvp", bufs=1))

          # ---- initial DMA order (startup-critical) ----
          wq_tiles = {}  # (h) -> (wqh, wql, wkh, wkl)

          def weights_dma(h):
              tl = []
              for mt, tag in ((h, "wqh"), (4 + h, "wkh")):
                  wt = wqp.tile([128, 8, 2, 128], F8, tag=tag, name=f"{tag}{h}")
                  nc.sync.dma_start(out=wt, in_=wqkh[mt])
                  tl.append(wt)
              wq_tiles[h] = tl

          # head-0 weights + first x slab first
          wt = wqp.tile([128, 8, 2, 128], F8, tag="wqh", name="wqh0")
          nc.sync.dma_start(out=wt[:, 0:2], in_=wqkh[0, :, 0:2])
          xres = [xp.tile([128, 8, 2, SLAB], F8, tag=f"x{ns}", name=f"x{ns}")
                  for ns in range(4)]
          nc.sync.dma_start(out=xres[0][:, 0:2], in_=xh[:, 0, 0:2])
          nc.sync.dma_start(out=wt[:, 2:8], in_=wqkh[0, :, 2:8])
          nc.sync.dma_start(out=xres[0][:, 2:5], in_=xh[:, 0, 2:5])
          wt2 = wqp.tile([128, 8, 2, 128], F8, tag="wkh", name="wkh0")
          nc.sync.dma_start(out=wt2, in_=wqkh[4])
          wq_tiles[0] = [wt, wt2]
          nc.sync.dma_start(out=xres[0][:, 5:8], in_=xh[:, 0, 5:8])
          # small consts needed by first psum copies / rope
          nc.sync.dma_start(out=tbqkt, in_=bqkt[:])
          nc.sync.dma_start(out=xres[1][:, 0:4], in_=xh[:, 1, 0:4])
          nc.sync.dma_start(out=tcos, in_=cost[:])
          nc.sync.dma_start(out=xres[1][:, 4:8], in_=xh[:, 1, 4:8])
          nc.sync.dma_start(out=tsin, in_=sinrt[:])
          nc.sync.dma_start(out=xres[2][:, 0:4], in_=xh[:, 2, 0:4])
          nc.sync.dma_start(out=xres[2][:, 4:8], in_=xh[:, 2, 4:8])
          nc.sync.dma_start(out=tones, in_=tonesd[:])
          nc.sync.dma_start(out=xres[3][:, 0:4], in_=xh[:, 3, 0:4])
          nc.sync.dma_start(out=tmask, in_=maskd[:])
          nc.sync.dma_start(out=xres[3][:, 4:8], in_=xh[:, 3, 4:8])
          nc.sync.dma_start(out=tbvc, in_=bvc[:])
          twvh, twvl = [], []
          xlres = {}

          def wv_dmas(which):
              if which == "h":
                  wvht = wvp.tile([128, 8, 2, GQ], F8, tag="wvh", name="wvht")
                  nc.sync.dma_start(out=wvht, in_=wvh.rearrange("a p b c -> p a b c"))
                  twvh.append(wvht)
              else:
                  wvlt = wvp.tile([128, 8, 2, GQ], F8, tag="wvl", name="wvlt")
                  nc.sync.dma_start(out=wvlt, in_=wvl.rearrange("a p b c -> p a b c"))
                  twvl.append(wvlt)

          def xl_dma(ns):
              xt = xsl.tile([128, 8, 2, SLAB], F8, tag="xl", name=f"xl{ns}")
              nc.sync.dma_start(out=xt, in_=xl[:, ns])
              xlres[ns] = xt

          # ---- per-head state ----
          qf8 = {}
          kpair = {}
          expT = {}
          lrec_cur = {}
          recrow = {}

          def chain_qk(h, which, ns):
              wqh_, wkh_ = wq_tiles[h]
              whi = wqh_ if which == "q" else wkh_
              sl = slice(ns * SLAB, (ns + 1) * SLAB)
              ps = mmp.tile([128, SLAB], F32, tag="mm")
              for kc2 in range(8):
                  nc.tensor.matmul(ps[:], whi[:, kc2, :, :], xres[ns][:, kc2, :, :],
                                   start=(kc2 == 0), stop=(kc2 == 7), perf_mode=DR)
              st = stg.tile([128, SLAB], BF16, tag="st")
              mt = h if which == "q" else 4 + h
              nc.vector.tensor_scalar(out=st[:], in0=ps[:], scalar1=INV_SC,
                                      scalar2=tbqkt[:, mt:mt + 1],
                                      op0=MULT, op1=ADD)
              # RoPE: out = st*cos + swap(st)*sin_rot   (tables pre-scaled by QSC)
              sw = stg.tile([128, SLAB], BF16, tag="sw")
              nc.vector.tensor_copy(out=sw[0:64, :], in_=st[64:128, :])
              nc.vector.tensor_copy(out=sw[64:128, :], in_=st[0:64, :])
              m1 = stg.tile([128, SLAB], BF16, tag="m1")
              nc.vector.tensor_tensor(out=m1[:], in0=st[:], in1=tcos[:, sl], op=MULT)
              nc.vector.tensor_tensor(out=sw[:], in0=sw[:], in1=tsin[:, sl], op=MULT)
              if which == "q":
                  nc.vector.tensor_tensor(out=qf8[h][:, sl], in0=m1[:], in1=sw[:], op=ADD)
              else:
                  kb = m1
                  nc.vector.tensor_tensor(out=kb[:], in0=m1[:], in1=sw[:], op=ADD)
                  if h == 0:
                      nc.scalar.copy(out=kpair[h][:, 0, sl], in_=kb[:])
                      nc.vector.tensor_tensor(out=kpair[h][:, 1, sl], in0=kb[:],
                                              in1=kpair[h][:, 0, sl], op=SUB)
                  else:
                      nc.gpsimd.tensor_scalar(out=kpair[h][:, 0, sl], in0=kb[:],
                                              scalar1=0.0, scalar2=None, op0=ADD)
                      nc.gpsimd.tensor_tensor(out=kpair[h][:, 1, sl], in0=kb[:],
                                              in1=kpair[h][:, 0, sl], op=SUB)

          def v_tile(t):
              ns, sti = divmod(t, 4)
              s0 = sti * 128
              pv = mmp.tile([128, GQ], F32, tag="mm")
              passes = [(xres[ns], twvh[0]), (xlres[ns], twvh[0]), (xres[ns], twvl[0])]
              for pi, (xt_, wv_) in enumerate(passes):
                  for kc2 in range(8):
                      nc.tensor.matmul(pv[:], xt_[:, kc2, :, s0:s0 + 128],
                                       wv_[:, kc2, :, :], start=(pi == 0 and kc2 == 0),
                                       stop=(pi == 2 and kc2 == 7), perf_mode=DR)
              nc.scalar.activation(out=vres[t], in_=pv[:], func=IDENT,
                                   scale=INV_SC)

          def rec_group(h, gq):
              # ship recip(ell) for q-blocks 4g..4g+3 to DRAM and back as a row
              rt = smt[0:4, 0:128]
              nc.tensor.transpose(rt, lrec_cur[h][:, 4 * gq:4 * gq + 4], ident_f[:])
              rts = lrp.tile([4, 128], F32, tag="rts")
              nc.vector.tensor_copy(out=rts[:], in_=rt)
              nc.sync.dma_start(out=lrt[h, 0, 4 * gq:4 * gq + 4, :], in_=rts[:])
              nc.sync.dma_start(out=recrow[h][:, 4 * gq:4 * gq + 4, :],
                                in_=lrt[h, :, 4 * gq:4 * gq + 4, :])

          def denom(h, b):
              # ell[q] for q-block b: sum_k exp tiles via ap-1 matmuls, then recip
              lp = smt[:, 128 + (b % 4):129 + (b % 4)]
              for j in range(b + 1):
                  nc.tensor.matmul(lp, expT[h][j][:, (b - j) * 128:(b - j + 1) * 128],
                                   tones[:], start=(j == 0), stop=(j == b))
              nc.vector.reciprocal(out=lrec_cur[h][:, b:b + 1], in_=lp)

          def sweep2_group(h, gq, split_at=None, mid_fn=None):
              # ct = (sum_k V^T[k] expS^T[k]) * recip -> split into fp8 hi/lo
              rbs = rbp.tile([128, 512], F32, tag="rbs")
              nc.gpsimd.partition_broadcast(
                  rbs[:], recrow[h][:, 4 * gq:4 * gq + 4, :])
              ct = mmp.tile([128, 512], F32, tag="mm")
              last = 4 * gq + 3
              for j in range(last + 1):
                  if split_at is not None and j == split_at:
                      mid_fn()
                  if j <= 4 * gq:
                      nc.tensor.matmul(ct[:], vres[j][:, h * 128:(h + 1) * 128],
                                       expT[h][j][:, (4 * gq - j) * 128:(4 * gq - j) * 128 + 512],
                                       start=(j == 0), stop=(j == last))
                  else:
                      w = (4 * gq + 4 - j) * 128
                      nc.tensor.matmul(ct[:, 512 - w:512], vres[j][:, h * 128:(h + 1) * 128],
                                       expT[h][j][:, 0:w], start=False, stop=(j == last))
              ctb = ctsp.tile([128, 512], BF16, tag="ctb")
              nc.vector.tensor_tensor(out=ctb[:], in0=ct[:], in1=rbs[:], op=MULT)
              nc.gpsimd.tensor_scalar(out=cth[(h, gq)][:], in0=ctb[:],
                                      scalar1=tbvc[:, h:h + 1], scalar2=None,
                                      op0=ADD)
              nc.vector.scalar_tensor_tensor(
                  out=ctl[gq][:, h, :], in0=ctb[:], scalar=tbvc[:, h:h + 1],
                  in1=cth[(h, gq)][:], op0=ADD, op1=SUB)

          def scores_head(h, interleave):
              expT[h] = []
              lrec_cur[h] = lrp.tile([128, 16], F32, tag="lrec", name="lrec", bufs=1)
              recrow[h] = lrp.tile([1, 16, 128], F32, tag="recrow", name="recrow", bufs=1)
              kp = kpair[h]
              qf = qf8[h]
              for i in range(NB):
                  w = (NB - i) * 128
                  ex = expp.tile([128, w], BF16, tag=f"expT{i}", name=f"expT{i}",
                                 bufs=2 if i < 4 else 1)
                  expT[h].append(ex)
              # chunk list; head 0 emits in slab-availability wavefront order
              chunks = []
              for i in range(NB):
                  w = (NB - i) * 128
                  for c0 in range(0, w, 512):
                      cw = min(512, w - c0)
                      p = max((i * 128 + c0 + cw - 1) // 512, i // 4)
                      chunks.append((p, i, c0, cw))
              if h == 0:
                  chunks.sort()
              nch = len(chunks)
              per_block = [0] * NB
              for _p, i_, _c, _w in chunks:
                  per_block[i_] += 1
              done = [0] * NB
              dfired = 0
              mi = 0
              if h == 0:
                  milestones = [((i + 1) * nch + NB - 1) // NB for i in range(NB)]
              else:
                  acc = 0
                  milestones = []
                  for i in range(NB):
                      acc += per_block[i]
                      milestones.append(acc)
              emitted_p = [-1]

              def on_block_complete(b):
                  # fire denoms (2-block lag), recips, and own sweeps
                  nonlocal dfired
                  while dfired <= b - 2:
                      d = dfired
                      denom(h, d)
                      dfired += 1
                      if d >= 3 and d % 4 == 3:
                          rec_group(h, d // 4)
                      if d >= 5 and (d - 5) % 4 == 0 and (d - 5) // 4 <= 2:
                          sweep2_group(h, (d - 5) // 4)

              for n, (p, i, c0, cw) in enumerate(chunks, 1):
                  if h == 0 and p > emitted_p[0]:
                      for ns_ in range(emitted_p[0] + 1, p + 1):
                          chain_qk(0, "q", ns_)
                          chain_qk(0, "k", ns_)
                      emitted_p[0] = p
                  ex = expT[h][i]
                  sp = sps.tile([128, 512], F32, tag="sp")
                  q0 = i * 128 + c0
                  nc.tensor.matmul(
                      sp[:, 0:cw], kp[:, :, i * 128:(i + 1) * 128],
                      qf[:, q0:q0 + cw].unsqueeze(1).broadcast_to((128, 2, cw)),
                      start=True, stop=True, perf_mode=DR)
                  nc.scalar.activation(out=ex[:, c0:c0 + cw], in_=sp[:, 0:cw],
                                       func=EXPF, scale=EXP_SCALE)
                  if c0 == 0:
                      nc.gpsimd.tensor_tensor(out=ex[:, 0:128], in0=ex[:, 0:128],
                                              in1=tmask[:], op=MULT)
                  done[i] += 1
                  if done[i] == per_block[i]:
                      on_block_complete(i)
                  while mi < NB and n >= milestones[mi]:
                      interleave(h, mi)
                      mi += 1
              while dfired < NB:
                  d = dfired
                  denom(h, d)
                  dfired += 1
                  if d >= 3 and d % 4 == 3:
                      rec_group(h, d // 4)
                  if d >= 5 and (d - 5) % 4 == 0 and (d - 5) // 4 <= 2:
                      sweep2_group(h, (d - 5) // 4)

          def alloc_qk(h):
              qf8[h] = qkp.tile([128, S], F8, tag="qf8", name=f"qf8_{h}")
              kpair[h] = qkp.tile([128, 2, S], F8, tag="kpair", name=f"kpair_{h}")

          wo1t = []
          wo2t = []
          tailp = ExitStack()

          def open_wop():
              xps.close()
              wop = tailp.enter_context(tc.tile_pool(name="wop", bufs=1))
              for kh in range(HG):
                  wt_ = wop.tile([128, 2, D], F8, tag=f"wo1_{kh}", name=f"wo1_{kh}")
                  nc.sync.dma_start(out=wt_, in_=wo1[kh])
                  wo1t.append(wt_)
              for gp in range(2):
                  wt_ = wop.tile([128, 2, D], F8, tag=f"wo2_{gp}", name=f"wo2_{gp}")
                  nc.sync.dma_start(out=wt_, in_=wo2[gp])
                  wo2t.append(wt_)

          ob_cur = [None]

          def p3_mt(gq, mt, tail=False, insec=False):
              if tail and mt % 2 == 0:
                  op = sps.tile([128, 512], F32, tag="sp", name="op")
              else:
                  op = mmp.tile([128, 512], F32, tag="mm")
              for kh in range(HG):
                  nc.tensor.matmul(
                      op[:], wo1t[kh][:, :, mt * 128:(mt + 1) * 128],
                      cth[(kh, gq)][:].unsqueeze(1).broadcast_to((128, 2, 512)),
                      start=(kh == 0), stop=False, perf_mode=DR)
              for gp in range(2):
                  nc.tensor.matmul(
                      op[:], wo2t[gp][:, :, mt * 128:(mt + 1) * 128],
                      ctl[gq][:, 2 * gp:2 * gp + 2, :],
                      start=False, stop=(gp == 1), perf_mode=DR)
              if mt % 2 == 0:
                  ob_cur[0] = obp.tile([128, 2, 512], BF16, tag="ob", name="ob")
              ob = ob_cur[0]
              half = ob[:, mt % 2, :]
              if mt % 2 == 0:
                  nc.vector.tensor_copy(out=half, in_=op[:])
              else:
                  nc.scalar.copy(out=half, in_=op[:])
              if mt % 2 == 1:
                  nc.sync.dma_start(
                      out=outt[mt - 1:mt + 1, :, gq * 512:(gq + 1) * 512]
                      .rearrange("m p s -> p m s"),
                      in_=ob[:])

          def mk_sched(h):
              # schedule of extra PE work per score block of head h
              sched = {i: [] for i in range(NB)}
              if h == 0:
                  sched[2].append(lambda: alloc_qk(1))
                  for idx, (which, ns) in enumerate(
                          (w, n) for n in range(4) for w in ("q", "k")):
                      sched[2 + idx].append(
                          lambda w=which, n=ns: chain_qk(1, w, n))
                  sched[2].append(lambda: xl_dma(2))
                  sched[5].append(lambda: xl_dma(3))
                  for t in range(13):           # v tiles 0-12 at blocks 3-15
                      sched[3 + t].append(lambda t=t: v_tile(t))
                  sched[10].append(lambda: weights_dma(2))
              else:
                  if h == 1:
                      def _mid():
                          for t in (13, 14, 15):
                              v_tile(t)
                          sec0.close()
                      sched[0].append(
                          lambda: sweep2_group(0, 3, split_at=13, mid_fn=_mid))
                  else:
                      sched[1].append(lambda: sweep2_group(h - 1, 3))
                  if h < 3:
                      sched[2].append(lambda: alloc_qk(h + 1))
                      for idx, (which, ns) in enumerate(
                              (w, n) for n in range(4) for w in ("q", "k")):
                          sched[2 + idx].append(
                              lambda w=which, n=ns: chain_qk(h + 1, w, n))
                      if h + 2 < HG:
                          sched[10].append(lambda: weights_dma(h + 2))
                  if h == 2:
                      sched[6].append(open_wop)
                  if h == 3:
                      for j in range(16):       # p3 gq0 at blocks 8-15
                          sched[8 + j // 2].append(
                              lambda mt=j: p3_mt(0, mt, insec=True))
                      for j in range(4):        # first p3 gq1 pieces
                          sched[14 + j // 2].append(
                              lambda mt=j: p3_mt(1, mt, insec=True))
              return sched

          def run_sched(sched, h, i):
              for fn in sched[i]:
                  fn()

          # ---- emit sections ----
          alloc_qk(0)
          weights_dma(1)
          wv_dmas("h")
          xl_dma(0)
          wv_dmas("l")
          xl_dma(1)
          for h in range(HG):
              sched = mk_sched(h)
              scores_head(h, lambda hh, i, sched=sched: run_sched(sched, hh, i))

          # ---- tail: rest of the output projection ----
          for mt in range(4, 16):
              p3_mt(1, mt, tail=True)
          sweep2_group(3, 3)
          for mt in range(16):
              p3_mt(2, mt, tail=True)
          for mt in range(16):
              p3_mt(3, mt, tail=True)
          tailp.close()
          phA.close()
          phB.close()
    nc.finalize()
    return nc


_NC_CACHE = {}


def _get_nc(reps=1):
    if reps not in _NC_CACHE:
        _NC_CACHE[reps] = build_nc(reps)
    return _NC_CACHE[reps]


def _rope_tables(position_ids_b):
    pos = position_ids_b.astype(np.float32)
    inv_freq = (1.0 / (ROPE_THETA ** (np.arange(0, DH, 2, dtype=np.float32) / np.float32(DH))))
    ang = pos[:, None] * inv_freq[None, :]          # [S, 64]
    emb = np.concatenate([ang, ang], axis=-1)       # [S, 128]
    cosT = np.ascontiguousarray(np.cos(emb).T) * np.float32(QSC)   # [128, S]
    sinT = np.sin(emb).T * np.float32(QSC)
    sin_rot = np.concatenate([-sinT[0:64], sinT[64:128]], axis=0)
    return cosT.astype(ml_dtypes.bfloat16), np.ascontiguousarray(sin_rot).astype(ml_dtypes.bfloat16)


def _make_in_maps(inputs):
    hidden_states = np.asarray(inputs["hidden_states"], dtype=np.float32)
    position_ids = np.asarray(inputs["position_ids"])
    Wqkv = np.asarray(inputs["Wqkv"], dtype=np.float32)
    bqkv = np.asarray(inputs["bqkv"], dtype=np.float32)
    Wo = np.asarray(inputs["Wo"], dtype=np.float32)

    mask = np.triu(np.ones((128, 128), dtype=np.float32)).astype(ml_dtypes.bfloat16)
    tones = np.full((128, 1), 1.0 / CSC, dtype=ml_dtypes.bfloat16)
    tabs = [_rope_tables(np.asarray(position_ids)[b]) for b in range(B)]

    def _hilo(M, sc):
        Ms = M * np.float32(sc)
        hi = Ms.astype(ml_dtypes.float8_e4m3)
        lo = (Ms - hi.astype(np.float32)).astype(ml_dtypes.float8_e4m3)
        return hi, lo

    def _pack_pairs(M):
        # [D, C] -> [8, 128, 2, C] with row r = kc2*256 + i*128 + p
        C = M.shape[1]
        return np.ascontiguousarray(M.reshape(8, 2, 128, C).transpose(0, 2, 1, 3))

    def _pack_x(M):
        # [D, S] -> [128, 4, 8, 2, SLAB] partition-major, slab-major free
        return np.ascontiguousarray(
            M.reshape(8, 2, 128, 4, SLAB).transpose(2, 3, 0, 1, 4))

    xts = []
    for b in range(B):
        hi, lo = _hilo(np.ascontiguousarray(hidden_states[b].T), XSC)
        xts.append((_pack_x(hi.astype(np.float32)).astype(ml_dtypes.float8_e4m3),
                    _pack_x(lo.astype(np.float32)).astype(ml_dtypes.float8_e4m3)))

    in_maps = []
    for c in range(NCORES):
        b, hg = divmod(c, HG)
        qcols = slice(hg * GQ, (hg + 1) * GQ)
        kcols = slice(D + hg * GQ, D + (hg + 1) * GQ)
        vcols = slice(2 * D + hg * GQ, 2 * D + (hg + 1) * GQ)
        wqk_c = np.ascontiguousarray(np.concatenate([Wqkv[:, qcols], Wqkv[:, kcols]], axis=1))
        qk_h, qk_l = _hilo(wqk_c, WSC)
        # per-mt packing: [8(mt), 128(p), 8(kc2), 2(i), 128(m)]
        def _pack_mt(M8):
            P = _pack_pairs(M8.astype(np.float32))          # [8, 128, 2, 1024]
            P = P.reshape(8, 128, 2, 8, 128)                 # [kc2, p, i, mt, m]
            return np.ascontiguousarray(P.transpose(3, 1, 0, 2, 4)).astype(ml_dtypes.float8_e4m3)
        wqkh_c = _pack_mt(qk_h)
        wqkl_c = _pack_mt(qk_l)
        wv_c = np.ascontiguousarray(Wqkv[:, vcols])
        v_h, v_l = _hilo(wv_c, WSC)
        wvh_c = _pack_pairs(v_h.astype(np.float32)).astype(ml_dtypes.float8_e4m3)
        wvl_c = _pack_pairs(v_l.astype(np.float32)).astype(ml_dtypes.float8_e4m3)
        # out-proj fp8 packs
        wo_c = np.ascontiguousarray(Wo[hg * GQ:(hg + 1) * GQ, :])   # [512, D]
        wo_h, wo_l = _hilo(wo_c, WSC)
        wo_h = wo_h.astype(np.float32)
        wo_l = wo_l.astype(np.float32)
        wo1_c = np.empty((HG, 128, 2, D), np.float32)
        for kh in range(HG):
            wo1_c[kh, :, 0, :] = wo_h[kh * 128:(kh + 1) * 128, :]
            wo1_c[kh, :, 1, :] = wo_l[kh * 128:(kh + 1) * 128, :]
        wo2_c = np.empty((2, 128, 2, D), np.float32)
        for gp in range(2):
            wo2_c[gp, :, 0, :] = wo_h[gp * 256:gp * 256 + 128, :]
            wo2_c[gp, :, 1, :] = wo_h[gp * 256 + 128:gp * 256 + 256, :]
        bqk_c = np.concatenate([bqkv[qcols], bqkv[kcols]]).reshape(8, 128).T
        bvc_c = np.ascontiguousarray(bqkv[vcols].reshape(HG, 128).T * np.float32(CSC))
        cosT, sin_rot = tabs[b]
        in_maps.append({
            "xh": xts[b][0], "xl": xts[b][1],
            "wqkh": wqkh_c, "wqkl": wqkl_c, "wvh": wvh_c, "wvl": wvl_c,
            "wo1": wo1_c.astype(ml_dtypes.float8_e4m3),
            "wo2": wo2_c.astype(ml_dtypes.float8_e4m3),
            "bqkt": np.ascontiguousarray(bqk_c),
            "bvc": bvc_c,
            "cost": cosT, "sinrt": sin_rot, "maskd": mask,
            "tonesd": tones,
        })
    return in_maps


def kernel(hidden_states, position_ids, Wqkv, bqkv, Wo, bo, _reps=1):
    bo = np.asarray(bo, dtype=np.float32)
    in_maps = _make_in_maps({
        "hidden_states": hidden_states, "position_ids": position_ids,
        "Wqkv": Wqkv, "bqkv": bqkv, "Wo": Wo, "bo": bo,
    })
    nc = _get_nc(_reps)
    res = run_bass_kernel_spmd(nc, in_maps, core_ids=list(range(NCORES)))

    out = np.empty((B, S, D), dtype=np.float32)
    for b in range(B):
        acc = res.results[b * HG]["outt"].reshape(D, S).astype(np.float32).copy()
        for hg in range(1, HG):
            acc += res.results[b * HG + hg]["outt"].reshape(D, S).astype(np.float32)
        out[b] = acc.T * np.float32(P3_SCALE) + bo[None, :]
    return out
